# revision 30
# baseline (speedup 1.0000x reference)
"""AttentionBlock Trainium2 Bass kernel.

Data-parallel over batch: 16 batches / 8 cores = 2 per core. Each core runs
the full block (groupnorm x2, q/kv projections, 8-head attention, output
projection, residual) on its 2 batch elements.

Key design points (v2):
- fp8e4m3 DoubleRow matmuls for all four projections (Wq/Wk prescaled x32 to
  escape fp8 denormals, compensated in the psum->sbuf bias add) and for the
  attention*V of st-pairs 1-3: 256-deep contraction at 0.5 cyc/row quarters
  projection PE time and AV PE time vs bf16.
- scores layout [s, t]; exp without max-subtraction but with a constant -4
  logit shift so exp output fits fp8 range (shift cancels in softmax).
- exp engine split: st-pairs 1-3 go to ACT (exp -> fp8 wts), st-pair 0 goes
  to DVE as a Schraudolph fast-exp (one tensor_scalar: i16(round(l*184.66 +
  15511.5)) bitcast bf16, ~3% multiplicative err, cancels mostly in softmax).
  Pair-0 wts/v tiles are bf16; AV for pair 0 runs as plain bf16 matmuls.
- wts/vT tiles hold an st-PAIR each ([128, 2, ...]) so DR matmuls can pair
  the contraction; softmax denominator rides along as a ones column of vT.
- normalize: one broadcast tensor_tensor per (head, th) scales psum by the
  per-partition reciprocal denominators -> aT bf16; PE transpose per channel
  block; psum->sbuf copy converts a to fp8 for the DR output projection.
- groupnorm applies run on GPSIMD (Pool) except the startup-critical batch-0
  pair; stats stay on DVE bn_stats (batch-0 x split ACT/DVE).
- software pipelining: AV matmuls trail their QK pair by one st-pair;
  projection/output matmuls are emitted as 1-matmul closures popped between
  attention slots; next-batch prep is pushed mid-ob.
- startup: y loads -> gn consts -> x loads -> biases -> fp8 weights; PE clock
  pre-warmed with dummy matmuls.
"""
import os
import sys

sys.path.insert(0, "/opt/trn_rl_repo")

import numpy as np

import concourse.bacc as bacc
import concourse.bass as bass
import concourse.tile as tile
from concourse import mybir
from concourse.bass_utils import run_bass_kernel_spmd

F32 = mybir.dt.float32
F32R = mybir.dt.float32r
BF16 = mybir.dt.bfloat16
FP8 = mybir.dt.float8e4
I16 = mybir.dt.int16
I32 = mybir.dt.int32
AF = mybir.ActivationFunctionType
OP = mybir.AluOpType
PM = mybir.MatmulPerfMode

B, C, H, W = 16, 512, 32, 32
T = H * W              # 1024
NH = 8                 # heads
CH = C // NH           # 64
GROUPS = 32
GSIZE = C // GROUPS    # 16 channels per group
EPS = 1e-5
N_CORES = 8
BPC = B // N_CORES     # batches per core
CB = C // 128          # 4 channel blocks
NT = T // 512          # 2 column halves of 512
ST = T // 128          # 8 seq tiles of 128
NP = ST // 2           # 4 st-pairs
WSCALE = 32.0          # Wq/Wk fp8 prescale (keeps weights out of denormals)
SHIFT = -4.0           # logit shift before exp (cancels in softmax)
SCH_A = 184.664375     # 2^7 / ln 2
SCH_C = 15511.5        # 16256 - 0.5 - 5.25 + SHIFT*SCH_A  (tuned Schraudolph)
BF16_PAIRS = (0, 1, 2, 3)   # all wts/v tiles bf16: even sts exp on ACT, odd
                            # sts Schraudolph on DVE — two independent psum
                            # chains so the engines never serialize on ps_s
ACT_ODD_HEADS = (3, 11)     # head-batches whose odd-st chain flips to ACT
                            # (fine engine-load balance)

DEBUG = bool(int(os.environ.get("KERNEL_DEBUG", "0")))


def _build():
    nc = bacc.Bacc(None, target_bir_lowering=False)

    x2 = nc.dram_tensor("x2", (BPC, C, T), F32, kind="ExternalInput")
    y2 = nc.dram_tensor("y2", (BPC, C, T), F32, kind="ExternalInput")
    wqt = nc.dram_tensor("wqt", (C, C), FP8, kind="ExternalInput")
    wkt = nc.dram_tensor("wkt", (C, C), FP8, kind="ExternalInput")
    wvt = nc.dram_tensor("wvt", (C, C), FP8, kind="ExternalInput")
    wpt = nc.dram_tensor("wpt", (C, C), FP8, kind="ExternalInput")
    bq_l = nc.dram_tensor("bq_l", (128, CB), F32, kind="ExternalInput")
    bk_l = nc.dram_tensor("bk_l", (128, CB), F32, kind="ExternalInput")
    bp_l = nc.dram_tensor("bp_l", (128, CB), F32, kind="ExternalInput")
    bv_bc = nc.dram_tensor("bv_bc", (128, NH, CH), F32, kind="ExternalInput")
    gnw_l = nc.dram_tensor("gnw_l", (128, CB), F32, kind="ExternalInput")
    gnb_l = nc.dram_tensor("gnb_l", (128, CB), F32, kind="ExternalInput")
    m1 = nc.dram_tensor("m1", (128, 128), F32, kind="ExternalInput")
    id128 = nc.dram_tensor("id128", (128, 128), BF16, kind="ExternalInput")
    out_d = nc.dram_tensor("out", (BPC, C, T), F32, kind="ExternalOutput")
    if DEBUG:
        dbg_a = nc.dram_tensor("dbg_a", (C, T), F32, kind="ExternalOutput")

    with tile.TileContext(nc) as tc:
        from contextlib import ExitStack
        with ExitStack() as ctx:
            consts = ctx.enter_context(tc.tile_pool(name="consts", bufs=1))
            px = ctx.enter_context(tc.tile_pool(name="px", bufs=2))
            py = ctx.enter_context(tc.tile_pool(name="py", bufs=1))
            pgn = ctx.enter_context(tc.tile_pool(name="pgn", bufs=2))
            pk = ctx.enter_context(tc.tile_pool(name="pk", bufs=4))
            pvt = ctx.enter_context(tc.tile_pool(name="pvt", bufs=int(os.environ.get("KPVT", 2 + NP))))
            pq = ctx.enter_context(tc.tile_pool(name="pq", bufs=4))
            pwts = ctx.enter_context(tc.tile_pool(name="pwts", bufs=int(os.environ.get("KPWTS", 8))))
            pa = ctx.enter_context(tc.tile_pool(name="pa", bufs=2))
            pat = ctx.enter_context(tc.tile_pool(name="pat", bufs=2))
            pdn = ctx.enter_context(tc.tile_pool(name="pdn", bufs=1))
            pdn3 = ctx.enter_context(tc.tile_pool(name="pdn3", bufs=2))
            pst = ctx.enter_context(tc.tile_pool(name="pst", bufs=4))
            ps_mm = ctx.enter_context(tc.tile_pool(name="ps_mm", bufs=2, space="PSUM"))
            ps_s = ctx.enter_context(tc.tile_pool(name="ps_s", bufs=2, space="PSUM"))
            ps_a0 = ctx.enter_context(tc.tile_pool(name="ps_a0", bufs=1, space="PSUM"))
            ps_a1 = ctx.enter_context(tc.tile_pool(name="ps_a1", bufs=1, space="PSUM"))

            # --- constants (weights fp8; DMAs ordered for startup overlap) ---
            wq_sb = consts.tile([128, CB, C], FP8, tag="wq")
            wk_sb = consts.tile([128, CB, C], FP8, tag="wk")
            wv_sb = consts.tile([128, CB, C], FP8, tag="wv")
            wp_sb = consts.tile([128, CB, C], FP8, tag="wp")

            def emit_vp_weight_loads():
                nc.sync.dma_start(out=wv_sb, in_=wvt.rearrange("(kb p) o -> p kb o", p=128))
                nc.sync.dma_start(out=wp_sb, in_=wpt.rearrange("(kb p) o -> p kb o", p=128))

            m1_sb = consts.tile([128, 128], F32, tag="m1")
            bq_sb = consts.tile([128, CB], F32, tag="bq")
            bk_sb = consts.tile([128, CB], F32, tag="bk")
            bp_sb = consts.tile([128, CB], F32, tag="bp")
            bv_sb = consts.tile([128, NH, CH], F32, tag="bv")
            gnw_sb = consts.tile([128, CB], F32, tag="gnw")
            gnb_sb = consts.tile([128, CB], F32, tag="gnb")
            id_sb = consts.tile([128, 128], BF16, tag="id")
            magic_sb = consts.tile([128, CB], I32, tag="magic")
            nc.vector.memset(magic_sb, 0x5f3759df)
            ones8_sb = consts.tile([128, NH], FP8, tag="ones8")
            nc.vector.memset(ones8_sb, 1.0)
            ones16_sb = consts.tile([128, NH], BF16, tag="ones16")
            nc.vector.memset(ones16_sb, 1.0)
            nbias_sb = consts.tile([128, 1], F32, tag="nbias")
            nc.vector.memset(nbias_sb, SHIFT)
            warm = consts.tile([1, 1], F32, tag="warm")
            nc.vector.memset(warm, 0.0)
            nc.scalar.activation(out=warm, in_=warm, func=AF.Exp)

            def emit_gn_consts():
                nc.sync.dma_start(out=m1_sb, in_=m1[:, :])
                nc.sync.dma_start(out=gnw_sb, in_=gnw_l[:, :])
                nc.sync.dma_start(out=gnb_sb, in_=gnb_l[:, :])

            def emit_bias_consts():
                nc.sync.dma_start(out=bk_sb, in_=bk_l[:, :])
                nc.sync.dma_start(out=bq_sb, in_=bq_l[:, :])
                nc.sync.dma_start(out=bv_sb, in_=bv_bc[:, :, :])
                nc.sync.dma_start(out=bp_sb, in_=bp_l[:, :])
                nc.sync.dma_start(out=id_sb, in_=id128[:, :])

            sched_state = {"sch_i": 0}

            def groupnorm(src_sb, dst_fn, fast_apply=False, act_stats=False):
                """src_sb: [128, CB, T] f32. dst_fn(cb, th)->AP (fp8 out)."""
                mv = pst.tile([128, CB, 2], F32, tag="mv")
                if act_stats:
                    # split stats: ACT (Copy/Square accum) covers cb0-1 while
                    # DVE bn_stats covers cb2-3 — halves the serial latency on
                    # the startup-critical tensor.
                    part = pst.tile([128, 2, 2, 2], F32, tag="part")
                    for si, (func, scale) in enumerate(
                            ((AF.Copy, 1.0 / T), (AF.Square, 1.0 / np.sqrt(T)))):
                        for cb in range(2):
                            for c2 in range(2):
                                trash = pwts.tile([128, 512], BF16, tag="trash", name="trash")
                                nc.scalar.activation(
                                    out=trash, in_=src_sb[:, cb, c2 * 512:(c2 + 1) * 512],
                                    func=func, scale=scale,
                                    accum_out=part[:, cb, si, c2:c2 + 1])
                    stats6b = pst.tile([128, 2, 6], F32, tag="stats6b")
                    for cb in (2, 3):
                        for c2 in range(2):
                            nc.vector.bn_stats(
                                out=stats6b[:, c2, :],
                                in_=src_sb[:, cb, c2 * 512:(c2 + 1) * 512])
                        nc.vector.bn_aggr(out=mv[:, cb, :], in_=stats6b)
                    # cb0-1: mv = (mean, E[x^2]) from the two half-col accums
                    nc.vector.tensor_tensor(
                        out=mv[:, 0:2, :].rearrange("p a b -> p (a b)"),
                        in0=part[:, :, :, 0].rearrange("p a b -> p (a b)"),
                        in1=part[:, :, :, 1].rearrange("p a b -> p (a b)"), op=OP.add)
                    # cb2-3: convert var -> E[x^2] in place
                    musqb = pst.tile([128, 2], F32, tag="musqb")
                    nc.vector.tensor_tensor(out=musqb, in0=mv[:, 2:4, 0], in1=mv[:, 2:4, 0], op=OP.mult)
                    nc.vector.tensor_tensor(out=mv[:, 2:4, 1], in0=musqb, in1=mv[:, 2:4, 1], op=OP.add)
                else:
                    stats6 = pst.tile([128, 2, 6], F32, tag="stats6")
                    for cb in range(CB):
                        for c2 in range(2):
                            nc.vector.bn_stats(
                                out=stats6[:, c2, :],
                                in_=src_sb[:, cb, c2 * 512:(c2 + 1) * 512])
                        nc.vector.bn_aggr(out=mv[:, cb, :], in_=stats6)
                    # m2 slot in-place: mv[:,:,1] = var + mean^2
                    musq = pst.tile([128, 4], F32, tag="musq")
                    nc.vector.tensor_tensor(out=musq, in0=mv[:, :, 0], in1=mv[:, :, 0], op=OP.mult)
                    nc.vector.tensor_tensor(out=mv[:, :, 1], in0=musq, in1=mv[:, :, 1], op=OP.add)
                psg = ps_s.tile([128, 8], F32, tag="sc", name="psg")
                nc.tensor.matmul(psg, m1_sb, mv.rearrange("p a b -> p (a b)"), start=True, stop=True)
                gsb = pst.tile([128, 8], F32, tag="gsb")
                nc.vector.tensor_copy(gsb, psg)  # m1 carries 1/GSIZE; cols interleaved (mean, m2)
                # var + eps = (m2 + eps) - mean^2, fused
                tmp4 = pst.tile([128, 4], F32, tag="tmp4")
                nc.vector.tensor_tensor(out=tmp4, in0=gsb[:, 0::2], in1=gsb[:, 0::2], op=OP.mult)
                vv = pst.tile([128, 4], F32, tag="vv")
                nc.vector.scalar_tensor_tensor(
                    out=vv, in0=gsb[:, 1::2], scalar=EPS, in1=tmp4,
                    op0=OP.add, op1=OP.subtract)
                # rstd = rsqrt(vv): quake seed + 1 Newton step (3 fused ops)
                bsh = pst.tile([128, 4], I32, tag="bsh")
                nc.vector.tensor_scalar(
                    out=bsh, in0=vv.bitcast(I32), scalar1=1, scalar2=None,
                    op0=OP.logical_shift_right)
                nc.vector.tensor_tensor(out=tmp4.bitcast(I32), in0=magic_sb, in1=bsh, op=OP.subtract)
                nrt = pst.tile([128, 4], F32, tag="nrt")
                for _ in range(1):
                    nc.vector.tensor_tensor(out=nrt, in0=tmp4, in1=tmp4, op=OP.mult)
                    nc.vector.scalar_tensor_tensor(
                        out=nrt, in0=nrt, scalar=-0.5, in1=vv, op0=OP.mult, op1=OP.mult)
                    nc.vector.scalar_tensor_tensor(
                        out=tmp4, in0=nrt, scalar=1.5, in1=tmp4, op0=OP.add, op1=OP.mult)
                ab = pst.tile([128, 8], F32, tag="ab")
                nc.vector.tensor_tensor(out=ab[:, 0:4], in0=tmp4, in1=gnw_sb, op=OP.mult)
                tmp4b = pst.tile([128, 4], F32, tag="tmp4b")
                nc.vector.tensor_tensor(out=tmp4b, in0=gsb[:, 0::2], in1=ab[:, 0:4], op=OP.mult)
                nc.vector.tensor_tensor(out=ab[:, 4:8], in0=gnb_sb, in1=tmp4b, op=OP.subtract)
                for th in range(NT):
                    for cb in range(CB):
                        if os.environ.get("KERNEL_BASE_GN") == "1":
                            eng = nc.gpsimd if (cb % 2 == 1 and not fast_apply) else nc.vector
                            eng.tensor_scalar(
                                out=dst_fn(cb, th), in0=src_sb[:, cb, th * 512:(th + 1) * 512],
                                scalar1=ab[:, cb:cb + 1], scalar2=ab[:, 4 + cb:5 + cb],
                                op0=OP.mult, op1=OP.add)
                        elif fast_apply:
                            # startup-critical: ACT is idle here and faster
                            nc.scalar.activation(
                                out=dst_fn(cb, th),
                                in_=src_sb[:, cb, th * 512:(th + 1) * 512],
                                func=AF.Identity, bias=ab[:, 4 + cb:5 + cb],
                                scale=ab[:, cb:cb + 1])
                        else:
                            nc.gpsimd.tensor_scalar(
                                out=dst_fn(cb, th), in0=src_sb[:, cb, th * 512:(th + 1) * 512],
                                scalar1=ab[:, cb:cb + 1], scalar2=ab[:, 4 + cb:5 + cb],
                                op0=OP.mult, op1=OP.add)

            def emit_input_loads(b, mid_fn=None):
                y_sb = py.tile([128, CB, T], F32, tag="y")
                for cb in range(CB):
                    nc.sync.dma_start(
                        out=y_sb[:, cb, :],
                        in_=y2[b].rearrange("(cb p) t -> p cb t", p=128)[:, cb, :])
                if mid_fn is not None:
                    mid_fn()  # gn consts + k/q weights jump the queue ahead of x
                x_sb = px.tile([128, CB, T], F32, tag="x")
                for cb in range(CB):
                    nc.sync.dma_start(
                        out=x_sb[:, cb, :],
                        in_=x2[b].rearrange("(cb p) t -> p cb t", p=128)[:, cb, :])
                return x_sb, y_sb

            def emit_gn_compute(x_sb, y_sb, fast_apply=False, act_stats=False):
                gny = pgn.tile([128, CB, T], FP8, tag="gn")
                groupnorm(y_sb, lambda cb, th: gny[:, cb, th * 512:(th + 1) * 512], fast_apply)
                gnx = pgn.tile([128, CB, T], FP8, tag="gn")
                groupnorm(x_sb, lambda cb, th: gnx[:, cb, th * 512:(th + 1) * 512], fast_apply,
                          act_stats=act_stats)
                return gnx, gny

            def q_unit(bctx, ob, pool=None):
                """Closure list: 4 DR matmuls computing q for one ob (2 heads)."""
                st8 = {}
                def mk(th, j):
                    def f():
                        gnx = bctx["gn"][0]
                        if "qp" not in st8:
                            qp0 = pq.tile([128, T], BF16, tag="qpad")
                            qp1 = pq.tile([128, T], BF16, tag="qpad")
                            nc.gpsimd.memset(qp0[64:128, :].bitcast(F32), 0.0)
                            nc.gpsimd.memset(qp1[0:64, :].bitcast(F32), 0.0)
                            st8["qp"] = (qp0, qp1)
                            bctx.setdefault("qps", {})[ob] = (qp0, qp1)
                        qp0, qp1 = st8["qp"]
                        if j == 0:
                            st8[th] = (ps_mm.tile([128, 512], F32, tag="mm", name="psu")
                                       if pool is None else
                                       pool.tile([128, 512], F32, tag="sc", name="psu"))
                        psq = st8[th]
                        nc.tensor.matmul(
                            psq,
                            wq_sb[:, 2 * j:2 * j + 2, ob * 128:(ob + 1) * 128],
                            gnx[:, 2 * j:2 * j + 2, th * 512:(th + 1) * 512],
                            start=(j == 0), stop=(j == 1), perf_mode=PM.DoubleRow)
                        if j == 1:
                            if os.environ.get("KERNEL_DVE_BIAS") == "1":
                                nc.vector.tensor_scalar(
                                    out=qp0[0:64, th * 512:(th + 1) * 512],
                                    in0=psq[0:64, :], scalar1=1.0 / WSCALE,
                                    scalar2=bq_sb[0:64, ob:ob + 1],
                                    op0=OP.mult, op1=OP.add)
                                nc.vector.tensor_scalar(
                                    out=qp1[64:128, th * 512:(th + 1) * 512],
                                    in0=psq[64:128, :], scalar1=1.0 / WSCALE,
                                    scalar2=bq_sb[64:128, ob:ob + 1],
                                    op0=OP.mult, op1=OP.add)
                            else:
                                nc.scalar.activation(
                                    out=qp0[0:64, th * 512:(th + 1) * 512],
                                    in_=psq[0:64, :], func=AF.Identity,
                                    bias=bq_sb[0:64, ob:ob + 1], scale=1.0 / WSCALE)
                                nc.scalar.activation(
                                    out=qp1[64:128, th * 512:(th + 1) * 512],
                                    in_=psq[64:128, :], func=AF.Identity,
                                    bias=bq_sb[64:128, ob:ob + 1], scale=1.0 / WSCALE)
                    return f
                return [mk(th, j) for th in range(NT) for j in range(2)]

            def k_unit(bctx, ob, pool=None):
                st8 = {}
                def mk(th, j):
                    def f():
                        gny = bctx["gn"][1]
                        if "t" not in st8:
                            st8["t"] = pk.tile([128, T], BF16, tag="k", name="k_ob")
                            bctx["k"][ob] = st8["t"]
                        if j == 0:
                            st8[th] = (ps_mm.tile([128, 512], F32, tag="mm", name="psu")
                                       if pool is None else
                                       pool.tile([128, 512], F32, tag="sc", name="psu"))
                        psk = st8[th]
                        nc.tensor.matmul(
                            psk,
                            wk_sb[:, 2 * j:2 * j + 2, ob * 128:(ob + 1) * 128],
                            gny[:, 2 * j:2 * j + 2, th * 512:(th + 1) * 512],
                            start=(j == 0), stop=(j == 1), perf_mode=PM.DoubleRow)
                        if j == 1:
                            if os.environ.get("KERNEL_DVE_BIAS") == "1":
                                nc.vector.tensor_scalar(
                                    out=st8["t"][:, th * 512:(th + 1) * 512],
                                    in0=psk, scalar1=1.0 / WSCALE,
                                    scalar2=bk_sb[:, ob:ob + 1], op0=OP.mult, op1=OP.add)
                            else:
                                nc.scalar.activation(
                                    out=st8["t"][:, th * 512:(th + 1) * 512],
                                    in_=psk, func=AF.Identity,
                                    bias=bk_sb[:, ob:ob + 1], scale=1.0 / WSCALE)
                    return f
                return [mk(th, j) for th in range(NT) for j in range(2)]

            def vt_unit(bctx, tt):
                """Closure list: 2 DR matmuls + bias for one vT seq tile.

                vT tiles hold an st-PAIR: [128, 2, NH, CH+1] bf16."""
                st8 = {}
                pair, sub = tt // 2, tt % 2
                def mk(j):
                    def f():
                        gny = bctx["gn"][1]
                        if j == 0:
                            st8["ps"] = ps_mm.tile([128, 512], F32, tag="mm", name="psu")
                        psv = st8["ps"]
                        nc.tensor.matmul(
                            psv,
                            gny[:, 2 * j:2 * j + 2, tt * 128:(tt + 1) * 128],
                            wv_sb[:, 2 * j:2 * j + 2, :],
                            start=(j == 0), stop=(j == 1), perf_mode=PM.DoubleRow)
                        if j == 1:
                            if bctx["vtp"][pair] is None:
                                bctx["vtp"][pair] = pvt.tile(
                                    [128, 2, NH, CH + 1], BF16, tag="vt", name="vt")
                            vt = bctx["vtp"][pair]
                            nc.vector.tensor_tensor(
                                out=vt[:, sub, :, 0:CH],
                                in0=psv.rearrange("p (h c) -> p h c", h=NH),
                                in1=bv_sb, op=OP.add)
                            nc.vector.tensor_copy(
                                vt[:, sub, :, CH:CH + 1],
                                ones16_sb.rearrange("p (h o) -> p h o", o=1))
                    return f
                return [mk(j) for j in range(2)]

            def pproj_unit(bctx, b, ob, pool=None, split_tail=False, store_q=None,
                           pool_tag="sc", act_resid=False):
                """Closure list: 4 DR matmuls + bias/residual/store for one out block.

                split_tail: j0 group (kb0-1, residual applied early), then a
                separate j1 group + final add + store — shortens the critical
                chain behind the last attention block.
                """
                st8 = {}
                def mk(th, j):
                    def f():
                        xr = bctx["x"]
                        a_sb = bctx["a"]
                        sl = slice(th * 512, (th + 1) * 512)
                        if j == 0 or split_tail:
                            st8[th] = (ps_mm.tile([128, 512], F32, tag="mm", name="psu")
                                       if pool is None else
                                       pool.tile([128, 512], F32, tag=pool_tag, name="psu"))
                        psh = st8[th]
                        nc.tensor.matmul(
                            psh,
                            wp_sb[:, 2 * j:2 * j + 2, ob * 128:(ob + 1) * 128],
                            a_sb[:, 2 * j:2 * j + 2, sl],
                            start=(j == 0 or split_tail),
                            stop=(j == 1 or split_tail),
                            perf_mode=PM.DoubleRow)
                        grp_end = (j == 0) if split_tail else (j == 1)
                        if grp_end:
                            nc.vector.scalar_tensor_tensor(
                                out=xr[:, ob, sl],
                                in0=psh, scalar=bp_sb[:, ob:ob + 1],
                                in1=xr[:, ob, sl], op0=OP.add, op1=OP.add)
                        if j == 1:
                            if split_tail:
                                nc.vector.tensor_tensor(
                                    out=xr[:, ob, sl], in0=psh, in1=xr[:, ob, sl],
                                    op=OP.add)
                            (store_q or nc.sync).dma_start(
                                out=out_d[b].rearrange("(cb p) t -> p cb t", p=128)[:, ob, sl],
                                in_=xr[:, ob, sl])
                    return f
                return [mk(th, j) for th in range(NT) for j in range(2)]

            def attention_head(bctx, ob, hh, qp, a_sb, lazy_vt=False, filler=None):
                h = 2 * ob + hh
                k_ob = bctx["k"][ob]
                vtp = bctx["vtp"]
                psa_t = [ps_a0.tile([128, 512], F32, tag="av0", name="psa0"),
                         ps_a1.tile([128, 512], F32, tag="av1", name="psa1")]
                psa = [t[:, 0:4 * (CH + 1)].rearrange("p (a b) -> p a b", b=CH + 1)
                       for t in psa_t]

                def emit_avs(pair, wts):
                    first = pair == 0
                    last = pair == NP - 1
                    for th in range(NT):
                        for sub in range(2):
                            for tc4 in range(4):
                                nc.tensor.matmul(
                                    psa[th][:, tc4, :],
                                    wts[:, sub, th, tc4 * 128:(tc4 + 1) * 128],
                                    vtp[pair][:, sub, h, :],
                                    start=(first and sub == 0 and tc4 == 0),
                                    stop=(last and sub == 1 and tc4 == 3),
                                    skip_group_check=True)
                        if filler is not None:
                            filler()
                            filler()

                prev = None
                hg = bctx["b"] * NH + h
                for pair in range(NP):
                    if lazy_vt and vtp[pair] is None:
                        for tt in (2 * pair, 2 * pair + 1):
                            for f in vt_unit(bctx, tt):
                                f()
                    wts = pwts.tile([128, 2, NT, 512], BF16, tag="wts")
                    for sub in range(2):
                        st = 2 * pair + sub
                        pss = ps_s.tile([128, NT, 512], F32, tag="sc")
                        for th in range(NT):
                            nc.tensor.matmul(
                                pss[:, th, :],
                                k_ob[:, st * 128:(st + 1) * 128],
                                qp[:, th * 512:(th + 1) * 512],
                                start=True, stop=True)
                            if filler is not None:
                                filler()
                        # exp engines alternate by st parity: even sts on ACT
                        # (exact exp), odd sts Schraudolph fast-exp on DVE —
                        # each engine owns one of the two ps_s buffer chains
                        if sub == 1 and hg not in ACT_ODD_HEADS:
                            nc.vector.tensor_scalar(
                                out=wts.bitcast(I16)[:, sub].rearrange("p a b -> p (a b)"),
                                in0=pss.rearrange("p a b -> p (a b)"),
                                scalar1=SCH_A, scalar2=SCH_C,
                                op0=OP.mult, op1=OP.add)
                        else:
                            nc.scalar.activation(
                                out=wts[:, sub].rearrange("p a b -> p (a b)"),
                                in_=pss.rearrange("p a b -> p (a b)"),
                                func=AF.Exp, bias=nbias_sb, scale=1.0)
                    # AVs run one pair behind: the next QK (exp input) stays
                    # at the head of the in-order PE queue
                    if prev is not None:
                        emit_avs(*prev)
                    prev = (pair, wts)
                emit_avs(*prev)
                finish_head(bctx, ob, hh, psa, a_sb)

            def finish_head(bctx, ob, hh, psa, a_sb):
                # denominators are per-partition columns now: copy+recip+scale.
                # high priority: these free the single-buffered AV psum banks,
                # so they must win DVE scheduling ties against filler ops.
                with tc.high_priority():
                    _finish_head(bctx, ob, hh, psa, a_sb)

            def _finish_head(bctx, ob, hh, psa, a_sb):
                h = 2 * ob + hh
                aT = bctx["aT"]
                for th in range(NT):
                    rr = pdn3.tile([128, 4], F32, tag="dn3")
                    if os.environ.get("KERNEL_DN_COPY") == "1":
                        dn = pdn.tile([128, 4], F32, tag="r0")
                        nc.vector.tensor_copy(dn, psa[th][:, :, CH])
                        nc.vector.reciprocal_approx_fast(out=rr, in_=dn)
                    else:
                        nc.vector.reciprocal_approx_fast(out=rr, in_=psa[th][:, :, CH])
                    nc.vector.tensor_tensor(
                        out=aT[:, th, :, h, :],
                        in0=psa[th][:, :, 0:CH],
                        in1=rr[:, :, None].broadcast_to([128, 4, CH]),
                        op=OP.mult)

            def finish_ob(bctx, ob, a_sb):
                """Transpose aT[t, c] blocks of channel-block ob back to a[c, t]."""
                aT = bctx["aT"]
                trp = ps_mm.tile([128, ST, 128], BF16, tag="mm", name="trp")
                for tb in range(ST):
                    th, tc4 = tb // 4, tb % 4
                    nc.tensor.matmul(
                        trp[:, tb, :],
                        aT[:, th, tc4, 2 * ob:2 * ob + 2, :].rearrange("p a b -> p (a b)"),
                        id_sb,
                        is_transpose=True)
                nc.scalar.activation(out=a_sb[:, ob, :],
                                     in_=trp.rearrange("p a b -> p (a b)"),
                                     func=AF.Copy)

            # ---------------- batch pipeline ----------------
            from collections import deque
            fillers = deque()

            def filler_pop():
                if fillers:
                    fillers.popleft()()

            def filler_flush():
                while fillers:
                    fillers.popleft()()

            bctxs = [dict() for _ in range(BPC)]
            _xy0 = emit_input_loads(0, mid_fn=emit_gn_consts)
            for wi in range(20):
                trash = ps_mm.tile([128, 64], F32, tag="mm", name="wtr0")
                nc.tensor.matmul(trash, _xy0[0][:, 0, 0:128], _xy0[0][:, 0, 0:64],
                                 start=True, stop=True)
            emit_bias_consts()
            nc.sync.dma_start(out=wq_sb, in_=wqt.rearrange("(kb p) o -> p kb o", p=128))
            nc.sync.dma_start(out=wk_sb, in_=wkt.rearrange("(kb p) o -> p kb o", p=128))
            emit_vp_weight_loads()
            bctxs[0]["x"] = _xy0[0]
            bctxs[0]["gn"] = emit_gn_compute(*_xy0, fast_apply=True, act_stats=True)
            bctxs[0]["k"] = [None] * CB
            bctxs[0]["vtp"] = [None] * NP

            # batch 0 ob0 prep emitted directly; rest queued as fillers that
            # drip into the attention pair slots (1 matmul per QK/AV pair)
            for f in k_unit(bctxs[0], 0, pool=ps_s):
                f()
            for f in q_unit(bctxs[0], 0, pool=ps_s):
                f()
            for ob2 in (1, 2, 3):
                fillers.extend(k_unit(bctxs[0], ob2))
                fillers.extend(q_unit(bctxs[0], ob2))

            for b in range(BPC):
                bctx = bctxs[b]
                bctx["b"] = b
                a_sb = pa.tile([128, CB, T], FP8, tag="a")
                bctx["a"] = a_sb
                bctx["aT"] = pat.tile([128, NT, 4, NH, CH], BF16, tag="aT", name="aT")
                for ob in range(CB):
                    if b > 0:
                        if ob == 0:
                            for ob2 in (1, 2):
                                fillers.extend(k_unit(bctx, ob2))
                                fillers.extend(q_unit(bctx, ob2))
                        if ob == 1:
                            fillers.extend(k_unit(bctx, 3))
                            fillers.extend(q_unit(bctx, 3))
                            for ob2 in range(CB):
                                fillers.extend(pproj_unit(bctxs[b - 1], b - 1, ob2, pool=ps_s))
                    if b + 1 < BPC:
                        if ob == 0:
                            nb = bctxs[b + 1]
                            nb["xy"] = emit_input_loads(b + 1)
                            nb["x"] = nb["xy"][0]
                        if ob == 1:
                            nb = bctxs[b + 1]
                            nb["gn"] = emit_gn_compute(*nb.pop("xy"))
                            nb["k"] = [None] * CB
                            nb["vtp"] = [None] * NP

                    qp0, qp1 = bctx["qps"].pop(ob)
                    for hh in (0, 1):
                        attention_head(bctx, ob, hh, (qp0, qp1)[hh], a_sb,
                                       lazy_vt=(b == 0 and ob == 0),
                                       filler=filler_pop)
                        if hh == 0 and b + 1 < BPC:
                            nb = bctxs[b + 1]
                            if ob == 2:
                                fillers.extend(k_unit(nb, 0))
                                for tt in range(ST):
                                    fillers.extend(vt_unit(nb, tt))
                            if ob == 3:
                                fillers.extend(q_unit(nb, 0))
                    if ob > 0:
                        finish_ob(bctx, ob - 1, a_sb)
                    if ob == CB - 1:
                        finish_ob(bctx, ob, a_sb)

                if b == BPC - 1:
                    # tail: drain queue, then final output projection directly.
                    filler_flush()
                    # dummy matmuls keep the PE clock ramped while the last
                    # head's softmax-normalize chain drains
                    for wi in range(10):
                        trash = ps_mm.tile([128, 512], F32, tag="mm", name="wtr")
                        nc.tensor.matmul(trash, wp_sb[:, 0, 0:128],
                                         a_sb[:, 0, 0:512],
                                         start=True, stop=True)
                    tail_pools = [(ps_a0, "av0"), (ps_a1, "av1"), (ps_s, "sc"), (None, "sc")]
                    for ob2 in range(CB):
                        pl, tg = tail_pools[ob2]
                        for f in pproj_unit(bctx, b, ob2, pool=pl, pool_tag=tg,
                                            store_q=nc.scalar if ob2 % 2 == 0 else nc.sync):
                            f()

    nc.finalize()
    return nc


_NC = None


def _get_nc():
    global _NC
    if _NC is None:
        _NC = _build()
    return _NC


def _prep_inputs(x, y, gn_w, gn_b, Wq, bq, Wkv, bkv, Wp, bp):
    scale = CH ** -0.25
    # reference splits k/v per head: kvh[:, h, :ch] / kvh[:, h, ch:] after
    # reshape to [b, NH, 2*ch, T] -> k_h = Wkv rows [h*128, h*128+64)
    import ml_dtypes
    FP8NP = ml_dtypes.float8_e4m3
    idx_k = np.concatenate([np.arange(h * 2 * CH, h * 2 * CH + CH) for h in range(NH)])
    idx_v = np.concatenate([np.arange(h * 2 * CH + CH, (h + 1) * 2 * CH) for h in range(NH)])
    # Wq/Wk prescaled by WSCALE to keep fp8 values out of denormal range;
    # compensated by 1/WSCALE in the psum->sbuf bias add.
    wqt = np.ascontiguousarray((Wq * (scale * WSCALE)).T).astype(FP8NP)
    wkt = np.ascontiguousarray((Wkv[idx_k] * (scale * WSCALE)).T).astype(FP8NP)
    wvt = np.ascontiguousarray(Wkv[idx_v].T).astype(FP8NP)
    wpt = np.ascontiguousarray(Wp.T).astype(FP8NP)
    bq_s = bq * scale
    bk_s = bkv[idx_k] * scale
    bv = bkv[idx_v]

    def part_layout(v):  # [C] -> [128, CB]: v[cb*128+p]
        return np.ascontiguousarray(v.reshape(CB, 128).T)

    bq_l = part_layout(bq_s)
    bk_l = part_layout(bk_s)
    bp_l = part_layout(bp)
    gnw_l = part_layout(gn_w)
    gnb_l = part_layout(gn_b)
    bv_bc = np.broadcast_to(bv.reshape(1, NH, CH), (128, NH, CH)).copy()
    m1 = np.zeros((128, 128), np.float32)
    for g in range(128 // GSIZE):
        m1[g * GSIZE:(g + 1) * GSIZE, g * GSIZE:(g + 1) * GSIZE] = 1.0 / GSIZE
    id128_h = np.eye(128, dtype=ml_dtypes.bfloat16)

    xf = x.reshape(B, C, T)
    yf = y.reshape(B, C, T)

    shared = {
        "wqt": wqt, "wkt": wkt, "wvt": wvt, "wpt": wpt,
        "bq_l": bq_l, "bk_l": bk_l, "bp_l": bp_l, "bv_bc": bv_bc,
        "gnw_l": gnw_l, "gnb_l": gnb_l, "m1": m1, "id128": id128_h,
    }
    in_maps = []
    for i in range(N_CORES):
        m = dict(shared)
        m["x2"] = np.ascontiguousarray(xf[i * BPC:(i + 1) * BPC])
        m["y2"] = np.ascontiguousarray(yf[i * BPC:(i + 1) * BPC])
        in_maps.append(m)
    return in_maps


def kernel(x, y, gn_w, gn_b, Wq, bq, Wkv, bkv, Wp, bp):
    args = [np.asarray(a, dtype=np.float32) for a in
            (x, y, gn_w, gn_b, Wq, bq, Wkv, bkv, Wp, bp)]
    in_maps = _prep_inputs(*args)
    nc = _get_nc()
    res = run_bass_kernel_spmd(nc, in_maps, core_ids=list(range(N_CORES)))
    out = np.empty((B, C, T), np.float32)
    for i in range(N_CORES):
        out[i * BPC:(i + 1) * BPC] = res.results[i]["out"]
    return out.reshape(B, C, H, W)


# revision 32
# speedup vs baseline: 1.0185x; 1.0185x over previous
"""AttentionBlock Trainium2 Bass kernel.

Data-parallel over batch: 16 batches / 8 cores = 2 per core. Each core runs
the full block (groupnorm x2, q/kv projections, 8-head attention, output
projection, residual) on its 2 batch elements.

Key design points (v2):
- fp8e4m3 DoubleRow matmuls for all four projections (Wq/Wk prescaled x32 to
  escape fp8 denormals, compensated in the psum->sbuf bias add) and for the
  attention*V of st-pairs 1-3: 256-deep contraction at 0.5 cyc/row quarters
  projection PE time and AV PE time vs bf16.
- scores layout [s, t]; exp without max-subtraction but with a constant -4
  logit shift so exp output fits fp8 range (shift cancels in softmax).
- exp engine split: st-pairs 1-3 go to ACT (exp -> fp8 wts), st-pair 0 goes
  to DVE as a Schraudolph fast-exp (one tensor_scalar: i16(round(l*184.66 +
  15511.5)) bitcast bf16, ~3% multiplicative err, cancels mostly in softmax).
  Pair-0 wts/v tiles are bf16; AV for pair 0 runs as plain bf16 matmuls.
- wts/vT tiles hold an st-PAIR each ([128, 2, ...]) so DR matmuls can pair
  the contraction; softmax denominator rides along as a ones column of vT.
- normalize: one broadcast tensor_tensor per (head, th) scales psum by the
  per-partition reciprocal denominators -> aT bf16; PE transpose per channel
  block; psum->sbuf copy converts a to fp8 for the DR output projection.
- groupnorm applies run on GPSIMD (Pool) except the startup-critical batch-0
  pair; stats stay on DVE bn_stats (batch-0 x split ACT/DVE).
- software pipelining: AV matmuls trail their QK pair by one st-pair;
  projection/output matmuls are emitted as 1-matmul closures popped between
  attention slots; next-batch prep is pushed mid-ob.
- startup: y loads -> gn consts -> x loads -> biases -> fp8 weights; PE clock
  pre-warmed with dummy matmuls.
"""
import os
import sys

sys.path.insert(0, "/opt/trn_rl_repo")

import numpy as np

import concourse.bacc as bacc
import concourse.bass as bass
import concourse.tile as tile
from concourse import mybir
from concourse.bass_utils import run_bass_kernel_spmd

F32 = mybir.dt.float32
F32R = mybir.dt.float32r
BF16 = mybir.dt.bfloat16
FP8 = mybir.dt.float8e4
I16 = mybir.dt.int16
I32 = mybir.dt.int32
AF = mybir.ActivationFunctionType
OP = mybir.AluOpType
PM = mybir.MatmulPerfMode

B, C, H, W = 16, 512, 32, 32
T = H * W              # 1024
NH = 8                 # heads
CH = C // NH           # 64
GROUPS = 32
GSIZE = C // GROUPS    # 16 channels per group
EPS = 1e-5
N_CORES = 8
BPC = B // N_CORES     # batches per core
CB = C // 128          # 4 channel blocks
NT = T // 512          # 2 column halves of 512
ST = T // 128          # 8 seq tiles of 128
NP = ST // 2           # 4 st-pairs
WSCALE = 32.0          # Wq/Wk fp8 prescale (keeps weights out of denormals)
SHIFT = -4.0           # logit shift before exp (cancels in softmax)
SCH_A = 184.664375     # 2^7 / ln 2
SCH_C = 15511.5        # 16256 - 0.5 - 5.25 + SHIFT*SCH_A  (tuned Schraudolph)
BF16_PAIRS = (0, 1, 2, 3)   # all wts/v tiles bf16: even sts exp on ACT, odd
                            # sts Schraudolph on DVE — two independent psum
                            # chains so the engines never serialize on ps_s
ACT_ODD_HEADS = (3, 11)     # head-batches whose odd-st chain flips to ACT
                            # (fine engine-load balance)

DEBUG = bool(int(os.environ.get("KERNEL_DEBUG", "0")))


def _build():
    nc = bacc.Bacc(None, target_bir_lowering=False)

    x2 = nc.dram_tensor("x2", (BPC, C, T), F32, kind="ExternalInput")
    y2 = nc.dram_tensor("y2", (BPC, C, T), F32, kind="ExternalInput")
    wqt = nc.dram_tensor("wqt", (C, C), FP8, kind="ExternalInput")
    wkt = nc.dram_tensor("wkt", (C, C), FP8, kind="ExternalInput")
    wvt = nc.dram_tensor("wvt", (C, C), FP8, kind="ExternalInput")
    wpt = nc.dram_tensor("wpt", (C, C), FP8, kind="ExternalInput")
    bq_l = nc.dram_tensor("bq_l", (128, CB), F32, kind="ExternalInput")
    bk_l = nc.dram_tensor("bk_l", (128, CB), F32, kind="ExternalInput")
    bp_l = nc.dram_tensor("bp_l", (128, CB), F32, kind="ExternalInput")
    bv_bc = nc.dram_tensor("bv_bc", (128, NH, CH), F32, kind="ExternalInput")
    gnw_l = nc.dram_tensor("gnw_l", (128, CB), F32, kind="ExternalInput")
    gnb_l = nc.dram_tensor("gnb_l", (128, CB), F32, kind="ExternalInput")
    m1 = nc.dram_tensor("m1", (128, 128), F32, kind="ExternalInput")
    id128 = nc.dram_tensor("id128", (128, 128), BF16, kind="ExternalInput")
    out_d = nc.dram_tensor("out", (BPC, C, T), F32, kind="ExternalOutput")
    if DEBUG:
        dbg_a = nc.dram_tensor("dbg_a", (C, T), F32, kind="ExternalOutput")

    with tile.TileContext(nc) as tc:
        from contextlib import ExitStack
        with ExitStack() as ctx:
            consts = ctx.enter_context(tc.tile_pool(name="consts", bufs=1))
            px = ctx.enter_context(tc.tile_pool(name="px", bufs=2))
            py = ctx.enter_context(tc.tile_pool(name="py", bufs=1))
            pgn = ctx.enter_context(tc.tile_pool(name="pgn", bufs=2))
            pk = ctx.enter_context(tc.tile_pool(name="pk", bufs=4))
            pvt = ctx.enter_context(tc.tile_pool(name="pvt", bufs=int(os.environ.get("KPVT", 2 + NP))))
            pq = ctx.enter_context(tc.tile_pool(name="pq", bufs=4))
            pwts = ctx.enter_context(tc.tile_pool(name="pwts", bufs=int(os.environ.get("KPWTS", 8))))
            pa = ctx.enter_context(tc.tile_pool(name="pa", bufs=2))
            pat = ctx.enter_context(tc.tile_pool(name="pat", bufs=2))
            pdn = ctx.enter_context(tc.tile_pool(name="pdn", bufs=1))
            pdn3 = ctx.enter_context(tc.tile_pool(name="pdn3", bufs=2))
            pst = ctx.enter_context(tc.tile_pool(name="pst", bufs=4))
            ps_mm = ctx.enter_context(tc.tile_pool(name="ps_mm", bufs=2, space="PSUM"))
            ps_s = ctx.enter_context(tc.tile_pool(name="ps_s", bufs=2, space="PSUM"))
            ps_a0 = ctx.enter_context(tc.tile_pool(name="ps_a0", bufs=1, space="PSUM"))
            ps_a1 = ctx.enter_context(tc.tile_pool(name="ps_a1", bufs=1, space="PSUM"))

            # --- constants (weights fp8; DMAs ordered for startup overlap) ---
            wq_sb = consts.tile([128, CB, C], FP8, tag="wq")
            wk_sb = consts.tile([128, CB, C], FP8, tag="wk")
            wv_sb = consts.tile([128, CB, C], FP8, tag="wv")
            wp_sb = consts.tile([128, CB, C], FP8, tag="wp")

            def emit_vp_weight_loads():
                nc.sync.dma_start(out=wv_sb, in_=wvt.rearrange("(kb p) o -> p kb o", p=128))
                nc.sync.dma_start(out=wp_sb, in_=wpt.rearrange("(kb p) o -> p kb o", p=128))

            m1_sb = consts.tile([128, 128], F32, tag="m1")
            bq_sb = consts.tile([128, CB], F32, tag="bq")
            bk_sb = consts.tile([128, CB], F32, tag="bk")
            bp_sb = consts.tile([128, CB], F32, tag="bp")
            bv_sb = consts.tile([128, NH, CH], F32, tag="bv")
            gnw_sb = consts.tile([128, CB], F32, tag="gnw")
            gnb_sb = consts.tile([128, CB], F32, tag="gnb")
            id_sb = consts.tile([128, 128], BF16, tag="id")
            magic_sb = consts.tile([128, CB], I32, tag="magic")
            nc.vector.memset(magic_sb, 0x5f3759df)
            ones8_sb = consts.tile([128, NH], FP8, tag="ones8")
            nc.vector.memset(ones8_sb, 1.0)
            ones16_sb = consts.tile([128, NH], BF16, tag="ones16")
            nc.vector.memset(ones16_sb, 1.0)
            nbias_sb = consts.tile([128, 1], F32, tag="nbias")
            nc.vector.memset(nbias_sb, SHIFT)
            # persistent per-head q tiles, fp8, zero-padded outside the head's
            # 64 channels; zeros are memset once and persist across batches
            q8_sb = [consts.tile([128, 2, T], FP8, tag=f"q8_{h}", name=f"q8_{h}")
                     for h in range(NH)]
            for h in range(NH):
                nc.gpsimd.memset(q8_sb[h], 0.0)
            warm = consts.tile([1, 1], F32, tag="warm")
            nc.vector.memset(warm, 0.0)
            nc.scalar.activation(out=warm, in_=warm, func=AF.Exp)

            def emit_gn_consts():
                nc.sync.dma_start(out=m1_sb, in_=m1[:, :])
                nc.sync.dma_start(out=gnw_sb, in_=gnw_l[:, :])
                nc.sync.dma_start(out=gnb_sb, in_=gnb_l[:, :])

            def emit_bias_consts():
                nc.sync.dma_start(out=bk_sb, in_=bk_l[:, :])
                nc.sync.dma_start(out=bq_sb, in_=bq_l[:, :])
                nc.sync.dma_start(out=bv_sb, in_=bv_bc[:, :, :])
                nc.sync.dma_start(out=bp_sb, in_=bp_l[:, :])
                nc.sync.dma_start(out=id_sb, in_=id128[:, :])

            sched_state = {"sch_i": 0}

            def groupnorm(src_sb, dst_fn, fast_apply=False, act_stats=False):
                """src_sb: [128, CB, T] f32. dst_fn(cb, th)->AP (fp8 out)."""
                mv = pst.tile([128, CB, 2], F32, tag="mv")
                if act_stats:
                    # split stats: ACT (Copy/Square accum) covers cb0-1 while
                    # DVE bn_stats covers cb2-3 — halves the serial latency on
                    # the startup-critical tensor.
                    part = pst.tile([128, 2, 2, 2], F32, tag="part")
                    for si, (func, scale) in enumerate(
                            ((AF.Copy, 1.0 / T), (AF.Square, 1.0 / np.sqrt(T)))):
                        for cb in range(2):
                            for c2 in range(2):
                                trash = pwts.tile([128, 512], BF16, tag="trash", name="trash")
                                nc.scalar.activation(
                                    out=trash, in_=src_sb[:, cb, c2 * 512:(c2 + 1) * 512],
                                    func=func, scale=scale,
                                    accum_out=part[:, cb, si, c2:c2 + 1])
                    stats6b = pst.tile([128, 2, 6], F32, tag="stats6b")
                    for cb in (2, 3):
                        for c2 in range(2):
                            nc.vector.bn_stats(
                                out=stats6b[:, c2, :],
                                in_=src_sb[:, cb, c2 * 512:(c2 + 1) * 512])
                        nc.vector.bn_aggr(out=mv[:, cb, :], in_=stats6b)
                    # cb0-1: mv = (mean, E[x^2]) from the two half-col accums
                    nc.vector.tensor_tensor(
                        out=mv[:, 0:2, :].rearrange("p a b -> p (a b)"),
                        in0=part[:, :, :, 0].rearrange("p a b -> p (a b)"),
                        in1=part[:, :, :, 1].rearrange("p a b -> p (a b)"), op=OP.add)
                    # cb2-3: convert var -> E[x^2] in place
                    musqb = pst.tile([128, 2], F32, tag="musqb")
                    nc.vector.tensor_tensor(out=musqb, in0=mv[:, 2:4, 0], in1=mv[:, 2:4, 0], op=OP.mult)
                    nc.vector.tensor_tensor(out=mv[:, 2:4, 1], in0=musqb, in1=mv[:, 2:4, 1], op=OP.add)
                else:
                    stats6 = pst.tile([128, 2, 6], F32, tag="stats6")
                    for cb in range(CB):
                        for c2 in range(2):
                            nc.vector.bn_stats(
                                out=stats6[:, c2, :],
                                in_=src_sb[:, cb, c2 * 512:(c2 + 1) * 512])
                        nc.vector.bn_aggr(out=mv[:, cb, :], in_=stats6)
                    # m2 slot in-place: mv[:,:,1] = var + mean^2
                    musq = pst.tile([128, 4], F32, tag="musq")
                    nc.vector.tensor_tensor(out=musq, in0=mv[:, :, 0], in1=mv[:, :, 0], op=OP.mult)
                    nc.vector.tensor_tensor(out=mv[:, :, 1], in0=musq, in1=mv[:, :, 1], op=OP.add)
                psg = ps_s.tile([128, 8], F32, tag="sc", name="psg")
                nc.tensor.matmul(psg, m1_sb, mv.rearrange("p a b -> p (a b)"), start=True, stop=True)
                gsb = pst.tile([128, 8], F32, tag="gsb")
                nc.vector.tensor_copy(gsb, psg)  # m1 carries 1/GSIZE; cols interleaved (mean, m2)
                # var + eps = (m2 + eps) - mean^2, fused
                tmp4 = pst.tile([128, 4], F32, tag="tmp4")
                nc.vector.tensor_tensor(out=tmp4, in0=gsb[:, 0::2], in1=gsb[:, 0::2], op=OP.mult)
                vv = pst.tile([128, 4], F32, tag="vv")
                nc.vector.scalar_tensor_tensor(
                    out=vv, in0=gsb[:, 1::2], scalar=EPS, in1=tmp4,
                    op0=OP.add, op1=OP.subtract)
                # rstd = rsqrt(vv): quake seed + 1 Newton step (3 fused ops)
                bsh = pst.tile([128, 4], I32, tag="bsh")
                nc.vector.tensor_scalar(
                    out=bsh, in0=vv.bitcast(I32), scalar1=1, scalar2=None,
                    op0=OP.logical_shift_right)
                nc.vector.tensor_tensor(out=tmp4.bitcast(I32), in0=magic_sb, in1=bsh, op=OP.subtract)
                nrt = pst.tile([128, 4], F32, tag="nrt")
                for _ in range(1):
                    nc.vector.tensor_tensor(out=nrt, in0=tmp4, in1=tmp4, op=OP.mult)
                    nc.vector.scalar_tensor_tensor(
                        out=nrt, in0=nrt, scalar=-0.5, in1=vv, op0=OP.mult, op1=OP.mult)
                    nc.vector.scalar_tensor_tensor(
                        out=tmp4, in0=nrt, scalar=1.5, in1=tmp4, op0=OP.add, op1=OP.mult)
                ab = pst.tile([128, 8], F32, tag="ab")
                nc.vector.tensor_tensor(out=ab[:, 0:4], in0=tmp4, in1=gnw_sb, op=OP.mult)
                tmp4b = pst.tile([128, 4], F32, tag="tmp4b")
                nc.vector.tensor_tensor(out=tmp4b, in0=gsb[:, 0::2], in1=ab[:, 0:4], op=OP.mult)
                nc.vector.tensor_tensor(out=ab[:, 4:8], in0=gnb_sb, in1=tmp4b, op=OP.subtract)
                for th in range(NT):
                    for cb in range(CB):
                        if os.environ.get("KERNEL_BASE_GN") == "1":
                            eng = nc.gpsimd if (cb % 2 == 1 and not fast_apply) else nc.vector
                            eng.tensor_scalar(
                                out=dst_fn(cb, th), in0=src_sb[:, cb, th * 512:(th + 1) * 512],
                                scalar1=ab[:, cb:cb + 1], scalar2=ab[:, 4 + cb:5 + cb],
                                op0=OP.mult, op1=OP.add)
                        elif fast_apply:
                            # startup-critical: ACT is idle here and faster
                            nc.scalar.activation(
                                out=dst_fn(cb, th),
                                in_=src_sb[:, cb, th * 512:(th + 1) * 512],
                                func=AF.Identity, bias=ab[:, 4 + cb:5 + cb],
                                scale=ab[:, cb:cb + 1])
                        else:
                            nc.gpsimd.tensor_scalar(
                                out=dst_fn(cb, th), in0=src_sb[:, cb, th * 512:(th + 1) * 512],
                                scalar1=ab[:, cb:cb + 1], scalar2=ab[:, 4 + cb:5 + cb],
                                op0=OP.mult, op1=OP.add)

            def emit_input_loads(b, mid_fn=None):
                y_sb = py.tile([128, CB, T], F32, tag="y")
                for cb in range(CB):
                    nc.sync.dma_start(
                        out=y_sb[:, cb, :],
                        in_=y2[b].rearrange("(cb p) t -> p cb t", p=128)[:, cb, :])
                if mid_fn is not None:
                    mid_fn()  # gn consts + k/q weights jump the queue ahead of x
                x_sb = px.tile([128, CB, T], F32, tag="x")
                for cb in range(CB):
                    nc.sync.dma_start(
                        out=x_sb[:, cb, :],
                        in_=x2[b].rearrange("(cb p) t -> p cb t", p=128)[:, cb, :])
                return x_sb, y_sb

            def emit_gn_compute(x_sb, y_sb, fast_apply=False, act_stats=False):
                gny = pgn.tile([128, CB, T], FP8, tag="gn")
                groupnorm(y_sb, lambda cb, th: gny[:, cb, th * 512:(th + 1) * 512], fast_apply)
                gnx = pgn.tile([128, CB, T], FP8, tag="gn")
                groupnorm(x_sb, lambda cb, th: gnx[:, cb, th * 512:(th + 1) * 512], fast_apply,
                          act_stats=act_stats)
                return gnx, gny

            def q_unit(bctx, ob, pool=None):
                """Closure list: 4 DR matmuls computing q for one ob (2 heads)."""
                st8 = {}
                def mk(th, j):
                    def f():
                        gnx = bctx["gn"][0]
                        if j == 0:
                            st8[th] = (ps_mm.tile([128, 512], F32, tag="mm", name="psu")
                                       if pool is None else
                                       pool.tile([128, 512], F32, tag="sc", name="psu"))
                        psq = st8[th]
                        nc.tensor.matmul(
                            psq,
                            wq_sb[:, 2 * j:2 * j + 2, ob * 128:(ob + 1) * 128],
                            gnx[:, 2 * j:2 * j + 2, th * 512:(th + 1) * 512],
                            start=(j == 0), stop=(j == 1), perf_mode=PM.DoubleRow)
                        if j == 1:
                            jj = ob % 2
                            nc.scalar.activation(
                                out=q8_sb[2 * ob][0:64, jj, th * 512:(th + 1) * 512],
                                in_=psq[0:64, :], func=AF.Identity,
                                bias=bq_sb[0:64, ob:ob + 1], scale=1.0 / WSCALE)
                            nc.scalar.activation(
                                out=q8_sb[2 * ob + 1][64:128, jj, th * 512:(th + 1) * 512],
                                in_=psq[64:128, :], func=AF.Identity,
                                bias=bq_sb[64:128, ob:ob + 1], scale=1.0 / WSCALE)
                    return f
                return [mk(th, j) for th in range(NT) for j in range(2)]

            def k_unit(bctx, ob, pool=None):
                st8 = {}
                def mk(th, j):
                    def f():
                        gny = bctx["gn"][1]
                        obp = ob // 2
                        if bctx["k8"][obp] is None:
                            bctx["k8"][obp] = pk.tile([128, 2, T], FP8, tag="k", name="k_obp")
                        k8p = bctx["k8"][obp]
                        if j == 0:
                            st8[th] = (ps_mm.tile([128, 512], F32, tag="mm", name="psu")
                                       if pool is None else
                                       pool.tile([128, 512], F32, tag="sc", name="psu"))
                        psk = st8[th]
                        nc.tensor.matmul(
                            psk,
                            wk_sb[:, 2 * j:2 * j + 2, ob * 128:(ob + 1) * 128],
                            gny[:, 2 * j:2 * j + 2, th * 512:(th + 1) * 512],
                            start=(j == 0), stop=(j == 1), perf_mode=PM.DoubleRow)
                        if j == 1:
                            nc.scalar.activation(
                                out=k8p[:, ob % 2, th * 512:(th + 1) * 512],
                                in_=psk, func=AF.Identity,
                                bias=bk_sb[:, ob:ob + 1], scale=1.0 / WSCALE)
                    return f
                return [mk(th, j) for th in range(NT) for j in range(2)]

            def vt_unit(bctx, tt):
                """Closure list: 2 DR matmuls + bias for one vT seq tile.

                vT tiles hold an st-PAIR: [128, 2, NH, CH+1] bf16."""
                st8 = {}
                pair, sub = tt // 2, tt % 2
                def mk(j):
                    def f():
                        gny = bctx["gn"][1]
                        if j == 0:
                            st8["ps"] = ps_mm.tile([128, 512], F32, tag="mm", name="psu")
                        psv = st8["ps"]
                        nc.tensor.matmul(
                            psv,
                            gny[:, 2 * j:2 * j + 2, tt * 128:(tt + 1) * 128],
                            wv_sb[:, 2 * j:2 * j + 2, :],
                            start=(j == 0), stop=(j == 1), perf_mode=PM.DoubleRow)
                        if j == 1:
                            if bctx["vtp"][pair] is None:
                                bctx["vtp"][pair] = pvt.tile(
                                    [128, 2, NH, CH + 1], BF16, tag="vt", name="vt")
                            vt = bctx["vtp"][pair]
                            nc.vector.tensor_tensor(
                                out=vt[:, sub, :, 0:CH],
                                in0=psv.rearrange("p (h c) -> p h c", h=NH),
                                in1=bv_sb, op=OP.add)
                            nc.vector.tensor_copy(
                                vt[:, sub, :, CH:CH + 1],
                                ones16_sb.rearrange("p (h o) -> p h o", o=1))
                    return f
                return [mk(j) for j in range(2)]

            def pproj_unit(bctx, b, ob, pool=None, split_tail=False, store_q=None,
                           pool_tag="sc", act_resid=False):
                """Closure list: 4 DR matmuls + bias/residual/store for one out block.

                split_tail: j0 group (kb0-1, residual applied early), then a
                separate j1 group + final add + store — shortens the critical
                chain behind the last attention block.
                """
                st8 = {}
                def mk(th, j):
                    def f():
                        xr = bctx["x"]
                        a_sb = bctx["a"]
                        sl = slice(th * 512, (th + 1) * 512)
                        if j == 0 or split_tail:
                            st8[th] = (ps_mm.tile([128, 512], F32, tag="mm", name="psu")
                                       if pool is None else
                                       pool.tile([128, 512], F32, tag=pool_tag, name="psu"))
                        psh = st8[th]
                        nc.tensor.matmul(
                            psh,
                            wp_sb[:, 2 * j:2 * j + 2, ob * 128:(ob + 1) * 128],
                            a_sb[:, 2 * j:2 * j + 2, sl],
                            start=(j == 0 or split_tail),
                            stop=(j == 1 or split_tail),
                            perf_mode=PM.DoubleRow)
                        grp_end = (j == 0) if split_tail else (j == 1)
                        if grp_end:
                            nc.vector.scalar_tensor_tensor(
                                out=xr[:, ob, sl],
                                in0=psh, scalar=bp_sb[:, ob:ob + 1],
                                in1=xr[:, ob, sl], op0=OP.add, op1=OP.add)
                        if j == 1:
                            if split_tail:
                                nc.vector.tensor_tensor(
                                    out=xr[:, ob, sl], in0=psh, in1=xr[:, ob, sl],
                                    op=OP.add)
                            (store_q or nc.sync).dma_start(
                                out=out_d[b].rearrange("(cb p) t -> p cb t", p=128)[:, ob, sl],
                                in_=xr[:, ob, sl])
                    return f
                return [mk(th, j) for th in range(NT) for j in range(2)]

            def attention_head(bctx, ob, hh, qp, a_sb, lazy_vt=False, filler=None):
                h = 2 * ob + hh
                k8p = bctx["k8"][ob // 2]
                vtp = bctx["vtp"]
                psa_t = [ps_a0.tile([128, 512], F32, tag="av0", name="psa0"),
                         ps_a1.tile([128, 512], F32, tag="av1", name="psa1")]
                psa = [t[:, 0:4 * (CH + 1)].rearrange("p (a b) -> p a b", b=CH + 1)
                       for t in psa_t]

                def emit_avs(pair, wts):
                    first = pair == 0
                    last = pair == NP - 1
                    for th in range(NT):
                        for sub in range(2):
                            for tc4 in range(4):
                                nc.tensor.matmul(
                                    psa[th][:, tc4, :],
                                    wts[:, sub, th, tc4 * 128:(tc4 + 1) * 128],
                                    vtp[pair][:, sub, h, :],
                                    start=(first and sub == 0 and tc4 == 0),
                                    stop=(last and sub == 1 and tc4 == 3),
                                    skip_group_check=True)
                        if filler is not None:
                            filler()
                            filler()

                prev = None
                hg = bctx["b"] * NH + h
                for pair in range(NP):
                    if lazy_vt and vtp[pair] is None:
                        for tt in (2 * pair, 2 * pair + 1):
                            for f in vt_unit(bctx, tt):
                                f()
                    wts = pwts.tile([128, 2, NT, 512], BF16, tag="wts")
                    for sub in range(2):
                        st = 2 * pair + sub
                        pss = ps_s.tile([128, NT, 512], F32, tag="sc")
                        for th in range(NT):
                            nc.tensor.matmul(
                                pss[:, th, :],
                                k8p[:, :, st * 128:(st + 1) * 128],
                                q8_sb[h][:, :, th * 512:(th + 1) * 512],
                                start=True, stop=True, perf_mode=PM.DoubleRow)
                            if filler is not None:
                                filler()
                        # exp engines alternate by st parity: even sts on ACT
                        # (exact exp), odd sts Schraudolph fast-exp on DVE —
                        # each engine owns one of the two ps_s buffer chains
                        if sub == 1 and hg not in ACT_ODD_HEADS:
                            nc.vector.tensor_scalar(
                                out=wts.bitcast(I16)[:, sub].rearrange("p a b -> p (a b)"),
                                in0=pss.rearrange("p a b -> p (a b)"),
                                scalar1=SCH_A, scalar2=SCH_C,
                                op0=OP.mult, op1=OP.add)
                        else:
                            nc.scalar.activation(
                                out=wts[:, sub].rearrange("p a b -> p (a b)"),
                                in_=pss.rearrange("p a b -> p (a b)"),
                                func=AF.Exp, bias=nbias_sb, scale=1.0)
                    # AVs run one pair behind: the next QK (exp input) stays
                    # at the head of the in-order PE queue
                    if prev is not None:
                        emit_avs(*prev)
                    prev = (pair, wts)
                emit_avs(*prev)
                finish_head(bctx, ob, hh, psa, a_sb)

            def finish_head(bctx, ob, hh, psa, a_sb):
                # denominators are per-partition columns now: copy+recip+scale.
                # high priority: these free the single-buffered AV psum banks,
                # so they must win DVE scheduling ties against filler ops.
                with tc.high_priority():
                    _finish_head(bctx, ob, hh, psa, a_sb)

            def _finish_head(bctx, ob, hh, psa, a_sb):
                h = 2 * ob + hh
                aT = bctx["aT"]
                for th in range(NT):
                    rr = pdn3.tile([128, 4], F32, tag="dn3")
                    if os.environ.get("KERNEL_DN_COPY") == "1":
                        dn = pdn.tile([128, 4], F32, tag="r0")
                        nc.vector.tensor_copy(dn, psa[th][:, :, CH])
                        nc.vector.reciprocal_approx_fast(out=rr, in_=dn)
                    else:
                        nc.vector.reciprocal_approx_fast(out=rr, in_=psa[th][:, :, CH])
                    nc.vector.tensor_tensor(
                        out=aT[:, th, :, h, :],
                        in0=psa[th][:, :, 0:CH],
                        in1=rr[:, :, None].broadcast_to([128, 4, CH]),
                        op=OP.mult)

            def finish_ob(bctx, ob, a_sb):
                """Transpose aT[t, c] blocks of channel-block ob back to a[c, t]."""
                aT = bctx["aT"]
                trp = ps_mm.tile([128, ST, 128], BF16, tag="mm", name="trp")
                for tb in range(ST):
                    th, tc4 = tb // 4, tb % 4
                    nc.tensor.matmul(
                        trp[:, tb, :],
                        aT[:, th, tc4, 2 * ob:2 * ob + 2, :].rearrange("p a b -> p (a b)"),
                        id_sb,
                        is_transpose=True)
                nc.scalar.activation(out=a_sb[:, ob, :],
                                     in_=trp.rearrange("p a b -> p (a b)"),
                                     func=AF.Copy)

            # ---------------- batch pipeline ----------------
            from collections import deque
            fillers = deque()

            def filler_pop():
                if fillers:
                    fillers.popleft()()

            def filler_flush():
                while fillers:
                    fillers.popleft()()

            bctxs = [dict() for _ in range(BPC)]
            _xy0 = emit_input_loads(0, mid_fn=emit_gn_consts)
            for wi in range(20):
                trash = ps_mm.tile([128, 64], F32, tag="mm", name="wtr0")
                nc.tensor.matmul(trash, _xy0[0][:, 0, 0:128], _xy0[0][:, 0, 0:64],
                                 start=True, stop=True)
            emit_bias_consts()
            nc.sync.dma_start(out=wq_sb, in_=wqt.rearrange("(kb p) o -> p kb o", p=128))
            nc.sync.dma_start(out=wk_sb, in_=wkt.rearrange("(kb p) o -> p kb o", p=128))
            emit_vp_weight_loads()
            bctxs[0]["x"] = _xy0[0]
            bctxs[0]["gn"] = emit_gn_compute(*_xy0, fast_apply=True, act_stats=True)
            bctxs[0]["k8"] = [None] * (CB // 2)
            bctxs[0]["vtp"] = [None] * NP

            # batch 0 ob0 prep emitted directly; rest queued as fillers that
            # drip into the attention pair slots (1 matmul per QK/AV pair)
            for f in k_unit(bctxs[0], 0, pool=ps_s):
                f()
            for f in q_unit(bctxs[0], 0, pool=ps_s):
                f()
            for ob2 in (1, 2, 3):
                fillers.extend(k_unit(bctxs[0], ob2))
                fillers.extend(q_unit(bctxs[0], ob2))

            for b in range(BPC):
                bctx = bctxs[b]
                bctx["b"] = b
                a_sb = pa.tile([128, CB, T], FP8, tag="a")
                bctx["a"] = a_sb
                bctx["aT"] = pat.tile([128, NT, 4, NH, CH], BF16, tag="aT", name="aT")
                for ob in range(CB):
                    if b > 0:
                        if ob == 0:
                            for ob2 in (1, 2):
                                fillers.extend(k_unit(bctx, ob2))
                                fillers.extend(q_unit(bctx, ob2))
                        if ob == 1:
                            fillers.extend(k_unit(bctx, 3))
                            fillers.extend(q_unit(bctx, 3))
                            for ob2 in range(CB):
                                fillers.extend(pproj_unit(bctxs[b - 1], b - 1, ob2, pool=ps_s))
                    if b + 1 < BPC:
                        if ob == 0:
                            nb = bctxs[b + 1]
                            nb["xy"] = emit_input_loads(b + 1)
                            nb["x"] = nb["xy"][0]
                        if ob == 1:
                            nb = bctxs[b + 1]
                            nb["gn"] = emit_gn_compute(*nb.pop("xy"))
                            nb["k8"] = [None] * (CB // 2)
                            nb["vtp"] = [None] * NP

                    for hh in (0, 1):
                        attention_head(bctx, ob, hh, None, a_sb,
                                       lazy_vt=(b == 0 and ob == 0),
                                       filler=filler_pop)
                        if hh == 0 and b + 1 < BPC:
                            nb = bctxs[b + 1]
                            if ob == 2:
                                fillers.extend(k_unit(nb, 0))
                                for tt in range(ST):
                                    fillers.extend(vt_unit(nb, tt))
                            if ob == 3:
                                fillers.extend(q_unit(nb, 0))
                    if ob > 0:
                        finish_ob(bctx, ob - 1, a_sb)
                    if ob == CB - 1:
                        finish_ob(bctx, ob, a_sb)

                if b == BPC - 1:
                    # tail: drain queue, then final output projection directly.
                    filler_flush()
                    # dummy matmuls keep the PE clock ramped while the last
                    # head's softmax-normalize chain drains
                    for wi in range(10):
                        trash = ps_mm.tile([128, 512], F32, tag="mm", name="wtr")
                        nc.tensor.matmul(trash, wp_sb[:, 0, 0:128],
                                         a_sb[:, 0, 0:512],
                                         start=True, stop=True)
                    tail_pools = [(ps_a0, "av0"), (ps_a1, "av1"), (ps_s, "sc"), (None, "sc")]
                    for ob2 in range(CB):
                        pl, tg = tail_pools[ob2]
                        for f in pproj_unit(bctx, b, ob2, pool=pl, pool_tag=tg,
                                            store_q=nc.scalar if ob2 % 2 == 0 else nc.sync):
                            f()

    nc.finalize()
    return nc


_NC = None


def _get_nc():
    global _NC
    if _NC is None:
        _NC = _build()
    return _NC


def _prep_inputs(x, y, gn_w, gn_b, Wq, bq, Wkv, bkv, Wp, bp):
    scale = CH ** -0.25
    # reference splits k/v per head: kvh[:, h, :ch] / kvh[:, h, ch:] after
    # reshape to [b, NH, 2*ch, T] -> k_h = Wkv rows [h*128, h*128+64)
    import ml_dtypes
    FP8NP = ml_dtypes.float8_e4m3
    idx_k = np.concatenate([np.arange(h * 2 * CH, h * 2 * CH + CH) for h in range(NH)])
    idx_v = np.concatenate([np.arange(h * 2 * CH + CH, (h + 1) * 2 * CH) for h in range(NH)])
    # Wq/Wk prescaled by WSCALE to keep fp8 values out of denormal range;
    # compensated by 1/WSCALE in the psum->sbuf bias add.
    wqt = np.ascontiguousarray((Wq * (scale * WSCALE)).T).astype(FP8NP)
    wkt = np.ascontiguousarray((Wkv[idx_k] * (scale * WSCALE)).T).astype(FP8NP)
    wvt = np.ascontiguousarray(Wkv[idx_v].T).astype(FP8NP)
    wpt = np.ascontiguousarray(Wp.T).astype(FP8NP)
    bq_s = bq * scale
    bk_s = bkv[idx_k] * scale
    bv = bkv[idx_v]

    def part_layout(v):  # [C] -> [128, CB]: v[cb*128+p]
        return np.ascontiguousarray(v.reshape(CB, 128).T)

    bq_l = part_layout(bq_s)
    bk_l = part_layout(bk_s)
    bp_l = part_layout(bp)
    gnw_l = part_layout(gn_w)
    gnb_l = part_layout(gn_b)
    bv_bc = np.broadcast_to(bv.reshape(1, NH, CH), (128, NH, CH)).copy()
    m1 = np.zeros((128, 128), np.float32)
    for g in range(128 // GSIZE):
        m1[g * GSIZE:(g + 1) * GSIZE, g * GSIZE:(g + 1) * GSIZE] = 1.0 / GSIZE
    id128_h = np.eye(128, dtype=ml_dtypes.bfloat16)

    xf = x.reshape(B, C, T)
    yf = y.reshape(B, C, T)

    shared = {
        "wqt": wqt, "wkt": wkt, "wvt": wvt, "wpt": wpt,
        "bq_l": bq_l, "bk_l": bk_l, "bp_l": bp_l, "bv_bc": bv_bc,
        "gnw_l": gnw_l, "gnb_l": gnb_l, "m1": m1, "id128": id128_h,
    }
    in_maps = []
    for i in range(N_CORES):
        m = dict(shared)
        m["x2"] = np.ascontiguousarray(xf[i * BPC:(i + 1) * BPC])
        m["y2"] = np.ascontiguousarray(yf[i * BPC:(i + 1) * BPC])
        in_maps.append(m)
    return in_maps


def kernel(x, y, gn_w, gn_b, Wq, bq, Wkv, bkv, Wp, bp):
    args = [np.asarray(a, dtype=np.float32) for a in
            (x, y, gn_w, gn_b, Wq, bq, Wkv, bkv, Wp, bp)]
    in_maps = _prep_inputs(*args)
    nc = _get_nc()
    res = run_bass_kernel_spmd(nc, in_maps, core_ids=list(range(N_CORES)))
    out = np.empty((B, C, T), np.float32)
    for i in range(N_CORES):
        out[i * BPC:(i + 1) * BPC] = res.results[i]["out"]
    return out.reshape(B, C, H, W)


# revision 34
# speedup vs baseline: 1.0386x; 1.0197x over previous
"""AttentionBlock Trainium2 Bass kernel.

Data-parallel over batch: 16 batches / 8 cores = 2 per core. Each core runs
the full block (groupnorm x2, q/kv projections, 8-head attention, output
projection, residual) on its 2 batch elements.

Key design points (v2):
- fp8e4m3 DoubleRow matmuls for all four projections (Wq/Wk prescaled x32 to
  escape fp8 denormals, compensated in the psum->sbuf bias add) and for the
  attention*V of st-pairs 1-3: 256-deep contraction at 0.5 cyc/row quarters
  projection PE time and AV PE time vs bf16.
- scores layout [s, t]; exp without max-subtraction but with a constant -4
  logit shift so exp output fits fp8 range (shift cancels in softmax).
- exp engine split: st-pairs 1-3 go to ACT (exp -> fp8 wts), st-pair 0 goes
  to DVE as a Schraudolph fast-exp (one tensor_scalar: i16(round(l*184.66 +
  15511.5)) bitcast bf16, ~3% multiplicative err, cancels mostly in softmax).
  Pair-0 wts/v tiles are bf16; AV for pair 0 runs as plain bf16 matmuls.
- wts/vT tiles hold an st-PAIR each ([128, 2, ...]) so DR matmuls can pair
  the contraction; softmax denominator rides along as a ones column of vT.
- normalize: one broadcast tensor_tensor per (head, th) scales psum by the
  per-partition reciprocal denominators -> aT bf16; PE transpose per channel
  block; psum->sbuf copy converts a to fp8 for the DR output projection.
- groupnorm applies run on GPSIMD (Pool) except the startup-critical batch-0
  pair; stats stay on DVE bn_stats (batch-0 x split ACT/DVE).
- software pipelining: AV matmuls trail their QK pair by one st-pair;
  projection/output matmuls are emitted as 1-matmul closures popped between
  attention slots; next-batch prep is pushed mid-ob.
- startup: y loads -> gn consts -> x loads -> biases -> fp8 weights; PE clock
  pre-warmed with dummy matmuls.
"""
import os
import sys

sys.path.insert(0, "/opt/trn_rl_repo")

import numpy as np

import concourse.bacc as bacc
import concourse.bass as bass
import concourse.tile as tile
from concourse import mybir
from concourse.bass_utils import run_bass_kernel_spmd

F32 = mybir.dt.float32
F32R = mybir.dt.float32r
BF16 = mybir.dt.bfloat16
FP8 = mybir.dt.float8e4
I16 = mybir.dt.int16
I32 = mybir.dt.int32
AF = mybir.ActivationFunctionType
OP = mybir.AluOpType
PM = mybir.MatmulPerfMode

B, C, H, W = 16, 512, 32, 32
T = H * W              # 1024
NH = 8                 # heads
CH = C // NH           # 64
GROUPS = 32
GSIZE = C // GROUPS    # 16 channels per group
EPS = 1e-5
N_CORES = 8
BPC = B // N_CORES     # batches per core
CB = C // 128          # 4 channel blocks
NT = T // 512          # 2 column halves of 512
ST = T // 128          # 8 seq tiles of 128
NP = ST // 2           # 4 st-pairs
WSCALE = 32.0          # Wq/Wk fp8 prescale (keeps weights out of denormals)
SHIFT = -4.0           # logit shift before exp (cancels in softmax)
SCH_A = 184.664375     # 2^7 / ln 2
SCH_C = 15511.5        # 16256 - 0.5 - 5.25 + SHIFT*SCH_A  (tuned Schraudolph)
BF16_PAIRS = (0, 1, 2, 3)   # all wts/v tiles bf16: even sts exp on ACT, odd
                            # sts Schraudolph on DVE — two independent psum
                            # chains so the engines never serialize on ps_s
ACT_ODD_HEADS = (3, 11)     # head-batches whose odd-st chain flips to ACT
                            # (fine engine-load balance)

DEBUG = bool(int(os.environ.get("KERNEL_DEBUG", "0")))


def _build():
    nc = bacc.Bacc(None, target_bir_lowering=False)

    x2 = nc.dram_tensor("x2", (BPC, C, T), F32, kind="ExternalInput")
    y2 = nc.dram_tensor("y2", (BPC, C, T), F32, kind="ExternalInput")
    wqt = nc.dram_tensor("wqt", (C, C), FP8, kind="ExternalInput")
    wkt = nc.dram_tensor("wkt", (C, C), FP8, kind="ExternalInput")
    wvt = nc.dram_tensor("wvt", (C, C), FP8, kind="ExternalInput")
    wpt = nc.dram_tensor("wpt", (C, C), FP8, kind="ExternalInput")
    bq_l = nc.dram_tensor("bq_l", (128, CB), F32, kind="ExternalInput")
    bk_l = nc.dram_tensor("bk_l", (128, CB), F32, kind="ExternalInput")
    bp_l = nc.dram_tensor("bp_l", (128, CB), F32, kind="ExternalInput")
    bv_bc = nc.dram_tensor("bv_bc", (128, NH, CH), F32, kind="ExternalInput")
    gnw_l = nc.dram_tensor("gnw_l", (128, CB), F32, kind="ExternalInput")
    gnb_l = nc.dram_tensor("gnb_l", (128, CB), F32, kind="ExternalInput")
    m1 = nc.dram_tensor("m1", (128, 128), F32, kind="ExternalInput")
    id128 = nc.dram_tensor("id128", (128, 128), BF16, kind="ExternalInput")
    out_d = nc.dram_tensor("out", (BPC, C, T), F32, kind="ExternalOutput")
    if DEBUG:
        dbg_a = nc.dram_tensor("dbg_a", (C, T), F32, kind="ExternalOutput")

    with tile.TileContext(nc) as tc:
        from contextlib import ExitStack
        with ExitStack() as ctx:
            consts = ctx.enter_context(tc.tile_pool(name="consts", bufs=1))
            px = ctx.enter_context(tc.tile_pool(name="px", bufs=2))
            py = ctx.enter_context(tc.tile_pool(name="py", bufs=1))
            pgn = ctx.enter_context(tc.tile_pool(name="pgn", bufs=2))
            pk = ctx.enter_context(tc.tile_pool(name="pk", bufs=4))
            pvt = ctx.enter_context(tc.tile_pool(name="pvt", bufs=int(os.environ.get("KPVT", 2 + NP))))
            pq = ctx.enter_context(tc.tile_pool(name="pq", bufs=4))
            pwts = ctx.enter_context(tc.tile_pool(name="pwts", bufs=int(os.environ.get("KPWTS", 8))))
            pa = ctx.enter_context(tc.tile_pool(name="pa", bufs=2))
            pat = ctx.enter_context(tc.tile_pool(name="pat", bufs=2))
            pdn = ctx.enter_context(tc.tile_pool(name="pdn", bufs=1))
            pdn3 = ctx.enter_context(tc.tile_pool(name="pdn3", bufs=2))
            pst = ctx.enter_context(tc.tile_pool(name="pst", bufs=4))
            ps_mm = ctx.enter_context(tc.tile_pool(name="ps_mm", bufs=2, space="PSUM"))
            ps_s = ctx.enter_context(tc.tile_pool(name="ps_s", bufs=2, space="PSUM"))
            ps_a0 = ctx.enter_context(tc.tile_pool(name="ps_a0", bufs=1, space="PSUM"))
            ps_a1 = ctx.enter_context(tc.tile_pool(name="ps_a1", bufs=1, space="PSUM"))

            # --- constants (weights fp8; DMAs ordered for startup overlap) ---
            wq_sb = consts.tile([128, CB, C], FP8, tag="wq")
            wk_sb = consts.tile([128, CB, C], FP8, tag="wk")
            wv_sb = consts.tile([128, CB, C], FP8, tag="wv")
            wp_sb = consts.tile([128, CB, C], FP8, tag="wp")

            def emit_vp_weight_loads():
                nc.sync.dma_start(out=wv_sb, in_=wvt.rearrange("(kb p) o -> p kb o", p=128))
                nc.sync.dma_start(out=wp_sb, in_=wpt.rearrange("(kb p) o -> p kb o", p=128))

            m1_sb = consts.tile([128, 128], F32, tag="m1")
            bq_sb = consts.tile([128, CB], F32, tag="bq")
            bk_sb = consts.tile([128, CB], F32, tag="bk")
            bp_sb = consts.tile([128, CB], F32, tag="bp")
            bv_sb = consts.tile([128, NH, CH], F32, tag="bv")
            gnw_sb = consts.tile([128, CB], F32, tag="gnw")
            gnb_sb = consts.tile([128, CB], F32, tag="gnb")
            id_sb = consts.tile([128, 128], BF16, tag="id")
            magic_sb = consts.tile([128, CB], I32, tag="magic")
            nc.vector.memset(magic_sb, 0x5f3759df)
            ones8_sb = consts.tile([128, NH], FP8, tag="ones8")
            nc.vector.memset(ones8_sb, 1.0)
            ones16_sb = consts.tile([128, NH], BF16, tag="ones16")
            nc.vector.memset(ones16_sb, 1.0)
            nbias_sb = consts.tile([128, 1], F32, tag="nbias")
            nc.vector.memset(nbias_sb, SHIFT)
            # persistent per-head q tiles, fp8, zero-padded outside the head's
            # 64 channels; zeros are memset once and persist across batches
            q8_sb = [consts.tile([128, 2, T], FP8, tag=f"q8_{h}", name=f"q8_{h}")
                     for h in range(NH)]
            for h in range(NH):
                nc.gpsimd.memset(q8_sb[h], 0.0)
            warm = consts.tile([1, 1], F32, tag="warm")
            nc.vector.memset(warm, 0.0)
            nc.scalar.activation(out=warm, in_=warm, func=AF.Exp)

            def emit_gn_consts():
                nc.sync.dma_start(out=m1_sb, in_=m1[:, :])
                nc.sync.dma_start(out=gnw_sb, in_=gnw_l[:, :])
                nc.sync.dma_start(out=gnb_sb, in_=gnb_l[:, :])

            def emit_bias_consts():
                nc.sync.dma_start(out=bk_sb, in_=bk_l[:, :])
                nc.sync.dma_start(out=bq_sb, in_=bq_l[:, :])
                nc.sync.dma_start(out=bv_sb, in_=bv_bc[:, :, :])
                nc.sync.dma_start(out=bp_sb, in_=bp_l[:, :])
                nc.sync.dma_start(out=id_sb, in_=id128[:, :])

            sched_state = {"sch_i": 0}

            def groupnorm(src_sb, dst_fn, fast_apply=False, act_stats=False):
                """src_sb: [128, CB, T] f32. dst_fn(cb, th)->AP (fp8 out)."""
                mv = pst.tile([128, CB, 2], F32, tag="mv")
                if act_stats:
                    # split stats: ACT (Copy/Square accum) covers cb0-1 while
                    # DVE bn_stats covers cb2-3 — halves the serial latency on
                    # the startup-critical tensor.
                    part = pst.tile([128, 2, 2, 2], F32, tag="part")
                    for si, (func, scale) in enumerate(
                            ((AF.Copy, 1.0 / T), (AF.Square, 1.0 / np.sqrt(T)))):
                        for cb in range(2):
                            for c2 in range(2):
                                trash = pwts.tile([128, 512], BF16, tag="trash", name="trash")
                                nc.scalar.activation(
                                    out=trash, in_=src_sb[:, cb, c2 * 512:(c2 + 1) * 512],
                                    func=func, scale=scale,
                                    accum_out=part[:, cb, si, c2:c2 + 1])
                    stats6b = pst.tile([128, 2, 6], F32, tag="stats6b")
                    for cb in (2, 3):
                        for c2 in range(2):
                            nc.vector.bn_stats(
                                out=stats6b[:, c2, :],
                                in_=src_sb[:, cb, c2 * 512:(c2 + 1) * 512])
                        nc.vector.bn_aggr(out=mv[:, cb, :], in_=stats6b)
                    # cb0-1: mv = (mean, E[x^2]) from the two half-col accums
                    nc.vector.tensor_tensor(
                        out=mv[:, 0:2, :].rearrange("p a b -> p (a b)"),
                        in0=part[:, :, :, 0].rearrange("p a b -> p (a b)"),
                        in1=part[:, :, :, 1].rearrange("p a b -> p (a b)"), op=OP.add)
                    # cb2-3: convert var -> E[x^2] in place
                    musqb = pst.tile([128, 2], F32, tag="musqb")
                    nc.vector.tensor_tensor(out=musqb, in0=mv[:, 2:4, 0], in1=mv[:, 2:4, 0], op=OP.mult)
                    nc.vector.tensor_tensor(out=mv[:, 2:4, 1], in0=musqb, in1=mv[:, 2:4, 1], op=OP.add)
                else:
                    stats6 = pst.tile([128, 2, 6], F32, tag="stats6")
                    for cb in range(CB):
                        for c2 in range(2):
                            nc.vector.bn_stats(
                                out=stats6[:, c2, :],
                                in_=src_sb[:, cb, c2 * 512:(c2 + 1) * 512])
                        nc.vector.bn_aggr(out=mv[:, cb, :], in_=stats6)
                    # m2 slot in-place: mv[:,:,1] = var + mean^2
                    musq = pst.tile([128, 4], F32, tag="musq")
                    nc.vector.tensor_tensor(out=musq, in0=mv[:, :, 0], in1=mv[:, :, 0], op=OP.mult)
                    nc.vector.tensor_tensor(out=mv[:, :, 1], in0=musq, in1=mv[:, :, 1], op=OP.add)
                psg = ps_s.tile([128, 8], F32, tag="sc", name="psg")
                nc.tensor.matmul(psg, m1_sb, mv.rearrange("p a b -> p (a b)"), start=True, stop=True)
                gsb = pst.tile([128, 8], F32, tag="gsb")
                nc.vector.tensor_copy(gsb, psg)  # m1 carries 1/GSIZE; cols interleaved (mean, m2)
                # var + eps = (m2 + eps) - mean^2, fused
                tmp4 = pst.tile([128, 4], F32, tag="tmp4")
                nc.vector.tensor_tensor(out=tmp4, in0=gsb[:, 0::2], in1=gsb[:, 0::2], op=OP.mult)
                vv = pst.tile([128, 4], F32, tag="vv")
                nc.vector.scalar_tensor_tensor(
                    out=vv, in0=gsb[:, 1::2], scalar=EPS, in1=tmp4,
                    op0=OP.add, op1=OP.subtract)
                # rstd = rsqrt(vv): quake seed + 1 Newton step (3 fused ops)
                bsh = pst.tile([128, 4], I32, tag="bsh")
                nc.vector.tensor_scalar(
                    out=bsh, in0=vv.bitcast(I32), scalar1=1, scalar2=None,
                    op0=OP.logical_shift_right)
                nc.vector.tensor_tensor(out=tmp4.bitcast(I32), in0=magic_sb, in1=bsh, op=OP.subtract)
                nrt = pst.tile([128, 4], F32, tag="nrt")
                for _ in range(1):
                    nc.vector.tensor_tensor(out=nrt, in0=tmp4, in1=tmp4, op=OP.mult)
                    nc.vector.scalar_tensor_tensor(
                        out=nrt, in0=nrt, scalar=-0.5, in1=vv, op0=OP.mult, op1=OP.mult)
                    nc.vector.scalar_tensor_tensor(
                        out=tmp4, in0=nrt, scalar=1.5, in1=tmp4, op0=OP.add, op1=OP.mult)
                ab = pst.tile([128, 8], F32, tag="ab")
                nc.vector.tensor_tensor(out=ab[:, 0:4], in0=tmp4, in1=gnw_sb, op=OP.mult)
                tmp4b = pst.tile([128, 4], F32, tag="tmp4b")
                nc.vector.tensor_tensor(out=tmp4b, in0=gsb[:, 0::2], in1=ab[:, 0:4], op=OP.mult)
                nc.vector.tensor_tensor(out=ab[:, 4:8], in0=gnb_sb, in1=tmp4b, op=OP.subtract)
                for th in range(NT):
                    for cb in range(CB):
                        if os.environ.get("KERNEL_BASE_GN") == "1":
                            eng = nc.gpsimd if (cb % 2 == 1 and not fast_apply) else nc.vector
                            eng.tensor_scalar(
                                out=dst_fn(cb, th), in0=src_sb[:, cb, th * 512:(th + 1) * 512],
                                scalar1=ab[:, cb:cb + 1], scalar2=ab[:, 4 + cb:5 + cb],
                                op0=OP.mult, op1=OP.add)
                        elif fast_apply:
                            # startup-critical: ACT is idle here and faster
                            nc.scalar.activation(
                                out=dst_fn(cb, th),
                                in_=src_sb[:, cb, th * 512:(th + 1) * 512],
                                func=AF.Identity, bias=ab[:, 4 + cb:5 + cb],
                                scale=ab[:, cb:cb + 1])
                        else:
                            nc.gpsimd.tensor_scalar(
                                out=dst_fn(cb, th), in0=src_sb[:, cb, th * 512:(th + 1) * 512],
                                scalar1=ab[:, cb:cb + 1], scalar2=ab[:, 4 + cb:5 + cb],
                                op0=OP.mult, op1=OP.add)

            def emit_input_loads(b, mid_fn=None):
                y_sb = py.tile([128, CB, T], F32, tag="y")
                for cb in range(CB):
                    nc.sync.dma_start(
                        out=y_sb[:, cb, :],
                        in_=y2[b].rearrange("(cb p) t -> p cb t", p=128)[:, cb, :])
                if mid_fn is not None:
                    mid_fn()  # gn consts + k/q weights jump the queue ahead of x
                x_sb = px.tile([128, CB, T], F32, tag="x")
                for cb in range(CB):
                    nc.sync.dma_start(
                        out=x_sb[:, cb, :],
                        in_=x2[b].rearrange("(cb p) t -> p cb t", p=128)[:, cb, :])
                return x_sb, y_sb

            def emit_gn_compute(x_sb, y_sb, fast_apply=False, act_stats=False):
                gny = pgn.tile([128, CB, T], FP8, tag="gn")
                groupnorm(y_sb, lambda cb, th: gny[:, cb, th * 512:(th + 1) * 512], fast_apply)
                gnx = pgn.tile([128, CB, T], FP8, tag="gn")
                groupnorm(x_sb, lambda cb, th: gnx[:, cb, th * 512:(th + 1) * 512], fast_apply,
                          act_stats=act_stats)
                return gnx, gny

            def q_unit(bctx, ob, pool=None):
                """Closure list: 4 DR matmuls computing q for one ob (2 heads)."""
                st8 = {}
                def mk(th, j):
                    def f():
                        gnx = bctx["gn"][0]
                        if j == 0:
                            st8[th] = (ps_mm.tile([128, 512], F32, tag="mm", name="psu")
                                       if pool is None else
                                       pool.tile([128, 512], F32, tag="sc", name="psu"))
                        psq = st8[th]
                        nc.tensor.matmul(
                            psq,
                            wq_sb[:, 2 * j:2 * j + 2, ob * 128:(ob + 1) * 128],
                            gnx[:, 2 * j:2 * j + 2, th * 512:(th + 1) * 512],
                            start=(j == 0), stop=(j == 1), perf_mode=PM.DoubleRow)
                        if j == 1:
                            jj = ob % 2
                            nc.scalar.activation(
                                out=q8_sb[2 * ob][0:64, jj, th * 512:(th + 1) * 512],
                                in_=psq[0:64, :], func=AF.Identity,
                                bias=bq_sb[0:64, ob:ob + 1], scale=1.0 / WSCALE)
                            nc.scalar.activation(
                                out=q8_sb[2 * ob + 1][64:128, jj, th * 512:(th + 1) * 512],
                                in_=psq[64:128, :], func=AF.Identity,
                                bias=bq_sb[64:128, ob:ob + 1], scale=1.0 / WSCALE)
                    return f
                return [mk(th, j) for th in range(NT) for j in range(2)]

            def k_unit(bctx, ob, pool=None):
                st8 = {}
                def mk(th, j):
                    def f():
                        gny = bctx["gn"][1]
                        obp = ob // 2
                        if bctx["k8"][obp] is None:
                            bctx["k8"][obp] = pk.tile([128, 2, T], FP8, tag="k", name="k_obp")
                        k8p = bctx["k8"][obp]
                        if j == 0:
                            st8[th] = (ps_mm.tile([128, 512], F32, tag="mm", name="psu")
                                       if pool is None else
                                       pool.tile([128, 512], F32, tag="sc", name="psu"))
                        psk = st8[th]
                        nc.tensor.matmul(
                            psk,
                            wk_sb[:, 2 * j:2 * j + 2, ob * 128:(ob + 1) * 128],
                            gny[:, 2 * j:2 * j + 2, th * 512:(th + 1) * 512],
                            start=(j == 0), stop=(j == 1), perf_mode=PM.DoubleRow)
                        if j == 1:
                            nc.scalar.activation(
                                out=k8p[:, ob % 2, th * 512:(th + 1) * 512],
                                in_=psk, func=AF.Identity,
                                bias=bk_sb[:, ob:ob + 1], scale=1.0 / WSCALE)
                    return f
                return [mk(th, j) for th in range(NT) for j in range(2)]

            def vt_unit(bctx, tt):
                """Closure list: 2 DR matmuls + bias for one vT seq tile.

                vT tiles hold an st-PAIR: [128, 2, NH, CH+1] bf16."""
                st8 = {}
                pair, sub = tt // 2, tt % 2
                def mk(j):
                    def f():
                        gny = bctx["gn"][1]
                        if j == 0:
                            st8["ps"] = ps_mm.tile([128, 512], F32, tag="mm", name="psu")
                        psv = st8["ps"]
                        nc.tensor.matmul(
                            psv,
                            gny[:, 2 * j:2 * j + 2, tt * 128:(tt + 1) * 128],
                            wv_sb[:, 2 * j:2 * j + 2, :],
                            start=(j == 0), stop=(j == 1), perf_mode=PM.DoubleRow)
                        if j == 1:
                            if bctx["vtp"][pair] is None:
                                bctx["vtp"][pair] = pvt.tile(
                                    [128, 2, NH, CH + 1], BF16, tag="vt", name="vt")
                            vt = bctx["vtp"][pair]
                            nc.vector.tensor_tensor(
                                out=vt[:, sub, :, 0:CH],
                                in0=psv.rearrange("p (h c) -> p h c", h=NH),
                                in1=bv_sb, op=OP.add)
                            nc.vector.tensor_copy(
                                vt[:, sub, :, CH:CH + 1],
                                ones16_sb.rearrange("p (h o) -> p h o", o=1))
                    return f
                return [mk(j) for j in range(2)]

            def pproj_unit(bctx, b, ob, pool=None, split_tail=False, store_q=None,
                           pool_tag="sc", act_resid=False):
                """Closure list: 4 DR matmuls + bias/residual/store for one out block.

                split_tail: j0 group (kb0-1, residual applied early), then a
                separate j1 group + final add + store — shortens the critical
                chain behind the last attention block.
                """
                st8 = {}
                def mk(th, j):
                    def f():
                        xr = bctx["x"]
                        a_sb = bctx["a"]
                        sl = slice(th * 512, (th + 1) * 512)
                        if j == 0 or split_tail:
                            st8[th] = (ps_mm.tile([128, 512], F32, tag="mm", name="psu")
                                       if pool is None else
                                       pool.tile([128, 512], F32, tag=pool_tag, name="psu"))
                        psh = st8[th]
                        nc.tensor.matmul(
                            psh,
                            wp_sb[:, 2 * j:2 * j + 2, ob * 128:(ob + 1) * 128],
                            a_sb[:, 2 * j:2 * j + 2, sl],
                            start=(j == 0 or split_tail),
                            stop=(j == 1 or split_tail),
                            perf_mode=PM.DoubleRow)
                        grp_end = (j == 0) if split_tail else (j == 1)
                        if grp_end:
                            nc.vector.scalar_tensor_tensor(
                                out=xr[:, ob, sl],
                                in0=psh, scalar=bp_sb[:, ob:ob + 1],
                                in1=xr[:, ob, sl], op0=OP.add, op1=OP.add)
                        if j == 1:
                            if split_tail:
                                nc.vector.tensor_tensor(
                                    out=xr[:, ob, sl], in0=psh, in1=xr[:, ob, sl],
                                    op=OP.add)
                            (store_q or nc.sync).dma_start(
                                out=out_d[b].rearrange("(cb p) t -> p cb t", p=128)[:, ob, sl],
                                in_=xr[:, ob, sl])
                    return f
                return [mk(th, j) for th in range(NT) for j in range(2)]

            def attention_head(bctx, ob, hh, qp, a_sb, lazy_vt=False, filler=None):
                """Emit one head's QK/exp slots; AV matmuls trail by one
                st-pair ACROSS head boundaries (bctx["pend"]), so the PE never
                waits on a just-issued exp and the ACT/DVE exp chains continue
                seamlessly from head to head."""
                h = 2 * ob + hh
                k8p = bctx["k8"][ob // 2]
                vtp = bctx["vtp"]
                psa_t = [ps_a0.tile([128, 512], F32, tag="av0", name="psa0"),
                         ps_a1.tile([128, 512], F32, tag="av1", name="psa1")]
                psa = [t[:, 0:4 * (CH + 1)].rearrange("p (a b) -> p a b", b=CH + 1)
                       for t in psa_t]
                bctx.setdefault("pend", [])

                def emit_avs(ctx):
                    hp, pair, wts, psa_p, a_sb_p = ctx
                    first = pair == 0
                    last = pair == NP - 1
                    for th in range(NT):
                        for sub in range(2):
                            for tc4 in range(4):
                                nc.tensor.matmul(
                                    psa_p[th][:, tc4, :],
                                    wts[:, sub, th, tc4 * 128:(tc4 + 1) * 128],
                                    vtp[pair][:, sub, hp, :],
                                    start=(first and sub == 0 and tc4 == 0),
                                    stop=(last and sub == 1 and tc4 == 3),
                                    skip_group_check=True)
                        if filler is not None:
                            filler()
                            filler()
                    if last:
                        finish_head(bctx, (hp // 2), hp % 2, psa_p, a_sb_p)

                hg = bctx["b"] * NH + h
                for pair in range(NP):
                    if lazy_vt and vtp[pair] is None:
                        for tt in (2 * pair, 2 * pair + 1):
                            for f in vt_unit(bctx, tt):
                                f()
                    wts = pwts.tile([128, 2, NT, 512], BF16, tag="wts")
                    for sub in range(2):
                        st = 2 * pair + sub
                        pss = ps_s.tile([128, NT, 512], F32, tag="sc")
                        for th in range(NT):
                            nc.tensor.matmul(
                                pss[:, th, :],
                                k8p[:, :, st * 128:(st + 1) * 128],
                                q8_sb[h][:, :, th * 512:(th + 1) * 512],
                                start=True, stop=True, perf_mode=PM.DoubleRow)
                            if filler is not None:
                                filler()
                        # exp engines alternate by st parity: even sts on ACT
                        # (exact exp), odd sts Schraudolph fast-exp on DVE —
                        # each engine owns one of the two ps_s buffer chains
                        if sub == 1 and hg not in ACT_ODD_HEADS:
                            nc.vector.tensor_scalar(
                                out=wts.bitcast(I16)[:, sub].rearrange("p a b -> p (a b)"),
                                in0=pss.rearrange("p a b -> p (a b)"),
                                scalar1=SCH_A, scalar2=SCH_C,
                                op0=OP.mult, op1=OP.add)
                        else:
                            nc.scalar.activation(
                                out=wts[:, sub].rearrange("p a b -> p (a b)"),
                                in_=pss.rearrange("p a b -> p (a b)"),
                                func=AF.Exp, bias=nbias_sb, scale=1.0)
                    bctx["pend"].append((h, pair, wts, psa, a_sb))
                    if len(bctx["pend"]) > 1:
                        emit_avs(bctx["pend"].pop(0))

            def attention_flush(bctx, filler=None):
                """Drain the trailing AV slot at batch end."""
                for ctx in bctx["pend"]:
                    hp, pair, wts, psa_p, a_sb_p = ctx
                    for th in range(NT):
                        for sub in range(2):
                            for tc4 in range(4):
                                nc.tensor.matmul(
                                    psa_p[th][:, tc4, :],
                                    wts[:, sub, th, tc4 * 128:(tc4 + 1) * 128],
                                    bctx["vtp"][pair][:, sub, hp, :],
                                    start=(pair == 0 and sub == 0 and tc4 == 0),
                                    stop=(pair == NP - 1 and sub == 1 and tc4 == 3),
                                    skip_group_check=True)
                    if pair == NP - 1:
                        finish_head(bctx, hp // 2, hp % 2, psa_p, a_sb_p)
                bctx["pend"] = []

            def finish_head(bctx, ob, hh, psa, a_sb):
                # denominators are per-partition columns now: copy+recip+scale.
                # high priority: these free the single-buffered AV psum banks,
                # so they must win DVE scheduling ties against filler ops.
                with tc.high_priority():
                    _finish_head(bctx, ob, hh, psa, a_sb)

            def _finish_head(bctx, ob, hh, psa, a_sb):
                h = 2 * ob + hh
                aT = bctx["aT"]
                for th in range(NT):
                    rr = pdn3.tile([128, 4], F32, tag="dn3")
                    if os.environ.get("KERNEL_DN_COPY") == "1":
                        dn = pdn.tile([128, 4], F32, tag="r0")
                        nc.vector.tensor_copy(dn, psa[th][:, :, CH])
                        nc.vector.reciprocal_approx_fast(out=rr, in_=dn)
                    else:
                        nc.vector.reciprocal_approx_fast(out=rr, in_=psa[th][:, :, CH])
                    nc.vector.tensor_tensor(
                        out=aT[:, th, :, h, :],
                        in0=psa[th][:, :, 0:CH],
                        in1=rr[:, :, None].broadcast_to([128, 4, CH]),
                        op=OP.mult)

            def finish_ob(bctx, ob, a_sb):
                """Transpose aT[t, c] blocks of channel-block ob back to a[c, t]."""
                aT = bctx["aT"]
                trp = ps_mm.tile([128, ST, 128], BF16, tag="mm", name="trp")
                for tb in range(ST):
                    th, tc4 = tb // 4, tb % 4
                    nc.tensor.matmul(
                        trp[:, tb, :],
                        aT[:, th, tc4, 2 * ob:2 * ob + 2, :].rearrange("p a b -> p (a b)"),
                        id_sb,
                        is_transpose=True)
                nc.scalar.activation(out=a_sb[:, ob, :],
                                     in_=trp.rearrange("p a b -> p (a b)"),
                                     func=AF.Copy)

            # ---------------- batch pipeline ----------------
            from collections import deque
            fillers = deque()

            def filler_pop():
                if fillers:
                    fillers.popleft()()

            def filler_flush():
                while fillers:
                    fillers.popleft()()

            bctxs = [dict() for _ in range(BPC)]
            _xy0 = emit_input_loads(0, mid_fn=emit_gn_consts)
            for wi in range(20):
                trash = ps_mm.tile([128, 64], F32, tag="mm", name="wtr0")
                nc.tensor.matmul(trash, _xy0[0][:, 0, 0:128], _xy0[0][:, 0, 0:64],
                                 start=True, stop=True)
            emit_bias_consts()
            nc.sync.dma_start(out=wq_sb, in_=wqt.rearrange("(kb p) o -> p kb o", p=128))
            nc.sync.dma_start(out=wk_sb, in_=wkt.rearrange("(kb p) o -> p kb o", p=128))
            emit_vp_weight_loads()
            bctxs[0]["x"] = _xy0[0]
            bctxs[0]["gn"] = emit_gn_compute(*_xy0, fast_apply=True, act_stats=True)
            bctxs[0]["k8"] = [None] * (CB // 2)
            bctxs[0]["vtp"] = [None] * NP

            # batch 0 ob0 prep emitted directly; rest queued as fillers that
            # drip into the attention pair slots (1 matmul per QK/AV pair)
            for f in k_unit(bctxs[0], 0, pool=ps_s):
                f()
            for f in q_unit(bctxs[0], 0, pool=ps_s):
                f()
            for ob2 in (1, 2, 3):
                fillers.extend(k_unit(bctxs[0], ob2))
                fillers.extend(q_unit(bctxs[0], ob2))

            for b in range(BPC):
                bctx = bctxs[b]
                bctx["b"] = b
                a_sb = pa.tile([128, CB, T], FP8, tag="a")
                bctx["a"] = a_sb
                bctx["aT"] = pat.tile([128, NT, 4, NH, CH], BF16, tag="aT", name="aT")
                for ob in range(CB):
                    if b > 0:
                        if ob == 0:
                            for ob2 in (1, 2):
                                fillers.extend(k_unit(bctx, ob2))
                                fillers.extend(q_unit(bctx, ob2))
                        if ob == 1:
                            fillers.extend(k_unit(bctx, 3))
                            fillers.extend(q_unit(bctx, 3))
                            for ob2 in range(CB):
                                fillers.extend(pproj_unit(bctxs[b - 1], b - 1, ob2, pool=ps_s))
                    if b + 1 < BPC:
                        if ob == 0:
                            nb = bctxs[b + 1]
                            nb["xy"] = emit_input_loads(b + 1)
                            nb["x"] = nb["xy"][0]
                        if ob == 1:
                            nb = bctxs[b + 1]
                            nb["gn"] = emit_gn_compute(*nb.pop("xy"))
                            nb["k8"] = [None] * (CB // 2)
                            nb["vtp"] = [None] * NP

                    for hh in (0, 1):
                        attention_head(bctx, ob, hh, None, a_sb,
                                       lazy_vt=(b == 0 and ob == 0),
                                       filler=filler_pop)
                        if hh == 0 and b + 1 < BPC:
                            nb = bctxs[b + 1]
                            if ob == 2:
                                fillers.extend(k_unit(nb, 0))
                                for tt in range(ST):
                                    fillers.extend(vt_unit(nb, tt))
                            if ob == 3:
                                fillers.extend(q_unit(nb, 0))
                    if ob > 0:
                        finish_ob(bctx, ob - 1, a_sb)
                    if ob == CB - 1:
                        attention_flush(bctx)
                        finish_ob(bctx, ob, a_sb)

                if b == BPC - 1:
                    # tail: drain queue, then final output projection directly.
                    filler_flush()
                    # dummy matmuls keep the PE clock ramped while the last
                    # head's softmax-normalize chain drains
                    for wi in range(10):
                        trash = ps_mm.tile([128, 512], F32, tag="mm", name="wtr")
                        nc.tensor.matmul(trash, wp_sb[:, 0, 0:128],
                                         a_sb[:, 0, 0:512],
                                         start=True, stop=True)
                    tail_pools = [(ps_a0, "av0"), (ps_a1, "av1"), (ps_s, "sc"), (None, "sc")]
                    for ob2 in range(CB):
                        pl, tg = tail_pools[ob2]
                        for f in pproj_unit(bctx, b, ob2, pool=pl, pool_tag=tg,
                                            store_q=nc.scalar if ob2 % 2 == 0 else nc.sync):
                            f()

    nc.finalize()
    return nc


_NC = None


def _get_nc():
    global _NC
    if _NC is None:
        _NC = _build()
    return _NC


def _prep_inputs(x, y, gn_w, gn_b, Wq, bq, Wkv, bkv, Wp, bp):
    scale = CH ** -0.25
    # reference splits k/v per head: kvh[:, h, :ch] / kvh[:, h, ch:] after
    # reshape to [b, NH, 2*ch, T] -> k_h = Wkv rows [h*128, h*128+64)
    import ml_dtypes
    FP8NP = ml_dtypes.float8_e4m3
    idx_k = np.concatenate([np.arange(h * 2 * CH, h * 2 * CH + CH) for h in range(NH)])
    idx_v = np.concatenate([np.arange(h * 2 * CH + CH, (h + 1) * 2 * CH) for h in range(NH)])
    # Wq/Wk prescaled by WSCALE to keep fp8 values out of denormal range;
    # compensated by 1/WSCALE in the psum->sbuf bias add.
    wqt = np.ascontiguousarray((Wq * (scale * WSCALE)).T).astype(FP8NP)
    wkt = np.ascontiguousarray((Wkv[idx_k] * (scale * WSCALE)).T).astype(FP8NP)
    wvt = np.ascontiguousarray(Wkv[idx_v].T).astype(FP8NP)
    wpt = np.ascontiguousarray(Wp.T).astype(FP8NP)
    bq_s = bq * scale
    bk_s = bkv[idx_k] * scale
    bv = bkv[idx_v]

    def part_layout(v):  # [C] -> [128, CB]: v[cb*128+p]
        return np.ascontiguousarray(v.reshape(CB, 128).T)

    bq_l = part_layout(bq_s)
    bk_l = part_layout(bk_s)
    bp_l = part_layout(bp)
    gnw_l = part_layout(gn_w)
    gnb_l = part_layout(gn_b)
    bv_bc = np.broadcast_to(bv.reshape(1, NH, CH), (128, NH, CH)).copy()
    m1 = np.zeros((128, 128), np.float32)
    for g in range(128 // GSIZE):
        m1[g * GSIZE:(g + 1) * GSIZE, g * GSIZE:(g + 1) * GSIZE] = 1.0 / GSIZE
    id128_h = np.eye(128, dtype=ml_dtypes.bfloat16)

    xf = x.reshape(B, C, T)
    yf = y.reshape(B, C, T)

    shared = {
        "wqt": wqt, "wkt": wkt, "wvt": wvt, "wpt": wpt,
        "bq_l": bq_l, "bk_l": bk_l, "bp_l": bp_l, "bv_bc": bv_bc,
        "gnw_l": gnw_l, "gnb_l": gnb_l, "m1": m1, "id128": id128_h,
    }
    in_maps = []
    for i in range(N_CORES):
        m = dict(shared)
        m["x2"] = np.ascontiguousarray(xf[i * BPC:(i + 1) * BPC])
        m["y2"] = np.ascontiguousarray(yf[i * BPC:(i + 1) * BPC])
        in_maps.append(m)
    return in_maps


def kernel(x, y, gn_w, gn_b, Wq, bq, Wkv, bkv, Wp, bp):
    args = [np.asarray(a, dtype=np.float32) for a in
            (x, y, gn_w, gn_b, Wq, bq, Wkv, bkv, Wp, bp)]
    in_maps = _prep_inputs(*args)
    nc = _get_nc()
    res = run_bass_kernel_spmd(nc, in_maps, core_ids=list(range(N_CORES)))
    out = np.empty((B, C, T), np.float32)
    for i in range(N_CORES):
        out[i * BPC:(i + 1) * BPC] = res.results[i]["out"]
    return out.reshape(B, C, H, W)


# revision 35
# speedup vs baseline: 1.0709x; 1.0311x over previous
"""AttentionBlock Trainium2 Bass kernel.

Data-parallel over batch: 16 batches / 8 cores = 2 per core. Each core runs
the full block (groupnorm x2, q/kv projections, 8-head attention, output
projection, residual) on its 2 batch elements.

Key design points (v2):
- fp8e4m3 DoubleRow matmuls for all four projections (Wq/Wk prescaled x32 to
  escape fp8 denormals, compensated in the psum->sbuf bias add) and for the
  attention*V of st-pairs 1-3: 256-deep contraction at 0.5 cyc/row quarters
  projection PE time and AV PE time vs bf16.
- scores layout [s, t]; exp without max-subtraction but with a constant -4
  logit shift so exp output fits fp8 range (shift cancels in softmax).
- exp engine split: st-pairs 1-3 go to ACT (exp -> fp8 wts), st-pair 0 goes
  to DVE as a Schraudolph fast-exp (one tensor_scalar: i16(round(l*184.66 +
  15511.5)) bitcast bf16, ~3% multiplicative err, cancels mostly in softmax).
  Pair-0 wts/v tiles are bf16; AV for pair 0 runs as plain bf16 matmuls.
- wts/vT tiles hold an st-PAIR each ([128, 2, ...]) so DR matmuls can pair
  the contraction; softmax denominator rides along as a ones column of vT.
- normalize: one broadcast tensor_tensor per (head, th) scales psum by the
  per-partition reciprocal denominators -> aT bf16; PE transpose per channel
  block; psum->sbuf copy converts a to fp8 for the DR output projection.
- groupnorm applies run on GPSIMD (Pool) except the startup-critical batch-0
  pair; stats stay on DVE bn_stats (batch-0 x split ACT/DVE).
- software pipelining: AV matmuls trail their QK pair by one st-pair;
  projection/output matmuls are emitted as 1-matmul closures popped between
  attention slots; next-batch prep is pushed mid-ob.
- startup: y loads -> gn consts -> x loads -> biases -> fp8 weights; PE clock
  pre-warmed with dummy matmuls.
"""
import os
import sys

sys.path.insert(0, "/opt/trn_rl_repo")

import numpy as np

import concourse.bacc as bacc
import concourse.bass as bass
import concourse.tile as tile
from concourse import mybir
from concourse.bass_utils import run_bass_kernel_spmd

F32 = mybir.dt.float32
F32R = mybir.dt.float32r
BF16 = mybir.dt.bfloat16
FP8 = mybir.dt.float8e4
I16 = mybir.dt.int16
I32 = mybir.dt.int32
AF = mybir.ActivationFunctionType
OP = mybir.AluOpType
PM = mybir.MatmulPerfMode

B, C, H, W = 16, 512, 32, 32
T = H * W              # 1024
NH = 8                 # heads
CH = C // NH           # 64
GROUPS = 32
GSIZE = C // GROUPS    # 16 channels per group
EPS = 1e-5
N_CORES = 8
BPC = B // N_CORES     # batches per core
CB = C // 128          # 4 channel blocks
NT = T // 512          # 2 column halves of 512
ST = T // 128          # 8 seq tiles of 128
NP = ST // 2           # 4 st-pairs
WSCALE = 32.0          # Wq/Wk fp8 prescale (keeps weights out of denormals)
SHIFT = -4.0           # logit shift before exp (cancels in softmax)
SCH_A = 184.664375     # 2^7 / ln 2
SCH_C = 15511.5        # 16256 - 0.5 - 5.25 + SHIFT*SCH_A  (tuned Schraudolph)
BF16_PAIRS = (0, 1, 2, 3)   # all wts/v tiles bf16: even sts exp on ACT, odd
                            # sts Schraudolph on DVE — two independent psum
                            # chains so the engines never serialize on ps_s
ACT_ODD_HEADS = (3, 11)     # head-batches whose odd-st chain flips to ACT
                            # (fine engine-load balance)

DEBUG = bool(int(os.environ.get("KERNEL_DEBUG", "0")))


def _build():
    nc = bacc.Bacc(None, target_bir_lowering=False)

    x2 = nc.dram_tensor("x2", (BPC, C, T), F32, kind="ExternalInput")
    y2 = nc.dram_tensor("y2", (BPC, C, T), F32, kind="ExternalInput")
    wqt = nc.dram_tensor("wqt", (C, C), FP8, kind="ExternalInput")
    wkt = nc.dram_tensor("wkt", (C, C), FP8, kind="ExternalInput")
    wvt = nc.dram_tensor("wvt", (C, C), FP8, kind="ExternalInput")
    wpt = nc.dram_tensor("wpt", (C, C), FP8, kind="ExternalInput")
    bq_l = nc.dram_tensor("bq_l", (128, CB), F32, kind="ExternalInput")
    bk_l = nc.dram_tensor("bk_l", (128, CB), F32, kind="ExternalInput")
    bp_l = nc.dram_tensor("bp_l", (128, CB), F32, kind="ExternalInput")
    bv_bc = nc.dram_tensor("bv_bc", (128, NH, CH), F32, kind="ExternalInput")
    gnw_l = nc.dram_tensor("gnw_l", (128, CB), F32, kind="ExternalInput")
    gnb_l = nc.dram_tensor("gnb_l", (128, CB), F32, kind="ExternalInput")
    m1 = nc.dram_tensor("m1", (128, 128), F32, kind="ExternalInput")
    id128 = nc.dram_tensor("id128", (128, 128), BF16, kind="ExternalInput")
    out_d = nc.dram_tensor("out", (BPC, C, T), F32, kind="ExternalOutput")
    if DEBUG:
        dbg_a = nc.dram_tensor("dbg_a", (C, T), F32, kind="ExternalOutput")

    with tile.TileContext(nc) as tc:
        from contextlib import ExitStack
        with ExitStack() as ctx:
            consts = ctx.enter_context(tc.tile_pool(name="consts", bufs=1))
            px = ctx.enter_context(tc.tile_pool(name="px", bufs=2))
            py = ctx.enter_context(tc.tile_pool(name="py", bufs=1))
            pgn = ctx.enter_context(tc.tile_pool(name="pgn", bufs=2))
            pk = ctx.enter_context(tc.tile_pool(name="pk", bufs=4))
            pvt = ctx.enter_context(tc.tile_pool(name="pvt", bufs=int(os.environ.get("KPVT", 2 + NP))))
            pq = ctx.enter_context(tc.tile_pool(name="pq", bufs=4))
            pwts = ctx.enter_context(tc.tile_pool(name="pwts", bufs=int(os.environ.get("KPWTS", 8))))
            pa = ctx.enter_context(tc.tile_pool(name="pa", bufs=2))
            pat = ctx.enter_context(tc.tile_pool(name="pat", bufs=2))
            pdn = ctx.enter_context(tc.tile_pool(name="pdn", bufs=1))
            pdn3 = ctx.enter_context(tc.tile_pool(name="pdn3", bufs=2))
            pst = ctx.enter_context(tc.tile_pool(name="pst", bufs=4))
            ps_mm = ctx.enter_context(tc.tile_pool(name="ps_mm", bufs=2, space="PSUM"))
            ps_s = ctx.enter_context(tc.tile_pool(name="ps_s", bufs=4, space="PSUM"))
            ps_a0 = ctx.enter_context(tc.tile_pool(name="ps_a0", bufs=1, space="PSUM"))
            ps_a1 = ctx.enter_context(tc.tile_pool(name="ps_a1", bufs=1, space="PSUM"))

            # --- constants (weights fp8; DMAs ordered for startup overlap) ---
            wq_sb = consts.tile([128, CB, C], FP8, tag="wq")
            wk_sb = consts.tile([128, CB, C], FP8, tag="wk")
            wv_sb = consts.tile([128, CB, C], FP8, tag="wv")
            wp_sb = consts.tile([128, CB, C], FP8, tag="wp")

            def emit_vp_weight_loads():
                nc.sync.dma_start(out=wv_sb, in_=wvt.rearrange("(kb p) o -> p kb o", p=128))
                nc.sync.dma_start(out=wp_sb, in_=wpt.rearrange("(kb p) o -> p kb o", p=128))

            m1_sb = consts.tile([128, 128], F32, tag="m1")
            bq_sb = consts.tile([128, CB], F32, tag="bq")
            bk_sb = consts.tile([128, CB], F32, tag="bk")
            bp_sb = consts.tile([128, CB], F32, tag="bp")
            bv_sb = consts.tile([128, NH, CH], F32, tag="bv")
            gnw_sb = consts.tile([128, CB], F32, tag="gnw")
            gnb_sb = consts.tile([128, CB], F32, tag="gnb")
            id_sb = consts.tile([128, 128], BF16, tag="id")
            magic_sb = consts.tile([128, CB], I32, tag="magic")
            nc.vector.memset(magic_sb, 0x5f3759df)
            ones8_sb = consts.tile([128, NH], FP8, tag="ones8")
            nc.vector.memset(ones8_sb, 1.0)
            ones16_sb = consts.tile([128, NH], BF16, tag="ones16")
            nc.vector.memset(ones16_sb, 1.0)
            nbias_sb = consts.tile([128, 1], F32, tag="nbias")
            nc.vector.memset(nbias_sb, SHIFT)
            # persistent per-head q tiles, fp8, zero-padded outside the head's
            # 64 channels; zeros are memset once and persist across batches
            q8_sb = [consts.tile([128, 2, T], FP8, tag=f"q8_{h}", name=f"q8_{h}")
                     for h in range(NH)]
            for h in range(NH):
                nc.gpsimd.memset(q8_sb[h], 0.0)
            warm = consts.tile([1, 1], F32, tag="warm")
            nc.vector.memset(warm, 0.0)
            nc.scalar.activation(out=warm, in_=warm, func=AF.Exp)

            def emit_gn_consts():
                nc.sync.dma_start(out=m1_sb, in_=m1[:, :])
                nc.sync.dma_start(out=gnw_sb, in_=gnw_l[:, :])
                nc.sync.dma_start(out=gnb_sb, in_=gnb_l[:, :])

            def emit_bias_consts():
                nc.sync.dma_start(out=bk_sb, in_=bk_l[:, :])
                nc.sync.dma_start(out=bq_sb, in_=bq_l[:, :])
                nc.sync.dma_start(out=bv_sb, in_=bv_bc[:, :, :])
                nc.sync.dma_start(out=bp_sb, in_=bp_l[:, :])
                nc.sync.dma_start(out=id_sb, in_=id128[:, :])

            sched_state = {"sch_i": 0}

            def groupnorm(src_sb, dst_fn, fast_apply=False, act_stats=False):
                """src_sb: [128, CB, T] f32. dst_fn(cb, th)->AP (fp8 out)."""
                mv = pst.tile([128, CB, 2], F32, tag="mv")
                if act_stats:
                    # split stats: ACT (Copy/Square accum) covers cb0-1 while
                    # DVE bn_stats covers cb2-3 — halves the serial latency on
                    # the startup-critical tensor.
                    part = pst.tile([128, 2, 2, 2], F32, tag="part")
                    for si, (func, scale) in enumerate(
                            ((AF.Copy, 1.0 / T), (AF.Square, 1.0 / np.sqrt(T)))):
                        for cb in range(2):
                            for c2 in range(2):
                                trash = pwts.tile([128, 512], BF16, tag="trash", name="trash")
                                nc.scalar.activation(
                                    out=trash, in_=src_sb[:, cb, c2 * 512:(c2 + 1) * 512],
                                    func=func, scale=scale,
                                    accum_out=part[:, cb, si, c2:c2 + 1])
                    stats6b = pst.tile([128, 2, 6], F32, tag="stats6b")
                    for cb in (2, 3):
                        for c2 in range(2):
                            nc.vector.bn_stats(
                                out=stats6b[:, c2, :],
                                in_=src_sb[:, cb, c2 * 512:(c2 + 1) * 512])
                        nc.vector.bn_aggr(out=mv[:, cb, :], in_=stats6b)
                    # cb0-1: mv = (mean, E[x^2]) from the two half-col accums
                    nc.vector.tensor_tensor(
                        out=mv[:, 0:2, :].rearrange("p a b -> p (a b)"),
                        in0=part[:, :, :, 0].rearrange("p a b -> p (a b)"),
                        in1=part[:, :, :, 1].rearrange("p a b -> p (a b)"), op=OP.add)
                    # cb2-3: convert var -> E[x^2] in place
                    musqb = pst.tile([128, 2], F32, tag="musqb")
                    nc.vector.tensor_tensor(out=musqb, in0=mv[:, 2:4, 0], in1=mv[:, 2:4, 0], op=OP.mult)
                    nc.vector.tensor_tensor(out=mv[:, 2:4, 1], in0=musqb, in1=mv[:, 2:4, 1], op=OP.add)
                else:
                    stats6 = pst.tile([128, 2, 6], F32, tag="stats6")
                    for cb in range(CB):
                        for c2 in range(2):
                            nc.vector.bn_stats(
                                out=stats6[:, c2, :],
                                in_=src_sb[:, cb, c2 * 512:(c2 + 1) * 512])
                        nc.vector.bn_aggr(out=mv[:, cb, :], in_=stats6)
                    # m2 slot in-place: mv[:,:,1] = var + mean^2
                    musq = pst.tile([128, 4], F32, tag="musq")
                    nc.vector.tensor_tensor(out=musq, in0=mv[:, :, 0], in1=mv[:, :, 0], op=OP.mult)
                    nc.vector.tensor_tensor(out=mv[:, :, 1], in0=musq, in1=mv[:, :, 1], op=OP.add)
                psg = ps_s.tile([128, 8], F32, tag="sc", name="psg")
                nc.tensor.matmul(psg, m1_sb, mv.rearrange("p a b -> p (a b)"), start=True, stop=True)
                gsb = pst.tile([128, 8], F32, tag="gsb")
                nc.vector.tensor_copy(gsb, psg)  # m1 carries 1/GSIZE; cols interleaved (mean, m2)
                # var + eps = (m2 + eps) - mean^2, fused
                tmp4 = pst.tile([128, 4], F32, tag="tmp4")
                nc.vector.tensor_tensor(out=tmp4, in0=gsb[:, 0::2], in1=gsb[:, 0::2], op=OP.mult)
                vv = pst.tile([128, 4], F32, tag="vv")
                nc.vector.scalar_tensor_tensor(
                    out=vv, in0=gsb[:, 1::2], scalar=EPS, in1=tmp4,
                    op0=OP.add, op1=OP.subtract)
                # rstd = rsqrt(vv): quake seed + 1 Newton step (3 fused ops)
                bsh = pst.tile([128, 4], I32, tag="bsh")
                nc.vector.tensor_scalar(
                    out=bsh, in0=vv.bitcast(I32), scalar1=1, scalar2=None,
                    op0=OP.logical_shift_right)
                nc.vector.tensor_tensor(out=tmp4.bitcast(I32), in0=magic_sb, in1=bsh, op=OP.subtract)
                nrt = pst.tile([128, 4], F32, tag="nrt")
                for _ in range(1):
                    nc.vector.tensor_tensor(out=nrt, in0=tmp4, in1=tmp4, op=OP.mult)
                    nc.vector.scalar_tensor_tensor(
                        out=nrt, in0=nrt, scalar=-0.5, in1=vv, op0=OP.mult, op1=OP.mult)
                    nc.vector.scalar_tensor_tensor(
                        out=tmp4, in0=nrt, scalar=1.5, in1=tmp4, op0=OP.add, op1=OP.mult)
                ab = pst.tile([128, 8], F32, tag="ab")
                nc.vector.tensor_tensor(out=ab[:, 0:4], in0=tmp4, in1=gnw_sb, op=OP.mult)
                tmp4b = pst.tile([128, 4], F32, tag="tmp4b")
                nc.vector.tensor_tensor(out=tmp4b, in0=gsb[:, 0::2], in1=ab[:, 0:4], op=OP.mult)
                nc.vector.tensor_tensor(out=ab[:, 4:8], in0=gnb_sb, in1=tmp4b, op=OP.subtract)
                for th in range(NT):
                    for cb in range(CB):
                        if os.environ.get("KERNEL_BASE_GN") == "1":
                            eng = nc.gpsimd if (cb % 2 == 1 and not fast_apply) else nc.vector
                            eng.tensor_scalar(
                                out=dst_fn(cb, th), in0=src_sb[:, cb, th * 512:(th + 1) * 512],
                                scalar1=ab[:, cb:cb + 1], scalar2=ab[:, 4 + cb:5 + cb],
                                op0=OP.mult, op1=OP.add)
                        elif fast_apply:
                            # startup-critical: ACT is idle here and faster
                            nc.scalar.activation(
                                out=dst_fn(cb, th),
                                in_=src_sb[:, cb, th * 512:(th + 1) * 512],
                                func=AF.Identity, bias=ab[:, 4 + cb:5 + cb],
                                scale=ab[:, cb:cb + 1])
                        else:
                            nc.gpsimd.tensor_scalar(
                                out=dst_fn(cb, th), in0=src_sb[:, cb, th * 512:(th + 1) * 512],
                                scalar1=ab[:, cb:cb + 1], scalar2=ab[:, 4 + cb:5 + cb],
                                op0=OP.mult, op1=OP.add)

            def emit_input_loads(b, mid_fn=None):
                y_sb = py.tile([128, CB, T], F32, tag="y")
                for cb in range(CB):
                    nc.sync.dma_start(
                        out=y_sb[:, cb, :],
                        in_=y2[b].rearrange("(cb p) t -> p cb t", p=128)[:, cb, :])
                if mid_fn is not None:
                    mid_fn()  # gn consts + k/q weights jump the queue ahead of x
                x_sb = px.tile([128, CB, T], F32, tag="x")
                for cb in range(CB):
                    nc.sync.dma_start(
                        out=x_sb[:, cb, :],
                        in_=x2[b].rearrange("(cb p) t -> p cb t", p=128)[:, cb, :])
                return x_sb, y_sb

            def emit_gn_compute(x_sb, y_sb, fast_apply=False, act_stats=False):
                gny = pgn.tile([128, CB, T], FP8, tag="gn")
                groupnorm(y_sb, lambda cb, th: gny[:, cb, th * 512:(th + 1) * 512], fast_apply)
                gnx = pgn.tile([128, CB, T], FP8, tag="gn")
                groupnorm(x_sb, lambda cb, th: gnx[:, cb, th * 512:(th + 1) * 512], fast_apply,
                          act_stats=act_stats)
                return gnx, gny

            def q_unit(bctx, ob, pool=None):
                """Closure list: 4 DR matmuls computing q for one ob (2 heads)."""
                st8 = {}
                def mk(th, j):
                    def f():
                        gnx = bctx["gn"][0]
                        if j == 0:
                            st8[th] = (ps_mm.tile([128, 512], F32, tag="mm", name="psu")
                                       if pool is None else
                                       pool.tile([128, 512], F32, tag="sc", name="psu"))
                        psq = st8[th]
                        nc.tensor.matmul(
                            psq,
                            wq_sb[:, 2 * j:2 * j + 2, ob * 128:(ob + 1) * 128],
                            gnx[:, 2 * j:2 * j + 2, th * 512:(th + 1) * 512],
                            start=(j == 0), stop=(j == 1), perf_mode=PM.DoubleRow)
                        if j == 1:
                            jj = ob % 2
                            nc.scalar.activation(
                                out=q8_sb[2 * ob][0:64, jj, th * 512:(th + 1) * 512],
                                in_=psq[0:64, :], func=AF.Identity,
                                bias=bq_sb[0:64, ob:ob + 1], scale=1.0 / WSCALE)
                            nc.scalar.activation(
                                out=q8_sb[2 * ob + 1][64:128, jj, th * 512:(th + 1) * 512],
                                in_=psq[64:128, :], func=AF.Identity,
                                bias=bq_sb[64:128, ob:ob + 1], scale=1.0 / WSCALE)
                    return f
                return [mk(th, j) for th in range(NT) for j in range(2)]

            def k_unit(bctx, ob, pool=None):
                st8 = {}
                def mk(th, j):
                    def f():
                        gny = bctx["gn"][1]
                        obp = ob // 2
                        if bctx["k8"][obp] is None:
                            bctx["k8"][obp] = pk.tile([128, 2, T], FP8, tag="k", name="k_obp")
                        k8p = bctx["k8"][obp]
                        if j == 0:
                            st8[th] = (ps_mm.tile([128, 512], F32, tag="mm", name="psu")
                                       if pool is None else
                                       pool.tile([128, 512], F32, tag="sc", name="psu"))
                        psk = st8[th]
                        nc.tensor.matmul(
                            psk,
                            wk_sb[:, 2 * j:2 * j + 2, ob * 128:(ob + 1) * 128],
                            gny[:, 2 * j:2 * j + 2, th * 512:(th + 1) * 512],
                            start=(j == 0), stop=(j == 1), perf_mode=PM.DoubleRow)
                        if j == 1:
                            nc.scalar.activation(
                                out=k8p[:, ob % 2, th * 512:(th + 1) * 512],
                                in_=psk, func=AF.Identity,
                                bias=bk_sb[:, ob:ob + 1], scale=1.0 / WSCALE)
                    return f
                return [mk(th, j) for th in range(NT) for j in range(2)]

            def vt_unit(bctx, tt):
                """Closure list: 2 DR matmuls + bias for one vT seq tile.

                vT tiles hold an st-PAIR: [128, 2, NH, CH+1] bf16."""
                st8 = {}
                pair, sub = tt // 2, tt % 2
                def mk(j):
                    def f():
                        gny = bctx["gn"][1]
                        if j == 0:
                            st8["ps"] = ps_mm.tile([128, 512], F32, tag="mm", name="psu")
                        psv = st8["ps"]
                        nc.tensor.matmul(
                            psv,
                            gny[:, 2 * j:2 * j + 2, tt * 128:(tt + 1) * 128],
                            wv_sb[:, 2 * j:2 * j + 2, :],
                            start=(j == 0), stop=(j == 1), perf_mode=PM.DoubleRow)
                        if j == 1:
                            if bctx["vtp"][pair] is None:
                                bctx["vtp"][pair] = pvt.tile(
                                    [128, 2, NH, CH + 1], BF16, tag="vt", name="vt")
                            vt = bctx["vtp"][pair]
                            nc.vector.tensor_tensor(
                                out=vt[:, sub, :, 0:CH],
                                in0=psv.rearrange("p (h c) -> p h c", h=NH),
                                in1=bv_sb, op=OP.add)
                            nc.vector.tensor_copy(
                                vt[:, sub, :, CH:CH + 1],
                                ones16_sb.rearrange("p (h o) -> p h o", o=1))
                    return f
                return [mk(j) for j in range(2)]

            def pproj_unit(bctx, b, ob, pool=None, split_tail=False, store_q=None,
                           pool_tag="sc", act_resid=False):
                """Closure list: 4 DR matmuls + bias/residual/store for one out block.

                split_tail: j0 group (kb0-1, residual applied early), then a
                separate j1 group + final add + store — shortens the critical
                chain behind the last attention block.
                """
                st8 = {}
                def mk(th, j):
                    def f():
                        xr = bctx["x"]
                        a_sb = bctx["a"]
                        sl = slice(th * 512, (th + 1) * 512)
                        if j == 0 or split_tail:
                            st8[th] = (ps_mm.tile([128, 512], F32, tag="mm", name="psu")
                                       if pool is None else
                                       pool.tile([128, 512], F32, tag=pool_tag, name="psu"))
                        psh = st8[th]
                        nc.tensor.matmul(
                            psh,
                            wp_sb[:, 2 * j:2 * j + 2, ob * 128:(ob + 1) * 128],
                            a_sb[:, 2 * j:2 * j + 2, sl],
                            start=(j == 0 or split_tail),
                            stop=(j == 1 or split_tail),
                            perf_mode=PM.DoubleRow)
                        grp_end = (j == 0) if split_tail else (j == 1)
                        if grp_end:
                            nc.vector.scalar_tensor_tensor(
                                out=xr[:, ob, sl],
                                in0=psh, scalar=bp_sb[:, ob:ob + 1],
                                in1=xr[:, ob, sl], op0=OP.add, op1=OP.add)
                        if j == 1:
                            if split_tail:
                                nc.vector.tensor_tensor(
                                    out=xr[:, ob, sl], in0=psh, in1=xr[:, ob, sl],
                                    op=OP.add)
                            (store_q or nc.sync).dma_start(
                                out=out_d[b].rearrange("(cb p) t -> p cb t", p=128)[:, ob, sl],
                                in_=xr[:, ob, sl])
                    return f
                return [mk(th, j) for th in range(NT) for j in range(2)]

            def attention_head(bctx, ob, hh, qp, a_sb, lazy_vt=False, filler=None):
                """Emit one head's QK/exp slots; AV matmuls trail by one
                st-pair ACROSS head boundaries (bctx["pend"]), so the PE never
                waits on a just-issued exp and the ACT/DVE exp chains continue
                seamlessly from head to head."""
                h = 2 * ob + hh
                k8p = bctx["k8"][ob // 2]
                vtp = bctx["vtp"]
                psa_t = [ps_a0.tile([128, 512], F32, tag="av0", name="psa0"),
                         ps_a1.tile([128, 512], F32, tag="av1", name="psa1")]
                psa = [t[:, 0:4 * (CH + 1)].rearrange("p (a b) -> p a b", b=CH + 1)
                       for t in psa_t]
                bctx.setdefault("pend", [])

                def emit_avs(ctx):
                    hp, pair, wts, psa_p, a_sb_p = ctx
                    first = pair == 0
                    last = pair == NP - 1
                    for th in range(NT):
                        for sub in range(2):
                            for tc4 in range(4):
                                nc.tensor.matmul(
                                    psa_p[th][:, tc4, :],
                                    wts[:, sub, th, tc4 * 128:(tc4 + 1) * 128],
                                    vtp[pair][:, sub, hp, :],
                                    start=(first and sub == 0 and tc4 == 0),
                                    stop=(last and sub == 1 and tc4 == 3),
                                    skip_group_check=True)
                        if filler is not None:
                            filler()
                            filler()
                    if last:
                        finish_head(bctx, (hp // 2), hp % 2, psa_p, a_sb_p)

                hg = bctx["b"] * NH + h
                for pair in range(NP):
                    if lazy_vt and vtp[pair] is None:
                        for tt in (2 * pair, 2 * pair + 1):
                            for f in vt_unit(bctx, tt):
                                f()
                    wts = pwts.tile([128, 2, NT, 512], BF16, tag="wts")
                    for sub in range(2):
                        st = 2 * pair + sub
                        # half-width score tiles, 4 psum chains: each engine
                        # owns two (st-parity x th), so one chain's exp can
                        # run while the other waits its QK round-trip
                        for th in range(NT):
                            pss = ps_s.tile([128, 512], F32, tag="sc")
                            nc.tensor.matmul(
                                pss,
                                k8p[:, :, st * 128:(st + 1) * 128],
                                q8_sb[h][:, :, th * 512:(th + 1) * 512],
                                start=True, stop=True, perf_mode=PM.DoubleRow)
                            if filler is not None:
                                filler()
                            if sub == 1 and hg not in ACT_ODD_HEADS:
                                nc.vector.tensor_scalar(
                                    out=wts.bitcast(I16)[:, sub, th, :],
                                    in0=pss,
                                    scalar1=SCH_A, scalar2=SCH_C,
                                    op0=OP.mult, op1=OP.add)
                            else:
                                nc.scalar.activation(
                                    out=wts[:, sub, th, :],
                                    in_=pss,
                                    func=AF.Exp, bias=nbias_sb, scale=1.0)
                    bctx["pend"].append((h, pair, wts, psa, a_sb))
                    if len(bctx["pend"]) > 1:
                        emit_avs(bctx["pend"].pop(0))

            def attention_flush(bctx, filler=None):
                """Drain the trailing AV slot at batch end."""
                for ctx in bctx["pend"]:
                    hp, pair, wts, psa_p, a_sb_p = ctx
                    for th in range(NT):
                        for sub in range(2):
                            for tc4 in range(4):
                                nc.tensor.matmul(
                                    psa_p[th][:, tc4, :],
                                    wts[:, sub, th, tc4 * 128:(tc4 + 1) * 128],
                                    bctx["vtp"][pair][:, sub, hp, :],
                                    start=(pair == 0 and sub == 0 and tc4 == 0),
                                    stop=(pair == NP - 1 and sub == 1 and tc4 == 3),
                                    skip_group_check=True)
                    if pair == NP - 1:
                        finish_head(bctx, hp // 2, hp % 2, psa_p, a_sb_p)
                bctx["pend"] = []

            def finish_head(bctx, ob, hh, psa, a_sb):
                # denominators are per-partition columns now: copy+recip+scale.
                # high priority: these free the single-buffered AV psum banks,
                # so they must win DVE scheduling ties against filler ops.
                with tc.high_priority():
                    _finish_head(bctx, ob, hh, psa, a_sb)

            def _finish_head(bctx, ob, hh, psa, a_sb):
                h = 2 * ob + hh
                aT = bctx["aT"]
                for th in range(NT):
                    rr = pdn3.tile([128, 4], F32, tag="dn3")
                    if os.environ.get("KERNEL_DN_COPY") == "1":
                        dn = pdn.tile([128, 4], F32, tag="r0")
                        nc.vector.tensor_copy(dn, psa[th][:, :, CH])
                        nc.vector.reciprocal_approx_fast(out=rr, in_=dn)
                    else:
                        nc.vector.reciprocal_approx_fast(out=rr, in_=psa[th][:, :, CH])
                    nc.vector.tensor_tensor(
                        out=aT[:, th, :, h, :],
                        in0=psa[th][:, :, 0:CH],
                        in1=rr[:, :, None].broadcast_to([128, 4, CH]),
                        op=OP.mult)

            def finish_ob(bctx, ob, a_sb):
                """Transpose aT[t, c] blocks of channel-block ob back to a[c, t]."""
                aT = bctx["aT"]
                trp = ps_mm.tile([128, ST, 128], BF16, tag="mm", name="trp")
                for tb in range(ST):
                    th, tc4 = tb // 4, tb % 4
                    nc.tensor.matmul(
                        trp[:, tb, :],
                        aT[:, th, tc4, 2 * ob:2 * ob + 2, :].rearrange("p a b -> p (a b)"),
                        id_sb,
                        is_transpose=True)
                nc.scalar.activation(out=a_sb[:, ob, :],
                                     in_=trp.rearrange("p a b -> p (a b)"),
                                     func=AF.Copy)

            # ---------------- batch pipeline ----------------
            from collections import deque
            fillers = deque()

            def filler_pop():
                if fillers:
                    fillers.popleft()()

            def filler_flush():
                while fillers:
                    fillers.popleft()()

            bctxs = [dict() for _ in range(BPC)]
            _xy0 = emit_input_loads(0, mid_fn=emit_gn_consts)
            for wi in range(20):
                trash = ps_mm.tile([128, 64], F32, tag="mm", name="wtr0")
                nc.tensor.matmul(trash, _xy0[0][:, 0, 0:128], _xy0[0][:, 0, 0:64],
                                 start=True, stop=True)
            emit_bias_consts()
            nc.sync.dma_start(out=wq_sb, in_=wqt.rearrange("(kb p) o -> p kb o", p=128))
            nc.sync.dma_start(out=wk_sb, in_=wkt.rearrange("(kb p) o -> p kb o", p=128))
            emit_vp_weight_loads()
            bctxs[0]["x"] = _xy0[0]
            bctxs[0]["gn"] = emit_gn_compute(*_xy0, fast_apply=True, act_stats=True)
            bctxs[0]["k8"] = [None] * (CB // 2)
            bctxs[0]["vtp"] = [None] * NP

            # batch 0 ob0 prep emitted directly; rest queued as fillers that
            # drip into the attention pair slots (1 matmul per QK/AV pair)
            for f in k_unit(bctxs[0], 0, pool=ps_s):
                f()
            for f in q_unit(bctxs[0], 0, pool=ps_s):
                f()
            for ob2 in (1, 2, 3):
                fillers.extend(k_unit(bctxs[0], ob2))
                fillers.extend(q_unit(bctxs[0], ob2))

            for b in range(BPC):
                bctx = bctxs[b]
                bctx["b"] = b
                a_sb = pa.tile([128, CB, T], FP8, tag="a")
                bctx["a"] = a_sb
                bctx["aT"] = pat.tile([128, NT, 4, NH, CH], BF16, tag="aT", name="aT")
                for ob in range(CB):
                    if b > 0:
                        if ob == 0:
                            for ob2 in (1, 2):
                                fillers.extend(k_unit(bctx, ob2))
                                fillers.extend(q_unit(bctx, ob2))
                        if ob == 1:
                            fillers.extend(k_unit(bctx, 3))
                            fillers.extend(q_unit(bctx, 3))
                            for ob2 in range(CB):
                                fillers.extend(pproj_unit(bctxs[b - 1], b - 1, ob2, pool=ps_s))
                    if b + 1 < BPC:
                        if ob == 0:
                            nb = bctxs[b + 1]
                            nb["xy"] = emit_input_loads(b + 1)
                            nb["x"] = nb["xy"][0]
                        if ob == 1:
                            nb = bctxs[b + 1]
                            nb["gn"] = emit_gn_compute(*nb.pop("xy"))
                            nb["k8"] = [None] * (CB // 2)
                            nb["vtp"] = [None] * NP

                    for hh in (0, 1):
                        attention_head(bctx, ob, hh, None, a_sb,
                                       lazy_vt=(b == 0 and ob == 0),
                                       filler=filler_pop)
                        if hh == 0 and b + 1 < BPC:
                            nb = bctxs[b + 1]
                            if ob == 2:
                                fillers.extend(k_unit(nb, 0))
                                for tt in range(ST):
                                    fillers.extend(vt_unit(nb, tt))
                            if ob == 3:
                                fillers.extend(q_unit(nb, 0))
                    if ob > 0:
                        finish_ob(bctx, ob - 1, a_sb)
                    if ob == CB - 1:
                        attention_flush(bctx)
                        finish_ob(bctx, ob, a_sb)

                if b == BPC - 1:
                    # tail: drain queue, then final output projection directly.
                    filler_flush()
                    # dummy matmuls keep the PE clock ramped while the last
                    # head's softmax-normalize chain drains
                    for wi in range(10):
                        trash = ps_mm.tile([128, 512], F32, tag="mm", name="wtr")
                        nc.tensor.matmul(trash, wp_sb[:, 0, 0:128],
                                         a_sb[:, 0, 0:512],
                                         start=True, stop=True)
                    tail_pools = [(ps_a0, "av0"), (ps_a1, "av1"), (ps_s, "sc"), (None, "sc")]
                    for ob2 in range(CB):
                        pl, tg = tail_pools[ob2]
                        for f in pproj_unit(bctx, b, ob2, pool=pl, pool_tag=tg,
                                            store_q=nc.scalar if ob2 % 2 == 0 else nc.sync):
                            f()

    nc.finalize()
    return nc


_NC = None


def _get_nc():
    global _NC
    if _NC is None:
        _NC = _build()
    return _NC


def _prep_inputs(x, y, gn_w, gn_b, Wq, bq, Wkv, bkv, Wp, bp):
    scale = CH ** -0.25
    # reference splits k/v per head: kvh[:, h, :ch] / kvh[:, h, ch:] after
    # reshape to [b, NH, 2*ch, T] -> k_h = Wkv rows [h*128, h*128+64)
    import ml_dtypes
    FP8NP = ml_dtypes.float8_e4m3
    idx_k = np.concatenate([np.arange(h * 2 * CH, h * 2 * CH + CH) for h in range(NH)])
    idx_v = np.concatenate([np.arange(h * 2 * CH + CH, (h + 1) * 2 * CH) for h in range(NH)])
    # Wq/Wk prescaled by WSCALE to keep fp8 values out of denormal range;
    # compensated by 1/WSCALE in the psum->sbuf bias add.
    wqt = np.ascontiguousarray((Wq * (scale * WSCALE)).T).astype(FP8NP)
    wkt = np.ascontiguousarray((Wkv[idx_k] * (scale * WSCALE)).T).astype(FP8NP)
    wvt = np.ascontiguousarray(Wkv[idx_v].T).astype(FP8NP)
    wpt = np.ascontiguousarray(Wp.T).astype(FP8NP)
    bq_s = bq * scale
    bk_s = bkv[idx_k] * scale
    bv = bkv[idx_v]

    def part_layout(v):  # [C] -> [128, CB]: v[cb*128+p]
        return np.ascontiguousarray(v.reshape(CB, 128).T)

    bq_l = part_layout(bq_s)
    bk_l = part_layout(bk_s)
    bp_l = part_layout(bp)
    gnw_l = part_layout(gn_w)
    gnb_l = part_layout(gn_b)
    bv_bc = np.broadcast_to(bv.reshape(1, NH, CH), (128, NH, CH)).copy()
    m1 = np.zeros((128, 128), np.float32)
    for g in range(128 // GSIZE):
        m1[g * GSIZE:(g + 1) * GSIZE, g * GSIZE:(g + 1) * GSIZE] = 1.0 / GSIZE
    id128_h = np.eye(128, dtype=ml_dtypes.bfloat16)

    xf = x.reshape(B, C, T)
    yf = y.reshape(B, C, T)

    shared = {
        "wqt": wqt, "wkt": wkt, "wvt": wvt, "wpt": wpt,
        "bq_l": bq_l, "bk_l": bk_l, "bp_l": bp_l, "bv_bc": bv_bc,
        "gnw_l": gnw_l, "gnb_l": gnb_l, "m1": m1, "id128": id128_h,
    }
    in_maps = []
    for i in range(N_CORES):
        m = dict(shared)
        m["x2"] = np.ascontiguousarray(xf[i * BPC:(i + 1) * BPC])
        m["y2"] = np.ascontiguousarray(yf[i * BPC:(i + 1) * BPC])
        in_maps.append(m)
    return in_maps


def kernel(x, y, gn_w, gn_b, Wq, bq, Wkv, bkv, Wp, bp):
    args = [np.asarray(a, dtype=np.float32) for a in
            (x, y, gn_w, gn_b, Wq, bq, Wkv, bkv, Wp, bp)]
    in_maps = _prep_inputs(*args)
    nc = _get_nc()
    res = run_bass_kernel_spmd(nc, in_maps, core_ids=list(range(N_CORES)))
    out = np.empty((B, C, T), np.float32)
    for i in range(N_CORES):
        out[i * BPC:(i + 1) * BPC] = res.results[i]["out"]
    return out.reshape(B, C, H, W)


# revision 36
# speedup vs baseline: 1.0728x; 1.0017x over previous
"""AttentionBlock Trainium2 Bass kernel.

Data-parallel over batch: 16 batches / 8 cores = 2 per core. Each core runs
the full block (groupnorm x2, q/kv projections, 8-head attention, output
projection, residual) on its 2 batch elements.

Key design points (v2):
- fp8e4m3 DoubleRow matmuls for all four projections (Wq/Wk prescaled x32 to
  escape fp8 denormals, compensated in the psum->sbuf bias add) and for the
  attention*V of st-pairs 1-3: 256-deep contraction at 0.5 cyc/row quarters
  projection PE time and AV PE time vs bf16.
- scores layout [s, t]; exp without max-subtraction but with a constant -4
  logit shift so exp output fits fp8 range (shift cancels in softmax).
- exp engine split: st-pairs 1-3 go to ACT (exp -> fp8 wts), st-pair 0 goes
  to DVE as a Schraudolph fast-exp (one tensor_scalar: i16(round(l*184.66 +
  15511.5)) bitcast bf16, ~3% multiplicative err, cancels mostly in softmax).
  Pair-0 wts/v tiles are bf16; AV for pair 0 runs as plain bf16 matmuls.
- wts/vT tiles hold an st-PAIR each ([128, 2, ...]) so DR matmuls can pair
  the contraction; softmax denominator rides along as a ones column of vT.
- normalize: one broadcast tensor_tensor per (head, th) scales psum by the
  per-partition reciprocal denominators -> aT bf16; PE transpose per channel
  block; psum->sbuf copy converts a to fp8 for the DR output projection.
- groupnorm applies run on GPSIMD (Pool) except the startup-critical batch-0
  pair; stats stay on DVE bn_stats (batch-0 x split ACT/DVE).
- software pipelining: AV matmuls trail their QK pair by one st-pair;
  projection/output matmuls are emitted as 1-matmul closures popped between
  attention slots; next-batch prep is pushed mid-ob.
- startup: y loads -> gn consts -> x loads -> biases -> fp8 weights; PE clock
  pre-warmed with dummy matmuls.
"""
import os
import sys

sys.path.insert(0, "/opt/trn_rl_repo")

import numpy as np

import concourse.bacc as bacc
import concourse.bass as bass
import concourse.tile as tile
from concourse import mybir
from concourse.bass_utils import run_bass_kernel_spmd

F32 = mybir.dt.float32
F32R = mybir.dt.float32r
BF16 = mybir.dt.bfloat16
FP8 = mybir.dt.float8e4
I16 = mybir.dt.int16
I32 = mybir.dt.int32
AF = mybir.ActivationFunctionType
OP = mybir.AluOpType
PM = mybir.MatmulPerfMode

B, C, H, W = 16, 512, 32, 32
T = H * W              # 1024
NH = 8                 # heads
CH = C // NH           # 64
GROUPS = 32
GSIZE = C // GROUPS    # 16 channels per group
EPS = 1e-5
N_CORES = 8
BPC = B // N_CORES     # batches per core
CB = C // 128          # 4 channel blocks
NT = T // 512          # 2 column halves of 512
ST = T // 128          # 8 seq tiles of 128
NP = ST // 2           # 4 st-pairs
WSCALE = 32.0          # Wq/Wk fp8 prescale (keeps weights out of denormals)
SHIFT = -4.0           # logit shift before exp (cancels in softmax)
SCH_A = 184.664375     # 2^7 / ln 2
SCH_C = 15511.5        # 16256 - 0.5 - 5.25 + SHIFT*SCH_A  (tuned Schraudolph)
BF16_PAIRS = (0, 1, 2, 3)   # all wts/v tiles bf16: even sts exp on ACT, odd
                            # sts Schraudolph on DVE — two independent psum
                            # chains so the engines never serialize on ps_s
ACT_ODD_HEADS = (11,)       # head-batches whose odd-st chain flips to ACT
                            # (fine engine-load balance)

DEBUG = bool(int(os.environ.get("KERNEL_DEBUG", "0")))


def _build():
    nc = bacc.Bacc(None, target_bir_lowering=False)

    x2 = nc.dram_tensor("x2", (BPC, C, T), F32, kind="ExternalInput")
    y2 = nc.dram_tensor("y2", (BPC, C, T), F32, kind="ExternalInput")
    wqt = nc.dram_tensor("wqt", (C, C), FP8, kind="ExternalInput")
    wkt = nc.dram_tensor("wkt", (C, C), FP8, kind="ExternalInput")
    wvt = nc.dram_tensor("wvt", (C, C), FP8, kind="ExternalInput")
    wpt = nc.dram_tensor("wpt", (C, C), FP8, kind="ExternalInput")
    bq_l = nc.dram_tensor("bq_l", (128, CB), F32, kind="ExternalInput")
    bk_l = nc.dram_tensor("bk_l", (128, CB), F32, kind="ExternalInput")
    bp_l = nc.dram_tensor("bp_l", (128, CB), F32, kind="ExternalInput")
    bv_bc = nc.dram_tensor("bv_bc", (128, NH, CH), F32, kind="ExternalInput")
    gnw_l = nc.dram_tensor("gnw_l", (128, CB), F32, kind="ExternalInput")
    gnb_l = nc.dram_tensor("gnb_l", (128, CB), F32, kind="ExternalInput")
    m1 = nc.dram_tensor("m1", (128, 128), F32, kind="ExternalInput")
    id128 = nc.dram_tensor("id128", (128, 128), BF16, kind="ExternalInput")
    out_d = nc.dram_tensor("out", (BPC, C, T), F32, kind="ExternalOutput")
    if DEBUG:
        dbg_a = nc.dram_tensor("dbg_a", (C, T), F32, kind="ExternalOutput")

    with tile.TileContext(nc) as tc:
        from contextlib import ExitStack
        with ExitStack() as ctx:
            consts = ctx.enter_context(tc.tile_pool(name="consts", bufs=1))
            px = ctx.enter_context(tc.tile_pool(name="px", bufs=2))
            py = ctx.enter_context(tc.tile_pool(name="py", bufs=1))
            pgn = ctx.enter_context(tc.tile_pool(name="pgn", bufs=2))
            pk = ctx.enter_context(tc.tile_pool(name="pk", bufs=4))
            pvt = ctx.enter_context(tc.tile_pool(name="pvt", bufs=int(os.environ.get("KPVT", 2 + NP))))
            pq = ctx.enter_context(tc.tile_pool(name="pq", bufs=4))
            pwts = ctx.enter_context(tc.tile_pool(name="pwts", bufs=int(os.environ.get("KPWTS", 8))))
            pa = ctx.enter_context(tc.tile_pool(name="pa", bufs=2))
            pat = ctx.enter_context(tc.tile_pool(name="pat", bufs=2))
            pdn = ctx.enter_context(tc.tile_pool(name="pdn", bufs=1))
            pdn3 = ctx.enter_context(tc.tile_pool(name="pdn3", bufs=2))
            pst = ctx.enter_context(tc.tile_pool(name="pst", bufs=4))
            ps_mm = ctx.enter_context(tc.tile_pool(name="ps_mm", bufs=2, space="PSUM"))
            ps_s = ctx.enter_context(tc.tile_pool(name="ps_s", bufs=4, space="PSUM"))
            ps_a0 = ctx.enter_context(tc.tile_pool(name="ps_a0", bufs=1, space="PSUM"))
            ps_a1 = ctx.enter_context(tc.tile_pool(name="ps_a1", bufs=1, space="PSUM"))

            # --- constants (weights fp8; DMAs ordered for startup overlap) ---
            wq_sb = consts.tile([128, CB, C], FP8, tag="wq")
            wk_sb = consts.tile([128, CB, C], FP8, tag="wk")
            wv_sb = consts.tile([128, CB, C], FP8, tag="wv")
            wp_sb = consts.tile([128, CB, C], FP8, tag="wp")

            def emit_vp_weight_loads():
                nc.sync.dma_start(out=wv_sb, in_=wvt.rearrange("(kb p) o -> p kb o", p=128))
                nc.sync.dma_start(out=wp_sb, in_=wpt.rearrange("(kb p) o -> p kb o", p=128))

            m1_sb = consts.tile([128, 128], F32, tag="m1")
            bq_sb = consts.tile([128, CB], F32, tag="bq")
            bk_sb = consts.tile([128, CB], F32, tag="bk")
            bp_sb = consts.tile([128, CB], F32, tag="bp")
            bv_sb = consts.tile([128, NH, CH], F32, tag="bv")
            gnw_sb = consts.tile([128, CB], F32, tag="gnw")
            gnb_sb = consts.tile([128, CB], F32, tag="gnb")
            id_sb = consts.tile([128, 128], BF16, tag="id")
            magic_sb = consts.tile([128, CB], I32, tag="magic")
            nc.vector.memset(magic_sb, 0x5f3759df)
            ones8_sb = consts.tile([128, NH], FP8, tag="ones8")
            nc.vector.memset(ones8_sb, 1.0)
            ones16_sb = consts.tile([128, NH], BF16, tag="ones16")
            nc.vector.memset(ones16_sb, 1.0)
            nbias_sb = consts.tile([128, 1], F32, tag="nbias")
            nc.vector.memset(nbias_sb, SHIFT)
            # persistent per-head q tiles, fp8, zero-padded outside the head's
            # 64 channels; zeros are memset once and persist across batches
            q8_sb = [consts.tile([128, 2, T], FP8, tag=f"q8_{h}", name=f"q8_{h}")
                     for h in range(NH)]
            for h in range(NH):
                nc.gpsimd.memset(q8_sb[h], 0.0)
            warm = consts.tile([1, 1], F32, tag="warm")
            nc.vector.memset(warm, 0.0)
            nc.scalar.activation(out=warm, in_=warm, func=AF.Exp)

            def emit_gn_consts():
                nc.sync.dma_start(out=m1_sb, in_=m1[:, :])
                nc.sync.dma_start(out=gnw_sb, in_=gnw_l[:, :])
                nc.sync.dma_start(out=gnb_sb, in_=gnb_l[:, :])

            def emit_bias_consts():
                nc.sync.dma_start(out=bk_sb, in_=bk_l[:, :])
                nc.sync.dma_start(out=bq_sb, in_=bq_l[:, :])
                nc.sync.dma_start(out=bv_sb, in_=bv_bc[:, :, :])
                nc.sync.dma_start(out=bp_sb, in_=bp_l[:, :])
                nc.sync.dma_start(out=id_sb, in_=id128[:, :])

            sched_state = {"sch_i": 0}

            def groupnorm(src_sb, dst_fn, fast_apply=False, act_stats=False):
                """src_sb: [128, CB, T] f32. dst_fn(cb, th)->AP (fp8 out)."""
                mv = pst.tile([128, CB, 2], F32, tag="mv")
                if act_stats:
                    # split stats: ACT (Copy/Square accum) covers cb0-1 while
                    # DVE bn_stats covers cb2-3 — halves the serial latency on
                    # the startup-critical tensor.
                    part = pst.tile([128, 2, 2, 2], F32, tag="part")
                    for si, (func, scale) in enumerate(
                            ((AF.Copy, 1.0 / T), (AF.Square, 1.0 / np.sqrt(T)))):
                        for cb in range(2):
                            for c2 in range(2):
                                trash = pwts.tile([128, 512], BF16, tag="trash", name="trash")
                                nc.scalar.activation(
                                    out=trash, in_=src_sb[:, cb, c2 * 512:(c2 + 1) * 512],
                                    func=func, scale=scale,
                                    accum_out=part[:, cb, si, c2:c2 + 1])
                    stats6b = pst.tile([128, 2, 6], F32, tag="stats6b")
                    for cb in (2, 3):
                        for c2 in range(2):
                            nc.vector.bn_stats(
                                out=stats6b[:, c2, :],
                                in_=src_sb[:, cb, c2 * 512:(c2 + 1) * 512])
                        nc.vector.bn_aggr(out=mv[:, cb, :], in_=stats6b)
                    # cb0-1: mv = (mean, E[x^2]) from the two half-col accums
                    nc.vector.tensor_tensor(
                        out=mv[:, 0:2, :].rearrange("p a b -> p (a b)"),
                        in0=part[:, :, :, 0].rearrange("p a b -> p (a b)"),
                        in1=part[:, :, :, 1].rearrange("p a b -> p (a b)"), op=OP.add)
                    # cb2-3: convert var -> E[x^2] in place
                    musqb = pst.tile([128, 2], F32, tag="musqb")
                    nc.vector.tensor_tensor(out=musqb, in0=mv[:, 2:4, 0], in1=mv[:, 2:4, 0], op=OP.mult)
                    nc.vector.tensor_tensor(out=mv[:, 2:4, 1], in0=musqb, in1=mv[:, 2:4, 1], op=OP.add)
                else:
                    stats6 = pst.tile([128, 2, 6], F32, tag="stats6")
                    for cb in range(CB):
                        for c2 in range(2):
                            nc.vector.bn_stats(
                                out=stats6[:, c2, :],
                                in_=src_sb[:, cb, c2 * 512:(c2 + 1) * 512])
                        nc.vector.bn_aggr(out=mv[:, cb, :], in_=stats6)
                    # m2 slot in-place: mv[:,:,1] = var + mean^2
                    musq = pst.tile([128, 4], F32, tag="musq")
                    nc.vector.tensor_tensor(out=musq, in0=mv[:, :, 0], in1=mv[:, :, 0], op=OP.mult)
                    nc.vector.tensor_tensor(out=mv[:, :, 1], in0=musq, in1=mv[:, :, 1], op=OP.add)
                psg = ps_s.tile([128, 8], F32, tag="sc", name="psg")
                nc.tensor.matmul(psg, m1_sb, mv.rearrange("p a b -> p (a b)"), start=True, stop=True)
                gsb = pst.tile([128, 8], F32, tag="gsb")
                nc.vector.tensor_copy(gsb, psg)  # m1 carries 1/GSIZE; cols interleaved (mean, m2)
                # var + eps = (m2 + eps) - mean^2, fused
                tmp4 = pst.tile([128, 4], F32, tag="tmp4")
                nc.vector.tensor_tensor(out=tmp4, in0=gsb[:, 0::2], in1=gsb[:, 0::2], op=OP.mult)
                vv = pst.tile([128, 4], F32, tag="vv")
                nc.vector.scalar_tensor_tensor(
                    out=vv, in0=gsb[:, 1::2], scalar=EPS, in1=tmp4,
                    op0=OP.add, op1=OP.subtract)
                # rstd = rsqrt(vv): quake seed + 1 Newton step (3 fused ops)
                bsh = pst.tile([128, 4], I32, tag="bsh")
                nc.vector.tensor_scalar(
                    out=bsh, in0=vv.bitcast(I32), scalar1=1, scalar2=None,
                    op0=OP.logical_shift_right)
                nc.vector.tensor_tensor(out=tmp4.bitcast(I32), in0=magic_sb, in1=bsh, op=OP.subtract)
                nrt = pst.tile([128, 4], F32, tag="nrt")
                for _ in range(1):
                    nc.vector.tensor_tensor(out=nrt, in0=tmp4, in1=tmp4, op=OP.mult)
                    nc.vector.scalar_tensor_tensor(
                        out=nrt, in0=nrt, scalar=-0.5, in1=vv, op0=OP.mult, op1=OP.mult)
                    nc.vector.scalar_tensor_tensor(
                        out=tmp4, in0=nrt, scalar=1.5, in1=tmp4, op0=OP.add, op1=OP.mult)
                ab = pst.tile([128, 8], F32, tag="ab")
                nc.vector.tensor_tensor(out=ab[:, 0:4], in0=tmp4, in1=gnw_sb, op=OP.mult)
                tmp4b = pst.tile([128, 4], F32, tag="tmp4b")
                nc.vector.tensor_tensor(out=tmp4b, in0=gsb[:, 0::2], in1=ab[:, 0:4], op=OP.mult)
                nc.vector.tensor_tensor(out=ab[:, 4:8], in0=gnb_sb, in1=tmp4b, op=OP.subtract)
                for th in range(NT):
                    for cb in range(CB):
                        if os.environ.get("KERNEL_BASE_GN") == "1":
                            eng = nc.gpsimd if (cb % 2 == 1 and not fast_apply) else nc.vector
                            eng.tensor_scalar(
                                out=dst_fn(cb, th), in0=src_sb[:, cb, th * 512:(th + 1) * 512],
                                scalar1=ab[:, cb:cb + 1], scalar2=ab[:, 4 + cb:5 + cb],
                                op0=OP.mult, op1=OP.add)
                        elif fast_apply:
                            # startup-critical: ACT is idle here and faster
                            nc.scalar.activation(
                                out=dst_fn(cb, th),
                                in_=src_sb[:, cb, th * 512:(th + 1) * 512],
                                func=AF.Identity, bias=ab[:, 4 + cb:5 + cb],
                                scale=ab[:, cb:cb + 1])
                        else:
                            nc.gpsimd.tensor_scalar(
                                out=dst_fn(cb, th), in0=src_sb[:, cb, th * 512:(th + 1) * 512],
                                scalar1=ab[:, cb:cb + 1], scalar2=ab[:, 4 + cb:5 + cb],
                                op0=OP.mult, op1=OP.add)

            def emit_input_loads(b, mid_fn=None):
                y_sb = py.tile([128, CB, T], F32, tag="y")
                for cb in range(CB):
                    nc.sync.dma_start(
                        out=y_sb[:, cb, :],
                        in_=y2[b].rearrange("(cb p) t -> p cb t", p=128)[:, cb, :])
                if mid_fn is not None:
                    mid_fn()  # gn consts + k/q weights jump the queue ahead of x
                x_sb = px.tile([128, CB, T], F32, tag="x")
                for cb in range(CB):
                    nc.sync.dma_start(
                        out=x_sb[:, cb, :],
                        in_=x2[b].rearrange("(cb p) t -> p cb t", p=128)[:, cb, :])
                return x_sb, y_sb

            def emit_gn_compute(x_sb, y_sb, fast_apply=False, act_stats=False):
                gny = pgn.tile([128, CB, T], FP8, tag="gn")
                groupnorm(y_sb, lambda cb, th: gny[:, cb, th * 512:(th + 1) * 512], fast_apply)
                gnx = pgn.tile([128, CB, T], FP8, tag="gn")
                groupnorm(x_sb, lambda cb, th: gnx[:, cb, th * 512:(th + 1) * 512], fast_apply,
                          act_stats=act_stats)
                return gnx, gny

            def q_unit(bctx, ob, pool=None):
                """Closure list: 4 DR matmuls computing q for one ob (2 heads)."""
                st8 = {}
                def mk(th, j):
                    def f():
                        gnx = bctx["gn"][0]
                        if j == 0:
                            st8[th] = (ps_mm.tile([128, 512], F32, tag="mm", name="psu")
                                       if pool is None else
                                       pool.tile([128, 512], F32, tag="sc", name="psu"))
                        psq = st8[th]
                        nc.tensor.matmul(
                            psq,
                            wq_sb[:, 2 * j:2 * j + 2, ob * 128:(ob + 1) * 128],
                            gnx[:, 2 * j:2 * j + 2, th * 512:(th + 1) * 512],
                            start=(j == 0), stop=(j == 1), perf_mode=PM.DoubleRow)
                        if j == 1:
                            jj = ob % 2
                            nc.scalar.activation(
                                out=q8_sb[2 * ob][0:64, jj, th * 512:(th + 1) * 512],
                                in_=psq[0:64, :], func=AF.Identity,
                                bias=bq_sb[0:64, ob:ob + 1], scale=1.0 / WSCALE)
                            nc.scalar.activation(
                                out=q8_sb[2 * ob + 1][64:128, jj, th * 512:(th + 1) * 512],
                                in_=psq[64:128, :], func=AF.Identity,
                                bias=bq_sb[64:128, ob:ob + 1], scale=1.0 / WSCALE)
                    return f
                return [mk(th, j) for th in range(NT) for j in range(2)]

            def k_unit(bctx, ob, pool=None):
                st8 = {}
                def mk(th, j):
                    def f():
                        gny = bctx["gn"][1]
                        obp = ob // 2
                        if bctx["k8"][obp] is None:
                            bctx["k8"][obp] = pk.tile([128, 2, T], FP8, tag="k", name="k_obp")
                        k8p = bctx["k8"][obp]
                        if j == 0:
                            st8[th] = (ps_mm.tile([128, 512], F32, tag="mm", name="psu")
                                       if pool is None else
                                       pool.tile([128, 512], F32, tag="sc", name="psu"))
                        psk = st8[th]
                        nc.tensor.matmul(
                            psk,
                            wk_sb[:, 2 * j:2 * j + 2, ob * 128:(ob + 1) * 128],
                            gny[:, 2 * j:2 * j + 2, th * 512:(th + 1) * 512],
                            start=(j == 0), stop=(j == 1), perf_mode=PM.DoubleRow)
                        if j == 1:
                            nc.scalar.activation(
                                out=k8p[:, ob % 2, th * 512:(th + 1) * 512],
                                in_=psk, func=AF.Identity,
                                bias=bk_sb[:, ob:ob + 1], scale=1.0 / WSCALE)
                    return f
                return [mk(th, j) for th in range(NT) for j in range(2)]

            def vt_unit(bctx, tt):
                """Closure list: 2 DR matmuls + bias for one vT seq tile.

                vT tiles hold an st-PAIR: [128, 2, NH, CH+1] bf16."""
                st8 = {}
                pair, sub = tt // 2, tt % 2
                def mk(j):
                    def f():
                        gny = bctx["gn"][1]
                        if j == 0:
                            st8["ps"] = ps_mm.tile([128, 512], F32, tag="mm", name="psu")
                        psv = st8["ps"]
                        nc.tensor.matmul(
                            psv,
                            gny[:, 2 * j:2 * j + 2, tt * 128:(tt + 1) * 128],
                            wv_sb[:, 2 * j:2 * j + 2, :],
                            start=(j == 0), stop=(j == 1), perf_mode=PM.DoubleRow)
                        if j == 1:
                            if bctx["vtp"][pair] is None:
                                bctx["vtp"][pair] = pvt.tile(
                                    [128, 2, NH, CH + 1], BF16, tag="vt", name="vt")
                            vt = bctx["vtp"][pair]
                            nc.vector.tensor_tensor(
                                out=vt[:, sub, :, 0:CH],
                                in0=psv.rearrange("p (h c) -> p h c", h=NH),
                                in1=bv_sb, op=OP.add)
                            nc.vector.tensor_copy(
                                vt[:, sub, :, CH:CH + 1],
                                ones16_sb.rearrange("p (h o) -> p h o", o=1))
                    return f
                return [mk(j) for j in range(2)]

            def pproj_unit(bctx, b, ob, pool=None, split_tail=False, store_q=None,
                           pool_tag="sc", act_resid=False):
                """Closure list: 4 DR matmuls + bias/residual/store for one out block.

                split_tail: j0 group (kb0-1, residual applied early), then a
                separate j1 group + final add + store — shortens the critical
                chain behind the last attention block.
                """
                st8 = {}
                def mk(th, j):
                    def f():
                        xr = bctx["x"]
                        a_sb = bctx["a"]
                        sl = slice(th * 512, (th + 1) * 512)
                        if j == 0 or split_tail:
                            st8[th] = (ps_mm.tile([128, 512], F32, tag="mm", name="psu")
                                       if pool is None else
                                       pool.tile([128, 512], F32, tag=pool_tag, name="psu"))
                        psh = st8[th]
                        nc.tensor.matmul(
                            psh,
                            wp_sb[:, 2 * j:2 * j + 2, ob * 128:(ob + 1) * 128],
                            a_sb[:, 2 * j:2 * j + 2, sl],
                            start=(j == 0 or split_tail),
                            stop=(j == 1 or split_tail),
                            perf_mode=PM.DoubleRow)
                        grp_end = (j == 0) if split_tail else (j == 1)
                        if grp_end:
                            nc.vector.scalar_tensor_tensor(
                                out=xr[:, ob, sl],
                                in0=psh, scalar=bp_sb[:, ob:ob + 1],
                                in1=xr[:, ob, sl], op0=OP.add, op1=OP.add)
                        if j == 1:
                            if split_tail:
                                nc.vector.tensor_tensor(
                                    out=xr[:, ob, sl], in0=psh, in1=xr[:, ob, sl],
                                    op=OP.add)
                            (store_q or nc.sync).dma_start(
                                out=out_d[b].rearrange("(cb p) t -> p cb t", p=128)[:, ob, sl],
                                in_=xr[:, ob, sl])
                    return f
                return [mk(th, j) for th in range(NT) for j in range(2)]

            def attention_head(bctx, ob, hh, qp, a_sb, lazy_vt=False, filler=None):
                """Emit one head's QK/exp slots; AV matmuls trail by one
                st-pair ACROSS head boundaries (bctx["pend"]), so the PE never
                waits on a just-issued exp and the ACT/DVE exp chains continue
                seamlessly from head to head."""
                h = 2 * ob + hh
                k8p = bctx["k8"][ob // 2]
                vtp = bctx["vtp"]
                psa_t = [ps_a0.tile([128, 512], F32, tag="av0", name="psa0"),
                         ps_a1.tile([128, 512], F32, tag="av1", name="psa1")]
                psa = [t[:, 0:4 * (CH + 1)].rearrange("p (a b) -> p a b", b=CH + 1)
                       for t in psa_t]
                bctx.setdefault("pend", [])

                def emit_avs(ctx):
                    hp, pair, wts, psa_p, a_sb_p = ctx
                    first = pair == 0
                    last = pair == NP - 1
                    for th in range(NT):
                        for sub in range(2):
                            for tc4 in range(4):
                                nc.tensor.matmul(
                                    psa_p[th][:, tc4, :],
                                    wts[:, sub, th, tc4 * 128:(tc4 + 1) * 128],
                                    vtp[pair][:, sub, hp, :],
                                    start=(first and sub == 0 and tc4 == 0),
                                    stop=(last and sub == 1 and tc4 == 3),
                                    skip_group_check=True)
                        if filler is not None:
                            filler()
                            filler()
                    if last:
                        finish_head(bctx, (hp // 2), hp % 2, psa_p, a_sb_p)

                hg = bctx["b"] * NH + h
                for pair in range(NP):
                    if lazy_vt and vtp[pair] is None:
                        for tt in (2 * pair, 2 * pair + 1):
                            for f in vt_unit(bctx, tt):
                                f()
                    wts = pwts.tile([128, 2, NT, 512], BF16, tag="wts")
                    for sub in range(2):
                        st = 2 * pair + sub
                        # half-width score tiles, 4 psum chains: each engine
                        # owns two (st-parity x th), so one chain's exp can
                        # run while the other waits its QK round-trip
                        for th in range(NT):
                            pss = ps_s.tile([128, 512], F32, tag="sc")
                            nc.tensor.matmul(
                                pss,
                                k8p[:, :, st * 128:(st + 1) * 128],
                                q8_sb[h][:, :, th * 512:(th + 1) * 512],
                                start=True, stop=True, perf_mode=PM.DoubleRow)
                            if filler is not None:
                                filler()
                            if sub == 1 and hg not in ACT_ODD_HEADS:
                                nc.vector.tensor_scalar(
                                    out=wts.bitcast(I16)[:, sub, th, :],
                                    in0=pss,
                                    scalar1=SCH_A, scalar2=SCH_C,
                                    op0=OP.mult, op1=OP.add)
                            else:
                                nc.scalar.activation(
                                    out=wts[:, sub, th, :],
                                    in_=pss,
                                    func=AF.Exp, bias=nbias_sb, scale=1.0)
                    bctx["pend"].append((h, pair, wts, psa, a_sb))
                    if len(bctx["pend"]) > 1:
                        emit_avs(bctx["pend"].pop(0))

            def attention_flush(bctx, filler=None):
                """Drain the trailing AV slot at batch end."""
                for ctx in bctx["pend"]:
                    hp, pair, wts, psa_p, a_sb_p = ctx
                    for th in range(NT):
                        for sub in range(2):
                            for tc4 in range(4):
                                nc.tensor.matmul(
                                    psa_p[th][:, tc4, :],
                                    wts[:, sub, th, tc4 * 128:(tc4 + 1) * 128],
                                    bctx["vtp"][pair][:, sub, hp, :],
                                    start=(pair == 0 and sub == 0 and tc4 == 0),
                                    stop=(pair == NP - 1 and sub == 1 and tc4 == 3),
                                    skip_group_check=True)
                    if pair == NP - 1:
                        finish_head(bctx, hp // 2, hp % 2, psa_p, a_sb_p)
                bctx["pend"] = []

            def finish_head(bctx, ob, hh, psa, a_sb):
                # denominators are per-partition columns now: copy+recip+scale.
                # high priority: these free the single-buffered AV psum banks,
                # so they must win DVE scheduling ties against filler ops.
                with tc.high_priority():
                    _finish_head(bctx, ob, hh, psa, a_sb)

            def _finish_head(bctx, ob, hh, psa, a_sb):
                h = 2 * ob + hh
                aT = bctx["aT"]
                for th in range(NT):
                    rr = pdn3.tile([128, 4], F32, tag="dn3")
                    if os.environ.get("KERNEL_DN_COPY") == "1":
                        dn = pdn.tile([128, 4], F32, tag="r0")
                        nc.vector.tensor_copy(dn, psa[th][:, :, CH])
                        nc.vector.reciprocal_approx_fast(out=rr, in_=dn)
                    else:
                        nc.vector.reciprocal_approx_fast(out=rr, in_=psa[th][:, :, CH])
                    nc.vector.tensor_tensor(
                        out=aT[:, th, :, h, :],
                        in0=psa[th][:, :, 0:CH],
                        in1=rr[:, :, None].broadcast_to([128, 4, CH]),
                        op=OP.mult)

            def finish_ob(bctx, ob, a_sb):
                """Transpose aT[t, c] blocks of channel-block ob back to a[c, t]."""
                aT = bctx["aT"]
                trp = ps_mm.tile([128, ST, 128], BF16, tag="mm", name="trp")
                for tb in range(ST):
                    th, tc4 = tb // 4, tb % 4
                    nc.tensor.matmul(
                        trp[:, tb, :],
                        aT[:, th, tc4, 2 * ob:2 * ob + 2, :].rearrange("p a b -> p (a b)"),
                        id_sb,
                        is_transpose=True)
                nc.scalar.activation(out=a_sb[:, ob, :],
                                     in_=trp.rearrange("p a b -> p (a b)"),
                                     func=AF.Copy)

            # ---------------- batch pipeline ----------------
            from collections import deque
            fillers = deque()

            def filler_pop():
                if fillers:
                    fillers.popleft()()

            def filler_flush():
                while fillers:
                    fillers.popleft()()

            bctxs = [dict() for _ in range(BPC)]
            _xy0 = emit_input_loads(0, mid_fn=emit_gn_consts)
            for wi in range(20):
                trash = ps_mm.tile([128, 64], F32, tag="mm", name="wtr0")
                nc.tensor.matmul(trash, _xy0[0][:, 0, 0:128], _xy0[0][:, 0, 0:64],
                                 start=True, stop=True)
            emit_bias_consts()
            nc.sync.dma_start(out=wq_sb, in_=wqt.rearrange("(kb p) o -> p kb o", p=128))
            nc.sync.dma_start(out=wk_sb, in_=wkt.rearrange("(kb p) o -> p kb o", p=128))
            emit_vp_weight_loads()
            bctxs[0]["x"] = _xy0[0]
            bctxs[0]["gn"] = emit_gn_compute(*_xy0, fast_apply=True, act_stats=True)
            bctxs[0]["k8"] = [None] * (CB // 2)
            bctxs[0]["vtp"] = [None] * NP

            # batch 0 ob0 prep emitted directly; rest queued as fillers that
            # drip into the attention pair slots (1 matmul per QK/AV pair)
            for f in k_unit(bctxs[0], 0, pool=ps_s):
                f()
            for f in q_unit(bctxs[0], 0, pool=ps_s):
                f()
            for ob2 in (1, 2, 3):
                fillers.extend(k_unit(bctxs[0], ob2))
                fillers.extend(q_unit(bctxs[0], ob2))

            for b in range(BPC):
                bctx = bctxs[b]
                bctx["b"] = b
                a_sb = pa.tile([128, CB, T], FP8, tag="a")
                bctx["a"] = a_sb
                bctx["aT"] = pat.tile([128, NT, 4, NH, CH], BF16, tag="aT", name="aT")
                for ob in range(CB):
                    if b > 0:
                        if ob == 0:
                            for ob2 in (1, 2):
                                fillers.extend(k_unit(bctx, ob2))
                                fillers.extend(q_unit(bctx, ob2))
                        if ob == 1:
                            fillers.extend(k_unit(bctx, 3))
                            fillers.extend(q_unit(bctx, 3))
                            for ob2 in range(CB):
                                fillers.extend(pproj_unit(bctxs[b - 1], b - 1, ob2, pool=ps_s))
                    if b + 1 < BPC:
                        if ob == 0:
                            nb = bctxs[b + 1]
                            nb["xy"] = emit_input_loads(b + 1)
                            nb["x"] = nb["xy"][0]
                        if ob == 1:
                            nb = bctxs[b + 1]
                            nb["gn"] = emit_gn_compute(*nb.pop("xy"))
                            nb["k8"] = [None] * (CB // 2)
                            nb["vtp"] = [None] * NP

                    for hh in (0, 1):
                        attention_head(bctx, ob, hh, None, a_sb,
                                       lazy_vt=(b == 0 and ob == 0),
                                       filler=filler_pop)
                        if hh == 0 and b + 1 < BPC:
                            nb = bctxs[b + 1]
                            if ob == 2:
                                fillers.extend(k_unit(nb, 0))
                                for tt in range(ST):
                                    fillers.extend(vt_unit(nb, tt))
                            if ob == 3:
                                fillers.extend(q_unit(nb, 0))
                    if ob > 0:
                        finish_ob(bctx, ob - 1, a_sb)
                        if b == BPC - 1:
                            fillers.extend(pproj_unit(bctx, b, ob - 1, pool=ps_s))
                    if ob == CB - 1:
                        attention_flush(bctx)
                        finish_ob(bctx, ob, a_sb)

                if b == BPC - 1:
                    # tail: drain queue, then the last output projection (ob3
                    # only; obs 0-2 were emitted as fillers after finish_ob).
                    filler_flush()
                    for wi in range(4):
                        trash = ps_mm.tile([128, 512], F32, tag="mm", name="wtr")
                        nc.tensor.matmul(trash, wp_sb[:, 0, 0:128],
                                         a_sb[:, 0, 0:512],
                                         start=True, stop=True)
                    for f in pproj_unit(bctx, b, CB - 1, pool=ps_a0, pool_tag="av0",
                                        split_tail=True, store_q=nc.scalar):
                        f()

    nc.finalize()
    return nc


_NC = None


def _get_nc():
    global _NC
    if _NC is None:
        _NC = _build()
    return _NC


def _prep_inputs(x, y, gn_w, gn_b, Wq, bq, Wkv, bkv, Wp, bp):
    scale = CH ** -0.25
    # reference splits k/v per head: kvh[:, h, :ch] / kvh[:, h, ch:] after
    # reshape to [b, NH, 2*ch, T] -> k_h = Wkv rows [h*128, h*128+64)
    import ml_dtypes
    FP8NP = ml_dtypes.float8_e4m3
    idx_k = np.concatenate([np.arange(h * 2 * CH, h * 2 * CH + CH) for h in range(NH)])
    idx_v = np.concatenate([np.arange(h * 2 * CH + CH, (h + 1) * 2 * CH) for h in range(NH)])
    # Wq/Wk prescaled by WSCALE to keep fp8 values out of denormal range;
    # compensated by 1/WSCALE in the psum->sbuf bias add.
    wqt = np.ascontiguousarray((Wq * (scale * WSCALE)).T).astype(FP8NP)
    wkt = np.ascontiguousarray((Wkv[idx_k] * (scale * WSCALE)).T).astype(FP8NP)
    wvt = np.ascontiguousarray(Wkv[idx_v].T).astype(FP8NP)
    wpt = np.ascontiguousarray(Wp.T).astype(FP8NP)
    bq_s = bq * scale
    bk_s = bkv[idx_k] * scale
    bv = bkv[idx_v]

    def part_layout(v):  # [C] -> [128, CB]: v[cb*128+p]
        return np.ascontiguousarray(v.reshape(CB, 128).T)

    bq_l = part_layout(bq_s)
    bk_l = part_layout(bk_s)
    bp_l = part_layout(bp)
    gnw_l = part_layout(gn_w)
    gnb_l = part_layout(gn_b)
    bv_bc = np.broadcast_to(bv.reshape(1, NH, CH), (128, NH, CH)).copy()
    m1 = np.zeros((128, 128), np.float32)
    for g in range(128 // GSIZE):
        m1[g * GSIZE:(g + 1) * GSIZE, g * GSIZE:(g + 1) * GSIZE] = 1.0 / GSIZE
    id128_h = np.eye(128, dtype=ml_dtypes.bfloat16)

    xf = x.reshape(B, C, T)
    yf = y.reshape(B, C, T)

    shared = {
        "wqt": wqt, "wkt": wkt, "wvt": wvt, "wpt": wpt,
        "bq_l": bq_l, "bk_l": bk_l, "bp_l": bp_l, "bv_bc": bv_bc,
        "gnw_l": gnw_l, "gnb_l": gnb_l, "m1": m1, "id128": id128_h,
    }
    in_maps = []
    for i in range(N_CORES):
        m = dict(shared)
        m["x2"] = np.ascontiguousarray(xf[i * BPC:(i + 1) * BPC])
        m["y2"] = np.ascontiguousarray(yf[i * BPC:(i + 1) * BPC])
        in_maps.append(m)
    return in_maps


def kernel(x, y, gn_w, gn_b, Wq, bq, Wkv, bkv, Wp, bp):
    args = [np.asarray(a, dtype=np.float32) for a in
            (x, y, gn_w, gn_b, Wq, bq, Wkv, bkv, Wp, bp)]
    in_maps = _prep_inputs(*args)
    nc = _get_nc()
    res = run_bass_kernel_spmd(nc, in_maps, core_ids=list(range(N_CORES)))
    out = np.empty((B, C, T), np.float32)
    for i in range(N_CORES):
        out[i * BPC:(i + 1) * BPC] = res.results[i]["out"]
    return out.reshape(B, C, H, W)


# revision 37
# speedup vs baseline: 1.0871x; 1.0134x over previous
"""AttentionBlock Trainium2 Bass kernel.

Data-parallel over batch: 16 batches / 8 cores = 2 per core. Each core runs
the full block (groupnorm x2, q/kv projections, 8-head attention, output
projection, residual) on its 2 batch elements.

Key design points (v2):
- fp8e4m3 DoubleRow matmuls for all four projections (Wq/Wk prescaled x32 to
  escape fp8 denormals, compensated in the psum->sbuf bias add) and for the
  attention*V of st-pairs 1-3: 256-deep contraction at 0.5 cyc/row quarters
  projection PE time and AV PE time vs bf16.
- scores layout [s, t]; exp without max-subtraction but with a constant -4
  logit shift so exp output fits fp8 range (shift cancels in softmax).
- exp engine split: st-pairs 1-3 go to ACT (exp -> fp8 wts), st-pair 0 goes
  to DVE as a Schraudolph fast-exp (one tensor_scalar: i16(round(l*184.66 +
  15511.5)) bitcast bf16, ~3% multiplicative err, cancels mostly in softmax).
  Pair-0 wts/v tiles are bf16; AV for pair 0 runs as plain bf16 matmuls.
- wts/vT tiles hold an st-PAIR each ([128, 2, ...]) so DR matmuls can pair
  the contraction; softmax denominator rides along as a ones column of vT.
- normalize: one broadcast tensor_tensor per (head, th) scales psum by the
  per-partition reciprocal denominators -> aT bf16; PE transpose per channel
  block; psum->sbuf copy converts a to fp8 for the DR output projection.
- groupnorm applies run on GPSIMD (Pool) except the startup-critical batch-0
  pair; stats stay on DVE bn_stats (batch-0 x split ACT/DVE).
- software pipelining: AV matmuls trail their QK pair by one st-pair;
  projection/output matmuls are emitted as 1-matmul closures popped between
  attention slots; next-batch prep is pushed mid-ob.
- startup: y loads -> gn consts -> x loads -> biases -> fp8 weights; PE clock
  pre-warmed with dummy matmuls.
"""
import os
import sys

sys.path.insert(0, "/opt/trn_rl_repo")

import numpy as np

import concourse.bacc as bacc
import concourse.bass as bass
import concourse.tile as tile
from concourse import mybir
from concourse.bass_utils import run_bass_kernel_spmd

F32 = mybir.dt.float32
F32R = mybir.dt.float32r
BF16 = mybir.dt.bfloat16
FP8 = mybir.dt.float8e4
I16 = mybir.dt.int16
I32 = mybir.dt.int32
AF = mybir.ActivationFunctionType
OP = mybir.AluOpType
PM = mybir.MatmulPerfMode

B, C, H, W = 16, 512, 32, 32
T = H * W              # 1024
NH = 8                 # heads
CH = C // NH           # 64
GROUPS = 32
GSIZE = C // GROUPS    # 16 channels per group
EPS = 1e-5
N_CORES = 8
BPC = B // N_CORES     # batches per core
CB = C // 128          # 4 channel blocks
NT = T // 512          # 2 column halves of 512
ST = T // 128          # 8 seq tiles of 128
NP = ST // 2           # 4 st-pairs
WSCALE = 32.0          # Wq/Wk fp8 prescale (keeps weights out of denormals)
SHIFT = -4.0           # logit shift before exp (cancels in softmax)
SCH_A = 184.664375     # 2^7 / ln 2
SCH_C = 15511.5        # 16256 - 0.5 - 5.25 + SHIFT*SCH_A  (tuned Schraudolph)
BF16_PAIRS = (0, 1, 2, 3)   # all wts/v tiles bf16: even sts exp on ACT, odd
                            # sts Schraudolph on DVE — two independent psum
                            # chains so the engines never serialize on ps_s
ACT_ODD_HEADS = (11,)       # head-batches whose odd-st chain flips to ACT
                            # (fine engine-load balance)

DEBUG = bool(int(os.environ.get("KERNEL_DEBUG", "0")))


def _build():
    nc = bacc.Bacc(None, target_bir_lowering=False)

    x2 = nc.dram_tensor("x2", (BPC, C, T), F32, kind="ExternalInput")
    y2 = nc.dram_tensor("y2", (BPC, C, T), F32, kind="ExternalInput")
    wqt = nc.dram_tensor("wqt", (C, C), FP8, kind="ExternalInput")
    wkt = nc.dram_tensor("wkt", (C, C), FP8, kind="ExternalInput")
    wvt = nc.dram_tensor("wvt", (C, C), FP8, kind="ExternalInput")
    wpt = nc.dram_tensor("wpt", (C, C), FP8, kind="ExternalInput")
    bq_l = nc.dram_tensor("bq_l", (128, CB), F32, kind="ExternalInput")
    bk_l = nc.dram_tensor("bk_l", (128, CB), F32, kind="ExternalInput")
    bp_l = nc.dram_tensor("bp_l", (128, CB), F32, kind="ExternalInput")
    bv_bc = nc.dram_tensor("bv_bc", (128, NH, CH), F32, kind="ExternalInput")
    gnw_l = nc.dram_tensor("gnw_l", (128, CB), F32, kind="ExternalInput")
    gnb_l = nc.dram_tensor("gnb_l", (128, CB), F32, kind="ExternalInput")
    m1 = nc.dram_tensor("m1", (128, 128), F32, kind="ExternalInput")
    id128 = nc.dram_tensor("id128", (128, 128), BF16, kind="ExternalInput")
    out_d = nc.dram_tensor("out", (BPC, C, T), F32, kind="ExternalOutput")
    if DEBUG:
        dbg_a = nc.dram_tensor("dbg_a", (C, T), F32, kind="ExternalOutput")

    with tile.TileContext(nc) as tc:
        from contextlib import ExitStack
        with ExitStack() as ctx:
            consts = ctx.enter_context(tc.tile_pool(name="consts", bufs=1))
            px = ctx.enter_context(tc.tile_pool(name="px", bufs=2))
            py = ctx.enter_context(tc.tile_pool(name="py", bufs=1))
            pgn = ctx.enter_context(tc.tile_pool(name="pgn", bufs=2))
            pk = ctx.enter_context(tc.tile_pool(name="pk", bufs=4))
            pvt = ctx.enter_context(tc.tile_pool(name="pvt", bufs=int(os.environ.get("KPVT", 2 + NP))))
            pq = ctx.enter_context(tc.tile_pool(name="pq", bufs=4))
            pwts = ctx.enter_context(tc.tile_pool(name="pwts", bufs=int(os.environ.get("KPWTS", 8))))
            pa = ctx.enter_context(tc.tile_pool(name="pa", bufs=2))
            pat = ctx.enter_context(tc.tile_pool(name="pat", bufs=2))
            pdn = ctx.enter_context(tc.tile_pool(name="pdn", bufs=1))
            pdn3 = ctx.enter_context(tc.tile_pool(name="pdn3", bufs=2))
            pst = ctx.enter_context(tc.tile_pool(name="pst", bufs=4))
            ps_mm = ctx.enter_context(tc.tile_pool(name="ps_mm", bufs=int(os.environ.get("KPSMM", 2)), space="PSUM"))
            ps_s = ctx.enter_context(tc.tile_pool(name="ps_s", bufs=int(os.environ.get("KPSS", 4)), space="PSUM"))
            ps_a0 = ctx.enter_context(tc.tile_pool(name="ps_a0", bufs=1, space="PSUM"))
            ps_a1 = ctx.enter_context(tc.tile_pool(name="ps_a1", bufs=1, space="PSUM"))

            # --- constants (weights fp8; DMAs ordered for startup overlap) ---
            wq_sb = consts.tile([128, CB, C], FP8, tag="wq")
            wk_sb = consts.tile([128, CB, C], FP8, tag="wk")
            wv_sb = consts.tile([128, CB, C], FP8, tag="wv")
            wp_sb = consts.tile([128, CB, C], FP8, tag="wp")

            def emit_vp_weight_loads():
                nc.sync.dma_start(out=wv_sb, in_=wvt.rearrange("(kb p) o -> p kb o", p=128))
                nc.sync.dma_start(out=wp_sb, in_=wpt.rearrange("(kb p) o -> p kb o", p=128))

            m1_sb = consts.tile([128, 128], F32, tag="m1")
            bq_sb = consts.tile([128, CB], F32, tag="bq")
            bk_sb = consts.tile([128, CB], F32, tag="bk")
            bp_sb = consts.tile([128, CB], F32, tag="bp")
            bv_sb = consts.tile([128, NH, CH], F32, tag="bv")
            gnw_sb = consts.tile([128, CB], F32, tag="gnw")
            gnb_sb = consts.tile([128, CB], F32, tag="gnb")
            id_sb = consts.tile([128, 128], BF16, tag="id")
            magic_sb = consts.tile([128, CB], I32, tag="magic")
            nc.vector.memset(magic_sb, 0x5f3759df)
            ones8_sb = consts.tile([128, NH], FP8, tag="ones8")
            nc.vector.memset(ones8_sb, 1.0)
            ones16_sb = consts.tile([128, NH], BF16, tag="ones16")
            nc.vector.memset(ones16_sb, 1.0)
            nbias_sb = consts.tile([128, 1], F32, tag="nbias")
            nc.vector.memset(nbias_sb, SHIFT)
            # persistent per-head q tiles, fp8, zero-padded outside the head's
            # 64 channels; zeros are memset once and persist across batches
            q8_sb = [consts.tile([128, 2, T], FP8, tag=f"q8_{h}", name=f"q8_{h}")
                     for h in range(NH)]
            for h in range(NH):
                nc.gpsimd.memset(q8_sb[h], 0.0)
            warm = consts.tile([1, 1], F32, tag="warm")
            nc.vector.memset(warm, 0.0)
            nc.scalar.activation(out=warm, in_=warm, func=AF.Exp)

            def emit_gn_consts():
                nc.sync.dma_start(out=m1_sb, in_=m1[:, :])
                nc.sync.dma_start(out=gnw_sb, in_=gnw_l[:, :])
                nc.sync.dma_start(out=gnb_sb, in_=gnb_l[:, :])

            def emit_bias_consts():
                nc.sync.dma_start(out=bk_sb, in_=bk_l[:, :])
                nc.sync.dma_start(out=bq_sb, in_=bq_l[:, :])
                nc.sync.dma_start(out=bv_sb, in_=bv_bc[:, :, :])
                nc.sync.dma_start(out=bp_sb, in_=bp_l[:, :])
                nc.sync.dma_start(out=id_sb, in_=id128[:, :])

            sched_state = {"sch_i": 0}

            def groupnorm(src_sb, dst_fn, fast_apply=False, act_stats=False):
                """src_sb: [128, CB, T] f32. dst_fn(cb, th)->AP (fp8 out)."""
                mv = pst.tile([128, CB, 2], F32, tag="mv")
                if act_stats:
                    # split stats: ACT (Copy/Square accum) covers cb0-1 while
                    # DVE bn_stats covers cb2-3 — halves the serial latency on
                    # the startup-critical tensor.
                    part = pst.tile([128, 2, 2, 2], F32, tag="part")
                    for si, (func, scale) in enumerate(
                            ((AF.Copy, 1.0 / T), (AF.Square, 1.0 / np.sqrt(T)))):
                        for cb in range(2):
                            for c2 in range(2):
                                trash = pwts.tile([128, 512], BF16, tag="trash", name="trash")
                                nc.scalar.activation(
                                    out=trash, in_=src_sb[:, cb, c2 * 512:(c2 + 1) * 512],
                                    func=func, scale=scale,
                                    accum_out=part[:, cb, si, c2:c2 + 1])
                    stats6b = pst.tile([128, 2, 6], F32, tag="stats6b")
                    for cb in (2, 3):
                        for c2 in range(2):
                            nc.vector.bn_stats(
                                out=stats6b[:, c2, :],
                                in_=src_sb[:, cb, c2 * 512:(c2 + 1) * 512])
                        nc.vector.bn_aggr(out=mv[:, cb, :], in_=stats6b)
                    # cb0-1: mv = (mean, E[x^2]) from the two half-col accums
                    nc.vector.tensor_tensor(
                        out=mv[:, 0:2, :].rearrange("p a b -> p (a b)"),
                        in0=part[:, :, :, 0].rearrange("p a b -> p (a b)"),
                        in1=part[:, :, :, 1].rearrange("p a b -> p (a b)"), op=OP.add)
                    # cb2-3: convert var -> E[x^2] in place
                    musqb = pst.tile([128, 2], F32, tag="musqb")
                    nc.vector.tensor_tensor(out=musqb, in0=mv[:, 2:4, 0], in1=mv[:, 2:4, 0], op=OP.mult)
                    nc.vector.tensor_tensor(out=mv[:, 2:4, 1], in0=musqb, in1=mv[:, 2:4, 1], op=OP.add)
                else:
                    stats6 = pst.tile([128, 2, 6], F32, tag="stats6")
                    for cb in range(CB):
                        for c2 in range(2):
                            nc.vector.bn_stats(
                                out=stats6[:, c2, :],
                                in_=src_sb[:, cb, c2 * 512:(c2 + 1) * 512])
                        nc.vector.bn_aggr(out=mv[:, cb, :], in_=stats6)
                    # m2 slot in-place: mv[:,:,1] = var + mean^2
                    musq = pst.tile([128, 4], F32, tag="musq")
                    nc.vector.tensor_tensor(out=musq, in0=mv[:, :, 0], in1=mv[:, :, 0], op=OP.mult)
                    nc.vector.tensor_tensor(out=mv[:, :, 1], in0=musq, in1=mv[:, :, 1], op=OP.add)
                psg = ps_s.tile([128, 8], F32, tag="sc", name="psg")
                nc.tensor.matmul(psg, m1_sb, mv.rearrange("p a b -> p (a b)"), start=True, stop=True)
                gsb = pst.tile([128, 8], F32, tag="gsb")
                nc.vector.tensor_copy(gsb, psg)  # m1 carries 1/GSIZE; cols interleaved (mean, m2)
                # var + eps = (m2 + eps) - mean^2, fused
                tmp4 = pst.tile([128, 4], F32, tag="tmp4")
                nc.vector.tensor_tensor(out=tmp4, in0=gsb[:, 0::2], in1=gsb[:, 0::2], op=OP.mult)
                vv = pst.tile([128, 4], F32, tag="vv")
                nc.vector.scalar_tensor_tensor(
                    out=vv, in0=gsb[:, 1::2], scalar=EPS, in1=tmp4,
                    op0=OP.add, op1=OP.subtract)
                # rstd = rsqrt(vv): quake seed + 1 Newton step (3 fused ops)
                bsh = pst.tile([128, 4], I32, tag="bsh")
                nc.vector.tensor_scalar(
                    out=bsh, in0=vv.bitcast(I32), scalar1=1, scalar2=None,
                    op0=OP.logical_shift_right)
                nc.vector.tensor_tensor(out=tmp4.bitcast(I32), in0=magic_sb, in1=bsh, op=OP.subtract)
                nrt = pst.tile([128, 4], F32, tag="nrt")
                for _ in range(1):
                    nc.vector.tensor_tensor(out=nrt, in0=tmp4, in1=tmp4, op=OP.mult)
                    nc.vector.scalar_tensor_tensor(
                        out=nrt, in0=nrt, scalar=-0.5, in1=vv, op0=OP.mult, op1=OP.mult)
                    nc.vector.scalar_tensor_tensor(
                        out=tmp4, in0=nrt, scalar=1.5, in1=tmp4, op0=OP.add, op1=OP.mult)
                ab = pst.tile([128, 8], F32, tag="ab")
                nc.vector.tensor_tensor(out=ab[:, 0:4], in0=tmp4, in1=gnw_sb, op=OP.mult)
                tmp4b = pst.tile([128, 4], F32, tag="tmp4b")
                nc.vector.tensor_tensor(out=tmp4b, in0=gsb[:, 0::2], in1=ab[:, 0:4], op=OP.mult)
                nc.vector.tensor_tensor(out=ab[:, 4:8], in0=gnb_sb, in1=tmp4b, op=OP.subtract)
                for th in range(NT):
                    for cb in range(CB):
                        if os.environ.get("KERNEL_BASE_GN") == "1":
                            eng = nc.gpsimd if (cb % 2 == 1 and not fast_apply) else nc.vector
                            eng.tensor_scalar(
                                out=dst_fn(cb, th), in0=src_sb[:, cb, th * 512:(th + 1) * 512],
                                scalar1=ab[:, cb:cb + 1], scalar2=ab[:, 4 + cb:5 + cb],
                                op0=OP.mult, op1=OP.add)
                        elif fast_apply:
                            # startup-critical: ACT is idle here and faster
                            nc.scalar.activation(
                                out=dst_fn(cb, th),
                                in_=src_sb[:, cb, th * 512:(th + 1) * 512],
                                func=AF.Identity, bias=ab[:, 4 + cb:5 + cb],
                                scale=ab[:, cb:cb + 1])
                        else:
                            nc.gpsimd.tensor_scalar(
                                out=dst_fn(cb, th), in0=src_sb[:, cb, th * 512:(th + 1) * 512],
                                scalar1=ab[:, cb:cb + 1], scalar2=ab[:, 4 + cb:5 + cb],
                                op0=OP.mult, op1=OP.add)

            def emit_input_loads(b, mid_fn=None):
                y_sb = py.tile([128, CB, T], F32, tag="y")
                for cb in range(CB):
                    nc.sync.dma_start(
                        out=y_sb[:, cb, :],
                        in_=y2[b].rearrange("(cb p) t -> p cb t", p=128)[:, cb, :])
                if mid_fn is not None:
                    mid_fn()  # gn consts + k/q weights jump the queue ahead of x
                x_sb = px.tile([128, CB, T], F32, tag="x")
                for cb in range(CB):
                    nc.sync.dma_start(
                        out=x_sb[:, cb, :],
                        in_=x2[b].rearrange("(cb p) t -> p cb t", p=128)[:, cb, :])
                return x_sb, y_sb

            def emit_gn_compute(x_sb, y_sb, fast_apply=False, act_stats=False):
                gny = pgn.tile([128, CB, T], FP8, tag="gn")
                groupnorm(y_sb, lambda cb, th: gny[:, cb, th * 512:(th + 1) * 512], fast_apply)
                gnx = pgn.tile([128, CB, T], FP8, tag="gn")
                groupnorm(x_sb, lambda cb, th: gnx[:, cb, th * 512:(th + 1) * 512], fast_apply,
                          act_stats=act_stats)
                return gnx, gny

            def q_unit(bctx, ob, pool=None):
                """Closure list: 4 DR matmuls computing q for one ob (2 heads)."""
                st8 = {}
                def mk(th, j):
                    def f():
                        gnx = bctx["gn"][0]
                        if j == 0:
                            st8[th] = (ps_mm.tile([128, 512], F32, tag="mm", name="psu")
                                       if pool is None else
                                       pool.tile([128, 512], F32, tag="sc", name="psu"))
                        psq = st8[th]
                        nc.tensor.matmul(
                            psq,
                            wq_sb[:, 2 * j:2 * j + 2, ob * 128:(ob + 1) * 128],
                            gnx[:, 2 * j:2 * j + 2, th * 512:(th + 1) * 512],
                            start=(j == 0), stop=(j == 1), perf_mode=PM.DoubleRow)
                        if j == 1:
                            jj = ob % 2
                            nc.scalar.activation(
                                out=q8_sb[2 * ob][0:64, jj, th * 512:(th + 1) * 512],
                                in_=psq[0:64, :], func=AF.Identity,
                                bias=bq_sb[0:64, ob:ob + 1], scale=1.0 / WSCALE)
                            nc.scalar.activation(
                                out=q8_sb[2 * ob + 1][64:128, jj, th * 512:(th + 1) * 512],
                                in_=psq[64:128, :], func=AF.Identity,
                                bias=bq_sb[64:128, ob:ob + 1], scale=1.0 / WSCALE)
                    return f
                return [mk(th, j) for th in range(NT) for j in range(2)]

            def k_unit(bctx, ob, pool=None):
                st8 = {}
                def mk(th, j):
                    def f():
                        gny = bctx["gn"][1]
                        obp = ob // 2
                        if bctx["k8"][obp] is None:
                            bctx["k8"][obp] = pk.tile([128, 2, T], FP8, tag="k", name="k_obp")
                        k8p = bctx["k8"][obp]
                        if j == 0:
                            st8[th] = (ps_mm.tile([128, 512], F32, tag="mm", name="psu")
                                       if pool is None else
                                       pool.tile([128, 512], F32, tag="sc", name="psu"))
                        psk = st8[th]
                        nc.tensor.matmul(
                            psk,
                            wk_sb[:, 2 * j:2 * j + 2, ob * 128:(ob + 1) * 128],
                            gny[:, 2 * j:2 * j + 2, th * 512:(th + 1) * 512],
                            start=(j == 0), stop=(j == 1), perf_mode=PM.DoubleRow)
                        if j == 1:
                            nc.scalar.activation(
                                out=k8p[:, ob % 2, th * 512:(th + 1) * 512],
                                in_=psk, func=AF.Identity,
                                bias=bk_sb[:, ob:ob + 1], scale=1.0 / WSCALE)
                    return f
                return [mk(th, j) for th in range(NT) for j in range(2)]

            def vt_unit(bctx, tt):
                """Closure list: 2 DR matmuls + bias for one vT seq tile.

                vT tiles hold an st-PAIR: [128, 2, NH, CH+1] bf16."""
                st8 = {}
                pair, sub = tt // 2, tt % 2
                def mk(j):
                    def f():
                        gny = bctx["gn"][1]
                        if j == 0:
                            st8["ps"] = ps_mm.tile([128, 512], F32, tag="mm", name="psu")
                        psv = st8["ps"]
                        nc.tensor.matmul(
                            psv,
                            gny[:, 2 * j:2 * j + 2, tt * 128:(tt + 1) * 128],
                            wv_sb[:, 2 * j:2 * j + 2, :],
                            start=(j == 0), stop=(j == 1), perf_mode=PM.DoubleRow)
                        if j == 1:
                            if bctx["vtp"][pair] is None:
                                bctx["vtp"][pair] = pvt.tile(
                                    [128, 2, NH, CH + 1], BF16, tag="vt", name="vt")
                            vt = bctx["vtp"][pair]
                            nc.vector.tensor_tensor(
                                out=vt[:, sub, :, 0:CH],
                                in0=psv.rearrange("p (h c) -> p h c", h=NH),
                                in1=bv_sb, op=OP.add)
                            nc.vector.tensor_copy(
                                vt[:, sub, :, CH:CH + 1],
                                ones16_sb.rearrange("p (h o) -> p h o", o=1))
                    return f
                return [mk(j) for j in range(2)]

            def pproj_unit(bctx, b, ob, pool=None, split_tail=False, store_q=None,
                           pool_tag="sc", act_resid=False):
                """Closure list: 4 DR matmuls + bias/residual/store for one out block.

                split_tail: j0 group (kb0-1, residual applied early), then a
                separate j1 group + final add + store — shortens the critical
                chain behind the last attention block.
                """
                st8 = {}
                def mk(th, j):
                    def f():
                        xr = bctx["x"]
                        a_sb = bctx["a"]
                        sl = slice(th * 512, (th + 1) * 512)
                        if j == 0 or split_tail:
                            st8[th] = (ps_mm.tile([128, 512], F32, tag="mm", name="psu")
                                       if pool is None else
                                       pool.tile([128, 512], F32, tag=pool_tag, name="psu"))
                        psh = st8[th]
                        nc.tensor.matmul(
                            psh,
                            wp_sb[:, 2 * j:2 * j + 2, ob * 128:(ob + 1) * 128],
                            a_sb[:, 2 * j:2 * j + 2, sl],
                            start=(j == 0 or split_tail),
                            stop=(j == 1 or split_tail),
                            perf_mode=PM.DoubleRow)
                        grp_end = (j == 0) if split_tail else (j == 1)
                        if grp_end:
                            nc.vector.scalar_tensor_tensor(
                                out=xr[:, ob, sl],
                                in0=psh, scalar=bp_sb[:, ob:ob + 1],
                                in1=xr[:, ob, sl], op0=OP.add, op1=OP.add)
                        if j == 1:
                            if split_tail:
                                nc.vector.tensor_tensor(
                                    out=xr[:, ob, sl], in0=psh, in1=xr[:, ob, sl],
                                    op=OP.add)
                            (store_q or nc.sync).dma_start(
                                out=out_d[b].rearrange("(cb p) t -> p cb t", p=128)[:, ob, sl],
                                in_=xr[:, ob, sl])
                    return f
                return [mk(th, j) for th in range(NT) for j in range(2)]

            def attention_head(bctx, ob, hh, qp, a_sb, lazy_vt=False, filler=None):
                """Emit one head's QK/exp slots; AV matmuls trail by one
                st-pair ACROSS head boundaries (bctx["pend"]), so the PE never
                waits on a just-issued exp and the ACT/DVE exp chains continue
                seamlessly from head to head."""
                h = 2 * ob + hh
                k8p = bctx["k8"][ob // 2]
                vtp = bctx["vtp"]
                psa_t = [ps_a0.tile([128, 512], F32, tag="av0", name="psa0"),
                         ps_a1.tile([128, 512], F32, tag="av1", name="psa1")]
                psa = [t[:, 0:4 * (CH + 1)].rearrange("p (a b) -> p a b", b=CH + 1)
                       for t in psa_t]
                bctx.setdefault("pend", [])

                def emit_avs(ctx):
                    hp, pair, wts, psa_p, a_sb_p = ctx
                    first = pair == 0
                    last = pair == NP - 1
                    for th in range(NT):
                        for sub in range(2):
                            for tc4 in range(4):
                                nc.tensor.matmul(
                                    psa_p[th][:, tc4, :],
                                    wts[:, sub, th, tc4 * 128:(tc4 + 1) * 128],
                                    vtp[pair][:, sub, hp, :],
                                    start=(first and sub == 0 and tc4 == 0),
                                    stop=(last and sub == 1 and tc4 == 3),
                                    skip_group_check=True)
                        if filler is not None:
                            filler()
                            filler()
                    if last:
                        finish_head(bctx, (hp // 2), hp % 2, psa_p, a_sb_p)

                hg = bctx["b"] * NH + h
                for pair in range(NP):
                    if lazy_vt and vtp[pair] is None:
                        for tt in (2 * pair, 2 * pair + 1):
                            for f in vt_unit(bctx, tt):
                                f()
                    wts = pwts.tile([128, 2, NT, 512], BF16, tag="wts")
                    for sub in range(2):
                        st = 2 * pair + sub
                        # half-width score tiles, 4 psum chains: each engine
                        # owns two (st-parity x th), so one chain's exp can
                        # run while the other waits its QK round-trip
                        for th in range(NT):
                            pss = ps_s.tile([128, 512], F32, tag="sc")
                            nc.tensor.matmul(
                                pss,
                                k8p[:, :, st * 128:(st + 1) * 128],
                                q8_sb[h][:, :, th * 512:(th + 1) * 512],
                                start=True, stop=True, perf_mode=PM.DoubleRow)
                            if filler is not None:
                                filler()
                            if sub == 1 and hg not in ACT_ODD_HEADS:
                                nc.vector.tensor_scalar(
                                    out=wts.bitcast(I16)[:, sub, th, :],
                                    in0=pss,
                                    scalar1=SCH_A, scalar2=SCH_C,
                                    op0=OP.mult, op1=OP.add)
                            else:
                                nc.scalar.activation(
                                    out=wts[:, sub, th, :],
                                    in_=pss,
                                    func=AF.Exp, bias=nbias_sb, scale=1.0)
                    bctx["pend"].append((h, pair, wts, psa, a_sb))
                    if len(bctx["pend"]) > 1:
                        emit_avs(bctx["pend"].pop(0))

            def attention_flush(bctx, filler=None):
                """Drain the trailing AV slot at batch end."""
                for ctx in bctx["pend"]:
                    hp, pair, wts, psa_p, a_sb_p = ctx
                    for th in range(NT):
                        for sub in range(2):
                            for tc4 in range(4):
                                nc.tensor.matmul(
                                    psa_p[th][:, tc4, :],
                                    wts[:, sub, th, tc4 * 128:(tc4 + 1) * 128],
                                    bctx["vtp"][pair][:, sub, hp, :],
                                    start=(pair == 0 and sub == 0 and tc4 == 0),
                                    stop=(pair == NP - 1 and sub == 1 and tc4 == 3),
                                    skip_group_check=True)
                    if pair == NP - 1:
                        finish_head(bctx, hp // 2, hp % 2, psa_p, a_sb_p)
                bctx["pend"] = []

            def finish_head(bctx, ob, hh, psa, a_sb):
                # denominators are per-partition columns now: copy+recip+scale.
                # high priority: these free the single-buffered AV psum banks,
                # so they must win DVE scheduling ties against filler ops.
                with tc.high_priority():
                    _finish_head(bctx, ob, hh, psa, a_sb)

            def _finish_head(bctx, ob, hh, psa, a_sb):
                h = 2 * ob + hh
                aT = bctx["aT"]
                for th in range(NT):
                    rr = pdn3.tile([128, 4], F32, tag="dn3")
                    if os.environ.get("KERNEL_DN_COPY") == "1":
                        dn = pdn.tile([128, 4], F32, tag="r0")
                        nc.vector.tensor_copy(dn, psa[th][:, :, CH])
                        nc.vector.reciprocal_approx_fast(out=rr, in_=dn)
                    else:
                        nc.vector.reciprocal_approx_fast(out=rr, in_=psa[th][:, :, CH])
                    nc.vector.tensor_tensor(
                        out=aT[:, th, :, h, :],
                        in0=psa[th][:, :, 0:CH],
                        in1=rr[:, :, None].broadcast_to([128, 4, CH]),
                        op=OP.mult)

            def finish_ob(bctx, ob, a_sb):
                """Transpose aT[t, c] blocks of channel-block ob back to a[c, t]."""
                aT = bctx["aT"]
                trp = ps_mm.tile([128, ST, 128], BF16, tag="mm", name="trp")
                for tb in range(ST):
                    th, tc4 = tb // 4, tb % 4
                    nc.tensor.matmul(
                        trp[:, tb, :],
                        aT[:, th, tc4, 2 * ob:2 * ob + 2, :].rearrange("p a b -> p (a b)"),
                        id_sb,
                        is_transpose=True)
                nc.scalar.activation(out=a_sb[:, ob, :],
                                     in_=trp.rearrange("p a b -> p (a b)"),
                                     func=AF.Copy)

            # ---------------- batch pipeline ----------------
            from collections import deque
            fillers = deque()

            def filler_pop():
                if fillers:
                    fillers.popleft()()

            def filler_flush():
                while fillers:
                    fillers.popleft()()

            bctxs = [dict() for _ in range(BPC)]
            _xy0 = emit_input_loads(0, mid_fn=emit_gn_consts)
            for wi in range(20):
                trash = ps_mm.tile([128, 64], F32, tag="mm", name="wtr0")
                nc.tensor.matmul(trash, _xy0[0][:, 0, 0:128], _xy0[0][:, 0, 0:64],
                                 start=True, stop=True)
            emit_bias_consts()
            nc.sync.dma_start(out=wq_sb, in_=wqt.rearrange("(kb p) o -> p kb o", p=128))
            nc.sync.dma_start(out=wk_sb, in_=wkt.rearrange("(kb p) o -> p kb o", p=128))
            emit_vp_weight_loads()
            bctxs[0]["x"] = _xy0[0]
            bctxs[0]["gn"] = emit_gn_compute(*_xy0, fast_apply=True, act_stats=True)
            bctxs[0]["k8"] = [None] * (CB // 2)
            bctxs[0]["vtp"] = [None] * NP

            # batch 0 ob0 prep emitted directly; rest queued as fillers that
            # drip into the attention pair slots (1 matmul per QK/AV pair)
            for f in k_unit(bctxs[0], 0, pool=ps_s):
                f()
            for f in q_unit(bctxs[0], 0, pool=ps_s):
                f()
            for ob2 in (1, 2, 3):
                fillers.extend(k_unit(bctxs[0], ob2))
                fillers.extend(q_unit(bctxs[0], ob2))

            for b in range(BPC):
                bctx = bctxs[b]
                bctx["b"] = b
                a_sb = pa.tile([128, CB, T], FP8, tag="a")
                bctx["a"] = a_sb
                bctx["aT"] = pat.tile([128, NT, 4, NH, CH], BF16, tag="aT", name="aT")
                for ob in range(CB):
                    if b > 0:
                        if ob == 0:
                            for ob2 in (1, 2):
                                fillers.extend(k_unit(bctx, ob2))
                                fillers.extend(q_unit(bctx, ob2))
                        if ob == 1:
                            fillers.extend(k_unit(bctx, 3))
                            fillers.extend(q_unit(bctx, 3))
                            for ob2 in range(CB):
                                fillers.extend(pproj_unit(bctxs[b - 1], b - 1, ob2, pool=ps_s))
                    if b + 1 < BPC:
                        if ob == 0:
                            nb = bctxs[b + 1]
                            nb["xy"] = emit_input_loads(b + 1)
                            nb["x"] = nb["xy"][0]
                        if ob == 1:
                            nb = bctxs[b + 1]
                            nb["gn"] = emit_gn_compute(*nb.pop("xy"))
                            nb["k8"] = [None] * (CB // 2)
                            nb["vtp"] = [None] * NP

                    for hh in (0, 1):
                        attention_head(bctx, ob, hh, None, a_sb,
                                       lazy_vt=(b == 0 and ob == 0),
                                       filler=filler_pop)
                        if hh == 0 and b + 1 < BPC:
                            nb = bctxs[b + 1]
                            if ob == 2:
                                fillers.extend(k_unit(nb, 0))
                                for tt in range(ST):
                                    fillers.extend(vt_unit(nb, tt))
                            if ob == 3:
                                fillers.extend(q_unit(nb, 0))
                    if ob > 0:
                        finish_ob(bctx, ob - 1, a_sb)
                        if b == BPC - 1:
                            fillers.extend(pproj_unit(bctx, b, ob - 1, pool=ps_s))
                    if ob == CB - 1:
                        attention_flush(bctx)
                        finish_ob(bctx, ob, a_sb)

                if b == BPC - 1:
                    # tail: drain queue, then the last output projection (ob3
                    # only; obs 0-2 were emitted as fillers after finish_ob).
                    filler_flush()
                    for wi in range(4):
                        trash = ps_mm.tile([128, 512], F32, tag="mm", name="wtr")
                        nc.tensor.matmul(trash, wp_sb[:, 0, 0:128],
                                         a_sb[:, 0, 0:512],
                                         start=True, stop=True)
                    for f in pproj_unit(bctx, b, CB - 1, pool=ps_a0, pool_tag="av0",
                                        split_tail=True, store_q=nc.scalar):
                        f()

    nc.finalize()
    return nc


_NC = None


def _get_nc():
    global _NC
    if _NC is None:
        _NC = _build()
    return _NC


def _prep_inputs(x, y, gn_w, gn_b, Wq, bq, Wkv, bkv, Wp, bp):
    scale = CH ** -0.25
    # reference splits k/v per head: kvh[:, h, :ch] / kvh[:, h, ch:] after
    # reshape to [b, NH, 2*ch, T] -> k_h = Wkv rows [h*128, h*128+64)
    import ml_dtypes
    FP8NP = ml_dtypes.float8_e4m3
    idx_k = np.concatenate([np.arange(h * 2 * CH, h * 2 * CH + CH) for h in range(NH)])
    idx_v = np.concatenate([np.arange(h * 2 * CH + CH, (h + 1) * 2 * CH) for h in range(NH)])
    # Wq/Wk prescaled by WSCALE to keep fp8 values out of denormal range;
    # compensated by 1/WSCALE in the psum->sbuf bias add.
    wqt = np.ascontiguousarray((Wq * (scale * WSCALE)).T).astype(FP8NP)
    wkt = np.ascontiguousarray((Wkv[idx_k] * (scale * WSCALE)).T).astype(FP8NP)
    wvt = np.ascontiguousarray(Wkv[idx_v].T).astype(FP8NP)
    wpt = np.ascontiguousarray(Wp.T).astype(FP8NP)
    bq_s = bq * scale
    bk_s = bkv[idx_k] * scale
    bv = bkv[idx_v]

    def part_layout(v):  # [C] -> [128, CB]: v[cb*128+p]
        return np.ascontiguousarray(v.reshape(CB, 128).T)

    bq_l = part_layout(bq_s)
    bk_l = part_layout(bk_s)
    bp_l = part_layout(bp)
    gnw_l = part_layout(gn_w)
    gnb_l = part_layout(gn_b)
    bv_bc = np.broadcast_to(bv.reshape(1, NH, CH), (128, NH, CH)).copy()
    m1 = np.zeros((128, 128), np.float32)
    for g in range(128 // GSIZE):
        m1[g * GSIZE:(g + 1) * GSIZE, g * GSIZE:(g + 1) * GSIZE] = 1.0 / GSIZE
    id128_h = np.eye(128, dtype=ml_dtypes.bfloat16)

    xf = x.reshape(B, C, T)
    yf = y.reshape(B, C, T)

    shared = {
        "wqt": wqt, "wkt": wkt, "wvt": wvt, "wpt": wpt,
        "bq_l": bq_l, "bk_l": bk_l, "bp_l": bp_l, "bv_bc": bv_bc,
        "gnw_l": gnw_l, "gnb_l": gnb_l, "m1": m1, "id128": id128_h,
    }
    in_maps = []
    for i in range(N_CORES):
        m = dict(shared)
        m["x2"] = np.ascontiguousarray(xf[i * BPC:(i + 1) * BPC])
        m["y2"] = np.ascontiguousarray(yf[i * BPC:(i + 1) * BPC])
        in_maps.append(m)
    return in_maps


def kernel(x, y, gn_w, gn_b, Wq, bq, Wkv, bkv, Wp, bp):
    args = [np.asarray(a, dtype=np.float32) for a in
            (x, y, gn_w, gn_b, Wq, bq, Wkv, bkv, Wp, bp)]
    in_maps = _prep_inputs(*args)
    nc = _get_nc()
    res = run_bass_kernel_spmd(nc, in_maps, core_ids=list(range(N_CORES)))
    out = np.empty((B, C, T), np.float32)
    for i in range(N_CORES):
        out[i * BPC:(i + 1) * BPC] = res.results[i]["out"]
    return out.reshape(B, C, H, W)
